# revision 1
# baseline (speedup 1.0000x reference)
"""CoLaKG model kernel for 8 Trainium2 NeuronCores (self-contained).

Pipeline (4 bass SPMD kernels; host does static prep + mechanical relayout only):
  K1 gemm : semantic projections (users+items) + merge, + s1/s2 GAT scalars
  K2 attn : item-neighbor GAT attention (indirect gather + softmax + wsum)
  K3 spmm : one LightGCN propagation layer (run 3x); dest-sharded PE segment-sum
  K4 final: gather 4 embedding tables at (user,item) rows, mean, dot product

Perf design (the pool/GPSIMD engine is the bottleneck: each 128-row
indirect gather costs ~1us of serial SWDGE time, so minimize gather calls):
 - wide 120-dest gather columns, 21 per subwindow + 2 extra-chunk
   columns per window (99.1% regular slot fill);
 - fp16 tables/weights everywhere (f32 psum accumulation);
 - layer 2 computed only at the ~87% of rows consumed downstream (k3b);
 - layer 3 computed only at the batch's 2*512 rows per core (k3c);
 - window tails trimmed; vectorized host packing; DMA-instruction counts
   kept below walrus' 16-bit semaphore wait-value limit.
"""
import copy
import numpy as np

import jax
jax.config.update("jax_compilation_cache_dir", "/tmp/.jax_bass_cache")
jax.config.update("jax_persistent_cache_min_entry_size_bytes", -1)
jax.config.update("jax_persistent_cache_min_compile_time_secs", 0.0)

import concourse.bass as bass
import concourse.mybir as mybir
from concourse.tile import TileContext
from concourse import bass_utils, library_config
import concourse.tile as tile_mod
from concourse.vector_clock import ScopedClock

F32 = mybir.dt.float32
F16 = mybir.dt.float16
I16 = mybir.dt.int16
I32 = mybir.dt.int32
AF = mybir.ActivationFunctionType

# ---------------------------------------------------------------- tile patch
MAX_WAITS = 1

def _split_sync_waits(nc, max_waits=MAX_WAITS):
    template = None
    counter = [0]
    for fn in nc.m.functions:
        for bb in fn.blocks:
            for inst in bb.instructions:
                if type(inst).__name__ == "InstNoOp":
                    template = copy.deepcopy(inst)
                    break
            if template is not None:
                break
        if template is not None:
            break
    for fn in nc.m.functions:
        for bb in fn.blocks:
            il = bb.instructions
            i = 0
            while i < len(il):
                inst = il[i]
                if template is None and type(inst).__name__ == "InstNoOp":
                    template = copy.deepcopy(inst)
                si = inst.sync_info
                if si is not None and si.on_wait is not None and len(si.on_wait) > max_waits:
                    assert template is not None, "no InstNoOp to clone"
                    waits = list(si.on_wait)
                    keep, rest = waits[:max_waits], waits[max_waits:]
                    si.on_wait.clear()
                    for w in keep:
                        si.on_wait.append(w)
                    carriers = []
                    while rest:
                        c = copy.deepcopy(template)
                        counter[0] += 1
                        c.name = f"I-waitsplit-{counter[0]}"
                        c.engine = inst.engine
                        c.sync_info = mybir.SyncInfo(on_wait=list(rest[:max_waits]), on_update=[])
                        carriers.append(c)
                        rest = rest[max_waits:]
                    for k, cinst in enumerate(carriers):
                        try:
                            nc.register_instruction(cinst, overwrite=True)
                        except Exception:
                            pass
                        il.insert(i + k, cinst)
                    i += len(carriers)
                i += 1

def _patched_drain_and_barrier(self, tick_clock, wait_clock):
    nc = self.nc
    nop0 = nc.sync.nop(nofuse=True, hint="predrain_waits")
    wait_clock.add_sem_waits(nop0.ins, ScopedClock({None: tick_clock.global_clock}))
    nc.sync.drain()
    nc.all_engine_barrier()
    assert self.sems is not None
    popped = nc._tile_sem_poison_stack.pop()
    assert popped is self._sem_poison
    nc.clear_and_free_semaphores(list(self.sems.allocated().values()))
    nc.all_engine_barrier()
    _split_sync_waits(nc)

tile_mod.TileContext._drain_and_barrier = _patched_drain_and_barrier

# ---------------------------------------------------------------- constants
NUM_USERS, NUM_ITEMS, D, SEM, HID, K = 60000, 30000, 64, 1024, 32, 32
N = NUM_USERS + NUM_ITEMS             # 90000
NPAD = 90112                          # 704*128
NCORE = 8
DPC = NPAD // NCORE                   # 11264 dest rows per core
SW = 120                              # dest rows per wide gather column
CPS = 21                              # regular columns per subwindow (cap 2688)
WIN = 480                             # psum cols per window
SPW = WIN // SW                       # 4 subwindows per window
CW = SPW * CPS                        # 84 regular columns per window
NEX = 2                               # extra-chunk columns per window
CH = CW + NEX
NWIN = 24                             # 24*480 = 11520 >= 11264
NSUB_CORE = (DPC + SW - 1) // SW      # 188 subwindows per core
SLOT = 128
UPC = NUM_USERS // NCORE              # 7500
IPC = NUM_ITEMS // NCORE              # 3750
IPAD = 3840
NBLK = IPAD // 128                    # 30
SUBB = 3                              # item blocks per attention sub-batch
NSUB = NBLK // SUBB                   # 10
GC = SUBB * K                         # 96 gather cols per K2 sub-batch
B = 4096
BPC = B // NCORE                      # 512

_BUILT = {}


def _elu(nc, pool, out_ap, in_ap, shape, tag):
    """out = elu(in) = max(x,0) + exp(min(x,0)) - 1   (no Elu in ACT table)."""
    mn = pool.tile(shape, F32, tag=tag + "_mn")
    nc.vector.tensor_scalar_min(mn[:], in_ap, 0.0)
    ex = pool.tile(shape, F32, tag=tag + "_ex")
    nc.scalar.activation(ex[:], mn[:], AF.Exp, scale=1.0)
    mx = pool.tile(shape, F32, tag=tag + "_mx")
    nc.vector.tensor_scalar_max(mx[:], in_ap, 0.0)
    nc.vector.tensor_add(out_ap, mx[:], ex[:])
    nc.vector.tensor_scalar_add(out_ap, out_ap, -1.0)


# ================================================================ K1: GEMM
def _build_k1():
    nc = bass.Bass("TRN2", target_bir_lowering=False)
    xu = nc.dram_tensor("xu", [SEM, UPC], F16, kind="ExternalInput")
    xi = nc.dram_tensor("xi", [SEM, IPC], F16, kind="ExternalInput")
    wu = nc.dram_tensor("wu", [SEM, 64], F16, kind="ExternalInput")
    wi = nc.dram_tensor("wi", [SEM, 66], F16, kind="ExternalInput")
    bu = nc.dram_tensor("bu", [64, 1], F32, kind="ExternalInput")
    bi = nc.dram_tensor("bi", [66, 1], F32, kind="ExternalInput")
    eu = nc.dram_tensor("eu", [64, UPC], F32, kind="ExternalInput")
    ei = nc.dram_tensor("ei", [64, IPC], F32, kind="ExternalInput")
    ou = nc.dram_tensor("ou", [64, UPC], F32, kind="ExternalOutput")
    oi = nc.dram_tensor("oi", [64, IPC], F32, kind="ExternalOutput")
    os12 = nc.dram_tensor("os12", [2, IPC], F32, kind="ExternalOutput")

    with TileContext(nc) as tc:
        with tc.tile_pool(name="w", bufs=1) as wp, \
             tc.tile_pool(name="x", bufs=3) as xp, \
             tc.tile_pool(name="o", bufs=2) as op, \
             tc.tile_pool(name="ps", bufs=2, space="PSUM") as pp:
            wu_sb = wp.tile([128, SEM // 128, 64], F16, tag="wu")
            nc.sync.dma_start(wu_sb[:], wu[:].rearrange("(a p) m -> p a m", p=128))
            wi_sb = wp.tile([128, SEM // 128, 66], F16, tag="wi")
            nc.sync.dma_start(wi_sb[:], wi[:].rearrange("(a p) m -> p a m", p=128))
            bu_sb = wp.tile([64, 1], F32, tag="bu")
            nc.sync.dma_start(bu_sb[:], bu[:])
            bi_sb = wp.tile([66, 1], F32, tag="bi")
            nc.sync.dma_start(bi_sb[:], bi[:])

            def gemm(xten, eten, wtile, btile, oten, m, rows, R, s12=None):
                for t in range(rows // R):
                    xt = xp.tile([128, SEM // 128, R], F16, tag="xt")
                    nc.sync.dma_start(
                        xt[:], xten[:, t * R:(t + 1) * R].rearrange("(a p) r -> p a r", p=128))
                    ps = pp.tile([m, R], F32, tag="ps")
                    for kk in range(SEM // 128):
                        nc.tensor.matmul(ps[:], wtile[:, kk, :], xt[:, kk, :],
                                         start=(kk == 0), stop=(kk == SEM // 128 - 1))
                    xb = op.tile([64, R], F32, tag="xb")
                    nc.vector.tensor_scalar_add(xb[:], ps[0:64, :], btile[0:64, :])
                    mg = op.tile([64, R], F32, tag="mg")
                    _elu(nc, op, mg[:], xb[:], [64, R], "e1")
                    et = op.tile([64, R], F32, tag="et")
                    nc.sync.dma_start(et[:], eten[:, t * R:(t + 1) * R])
                    nc.vector.tensor_add(mg[:], mg[:], et[:])
                    nc.scalar.mul(mg[:], mg[:], 0.5)
                    nc.sync.dma_start(oten[:, t * R:(t + 1) * R], mg[:])
                    if s12 is not None:
                        sv = op.tile([2, R], F32, tag="sv")
                        nc.scalar.copy(sv[:], ps[64:66, :])
                        nc.sync.dma_start(s12[:, t * R:(t + 1) * R], sv[:])

            gemm(xu, eu, wu_sb, bu_sb, ou, 64, UPC, 500)
            gemm(xi, ei, wi_sb, bi_sb, oi, 66, IPC, 375, s12=os12)
    return nc


# ================================================================ K2: attention
TBLW = 68                             # 64 emb + s1 + pad (136B fp16 rows)

def _build_k2():
    nc = bass.Bass("TRN2", target_bir_lowering=False)
    tbl = nc.dram_tensor("tbl", [NUM_ITEMS, TBLW], F16, kind="ExternalInput")
    adji = nc.dram_tensor("adji", [128, NBLK * K], I32, kind="ExternalInput")
    s2r = nc.dram_tensor("s2r", [128, NBLK * K], F32, kind="ExternalInput")
    itm = nc.dram_tensor("itm", [128, NBLK * 64], F32, kind="ExternalInput")
    oit = nc.dram_tensor("oit", [128, NBLK * 64], F16, kind="ExternalOutput")

    with TileContext(nc) as tc:
        with tc.tile_pool(name="g", bufs=2) as gp, \
             tc.tile_pool(name="t", bufs=2) as tp, \
             tc.tile_pool(name="s", bufs=1) as sp, \
             tc.tile_pool(name="m", bufs=2) as mp:
            adj_sb = sp.tile([128, NBLK * K], I32, tag="adj")
            nc.sync.dma_start(adj_sb[:], adji[:])
            s2_sb = sp.tile([128, NBLK * K], F32, tag="s2")
            nc.sync.dma_start(s2_sb[:], s2r[:])
            itm_sb = sp.tile([128, NBLK * 64], F32, tag="itm")
            nc.sync.dma_start(itm_sb[:], itm[:])
            for u in range(NSUB):
                g = gp.tile([128, GC, TBLW], F16, tag="g")
                for j in range(GC):
                    nc.gpsimd.indirect_dma_start(
                        out=g[:, j, :], out_offset=None, in_=tbl[:],
                        in_offset=bass.IndirectOffsetOnAxis(
                            ap=adj_sb[:, u * GC + j:u * GC + j + 1], axis=0))
                s1f = mp.tile([128, GC], F32, tag="s1f")
                nc.scalar.copy(s1f[:], g[:, :, 64])
                lg = mp.tile([128, GC], F32, tag="lg")
                nc.vector.tensor_add(lg[:], s1f[:], s2_sb[:, u * GC:(u + 1) * GC])
                lr = mp.tile([128, GC], F32, tag="lr")
                nc.scalar.mul(lr[:], lg[:], 0.2)
                nc.vector.tensor_max(lg[:], lg[:], lr[:])
                ex = mp.tile([128, SUBB, K], F32, tag="ex")
                nc.scalar.activation(ex[:].rearrange("p a b -> p (a b)"), lg[:],
                                     AF.Exp, scale=1.0)
                sm = mp.tile([128, SUBB], F32, tag="sm")
                nc.vector.reduce_sum(sm[:], ex[:], axis=mybir.AxisListType.X)
                nc.vector.reciprocal(sm[:], sm[:])
                att = mp.tile([128, SUBB, K], F16, tag="att")
                for bb in range(SUBB):
                    nc.vector.tensor_scalar_mul(att[:, bb, :], ex[:, bb, :], sm[:, bb:bb + 1])
                tmp = tp.tile([128, SUBB, K, 64], F16, tag="tmp")
                av = att[:]
                att_b = bass.AP(av.tensor, av.offset, list(av.ap) + [[0, 64]])
                nc.vector.tensor_mul(
                    tmp[:], g[:].rearrange("p (b k) d -> p b k d", b=SUBB)[:, :, :, 0:64],
                    att_b)
                hp = mp.tile([128, SUBB, 64], F32, tag="hp")
                nc.vector.reduce_sum(hp[:], tmp[:].rearrange("p b k d -> p b d k"),
                                     axis=mybir.AxisListType.X)
                he = mp.tile([128, SUBB * 64], F32, tag="he")
                _elu(nc, mp, he[:], hp[:].rearrange("p b d -> p (b d)"),
                     [128, SUBB * 64], "e2")
                fo = mp.tile([128, SUBB * 64], F32, tag="fo")
                nc.vector.tensor_add(fo[:], he[:],
                                     itm_sb[:, u * SUBB * 64:(u + 1) * SUBB * 64])
                fo16 = mp.tile([128, SUBB * 64], F16, tag="fo16")
                nc.scalar.mul(fo16[:], fo[:], 0.5)
                nc.sync.dma_start(oit[:, u * SUBB * 64:(u + 1) * SUBB * 64], fo16[:])
    return nc


# ================================================================ K3: spmm layer
def _spmm_body(nc, tbl, idx, wreg, wext, out, nwin, nsub_of):
    """Shared SpMM kernel body: wide 60-dest gather columns, 11 per subwindow,
    fp16 table/weights, f32 psum accumulation."""
    with TileContext(nc) as tc:
        with tc.tile_pool(name="s", bufs=1) as sp, \
             tc.tile_pool(name="g", bufs=3) as gp, \
             tc.tile_pool(name="w", bufs=3) as wp, \
             tc.tile_pool(name="o", bufs=3) as op, \
             tc.tile_pool(name="ps", bufs=4, space="PSUM") as pp:
            idx_sb = sp.tile([128, nwin * CH], I32, tag="idx")
            nc.sync.dma_start(idx_sb[:], idx[:])
            for w in range(nwin):
                nsub = nsub_of(w)
                ncol = nsub * CPS
                wr = wp.tile([128, CW * SW], F16, tag="wr")
                nc.sync.dma_start(wr[:], wreg[:, w * CW * SW:(w + 1) * CW * SW])
                we = wp.tile([128, NEX, WIN], F16, tag="we")
                nc.sync.dma_start(we[:].rearrange("p a b -> p (a b)"),
                                  wext[:, w * NEX * WIN:(w + 1) * NEX * WIN])
                gt = gp.tile([128, CH, 64], F16, tag="g")
                for j in list(range(ncol)) + [CW, CW + 1]:
                    nc.gpsimd.indirect_dma_start(
                        out=gt[:, j, :], out_offset=None, in_=tbl[:],
                        in_offset=bass.IndirectOffsetOnAxis(
                            ap=idx_sb[:, w * CH + j:w * CH + j + 1], axis=0))
                ps = pp.tile([64, WIN], F32, tag="ps")
                pse = pp.tile([64, WIN], F32, tag="pse")
                nc.tensor.matmul(pse[:], gt[:, CW, :], we[:, 0, :],
                                 start=True, stop=False)
                nc.tensor.matmul(pse[:], gt[:, CW + 1, :], we[:, 1, :],
                                 start=False, stop=True)
                for sc in range(nsub):
                    for k in range(CPS):
                        col = sc * CPS + k
                        nc.tensor.matmul(
                            ps[:, sc * SW:(sc + 1) * SW],
                            gt[:, col, :],
                            wr[:, col * SW:(col + 1) * SW],
                            start=(k == 0), stop=(k == CPS - 1))
                ot32 = op.tile([64, WIN], F32, tag="ot32")
                nc.scalar.copy(ot32[:], ps[:])
                ot = op.tile([64, WIN], F16, tag="ot")
                nc.vector.tensor_add(ot[:], ot32[:], pse[:])
                nc.sync.dma_start(out[:, w * WIN:(w + 1) * WIN], ot[:])


def _build_k3():
    nc = bass.Bass("TRN2", target_bir_lowering=False)
    tbl = nc.dram_tensor("tbl", [NPAD, 64], F16, kind="ExternalInput")
    idx = nc.dram_tensor("idx", [128, NWIN * CH], I32, kind="ExternalInput")
    wreg = nc.dram_tensor("wreg", [128, NWIN * CW * SW], F16, kind="ExternalInput")
    wext = nc.dram_tensor("wext", [128, NWIN * NEX * WIN], F16, kind="ExternalInput")
    out = nc.dram_tensor("out", [64, NWIN * WIN], F16, kind="ExternalOutput")
    _spmm_body(nc, tbl, idx, wreg, wext, out, NWIN,
               lambda w: SPW if w < NWIN - 1 else NSUB_CORE - (NWIN - 1) * SPW)
    return nc


# ====================================================== K3c: compact layer 3
# Layer 3 output is only consumed by K4 at the batch's (user, item) rows, so
# each core computes just its 2*BPC = 1024 destination rows (dest list =
# [users_c, NUM_USERS + items_c]), packed with the same wide-column scheme.
NDST3 = 2 * BPC                       # 1024 dest list positions per core
NSUB3 = (NDST3 + SW - 1) // SW        # 18 subwindows
NWIN3 = (NSUB3 + SPW - 1) // SPW      # 3 windows

# Layer 2 is only consumed at rows feeding the compact layer 3 (sources of
# its edges) and K4's fidx rows (~87% of the table) -> list-addressed pass.
L2 = 9840                             # padded per-core layer-2 dest list len
NSUB2 = (L2 + SW - 1) // SW           # 164 subwindows
NWIN2 = (NSUB2 + SPW - 1) // SPW      # 21 windows

def _build_k3b():
    nc = bass.Bass("TRN2", target_bir_lowering=False)
    tbl = nc.dram_tensor("tbl", [NPAD, 64], F16, kind="ExternalInput")
    idx = nc.dram_tensor("idx", [128, NWIN2 * CH], I32, kind="ExternalInput")
    wreg = nc.dram_tensor("wreg", [128, NWIN2 * CW * SW], F16, kind="ExternalInput")
    wext = nc.dram_tensor("wext", [128, NWIN2 * NEX * WIN], F16, kind="ExternalInput")
    out = nc.dram_tensor("out", [64, NWIN2 * WIN], F16, kind="ExternalOutput")
    _spmm_body(nc, tbl, idx, wreg, wext, out, NWIN2,
               lambda w: min(SPW, NSUB2 - w * SPW))
    return nc


def _build_k3c():
    nc = bass.Bass("TRN2", target_bir_lowering=False)
    tbl = nc.dram_tensor("tbl", [NPAD, 64], F16, kind="ExternalInput")
    idx = nc.dram_tensor("idx", [128, NWIN3 * CH], I32, kind="ExternalInput")
    wreg = nc.dram_tensor("wreg", [128, NWIN3 * CW * SW], F16, kind="ExternalInput")
    wext = nc.dram_tensor("wext", [128, NWIN3 * NEX * WIN], F16, kind="ExternalInput")
    out = nc.dram_tensor("out", [64, NWIN3 * WIN], F16, kind="ExternalOutput")
    _spmm_body(nc, tbl, idx, wreg, wext, out, NWIN3,
               lambda w: min(SPW, NSUB3 - w * SPW))
    return nc


# ================================================================ K4: final
def _build_k4():
    nc = bass.Bass("TRN2", target_bir_lowering=False)
    # e0|e1|e2 interleaved row-wise so each batch row is gathered once
    tb012 = nc.dram_tensor("tb012", [NPAD, 192], F16, kind="ExternalInput")
    tb3d = nc.dram_tensor("tb3d", [128, 8 * 64], F16, kind="ExternalInput")
    fidx = nc.dram_tensor("fidx", [128, 8], I32, kind="ExternalInput")
    out = nc.dram_tensor("out", [128, 4], F32, kind="ExternalOutput")

    with TileContext(nc) as tc:
        with tc.tile_pool(name="g", bufs=2) as gp, \
             tc.tile_pool(name="m", bufs=1) as mp:
            it = mp.tile([128, 8], I32, tag="it")
            nc.sync.dma_start(it[:], fidx[:])
            t3 = mp.tile([128, 8, 64], F16, tag="t3")
            nc.sync.dma_start(t3[:].rearrange("p a b -> p (a b)"), tb3d[:])
            acc = mp.tile([128, 8, 64], F32, tag="acc")
            nc.scalar.copy(acc[:], t3[:])
            g = gp.tile([128, 8, 192], F16, tag="g")
            for t in range(8):
                nc.gpsimd.indirect_dma_start(
                    out=g[:, t, :], out_offset=None, in_=tb012[:],
                    in_offset=bass.IndirectOffsetOnAxis(ap=it[:, t:t + 1], axis=0))
            for sl in range(3):
                gf = gp.tile([128, 8, 64], F32, tag="gf")
                nc.scalar.copy(gf[:], g[:].rearrange("p a (s b) -> p a s b", s=3)[:, :, sl, :])
                nc.vector.tensor_add(acc[:], acc[:], gf[:])
            nc.scalar.mul(acc[:], acc[:], 0.25)
            prod = mp.tile([128, 4, 64], F32, tag="prod")
            nc.vector.tensor_mul(prod[:], acc[:, 0:4, :], acc[:, 4:8, :])
            res = mp.tile([128, 4], F32, tag="res")
            nc.vector.reduce_sum(res[:], prod[:], axis=mybir.AxisListType.X)
            nc.sync.dma_start(out[:], res[:])
    return nc


# ================================================================ host packing
def _pack_spmm(rows, cols, vals):
    """Vectorized packing of the COO graph into wide-column gather grids."""
    idx_arr = np.zeros((NCORE, 128, NWIN * CH), np.int32)
    wreg = np.zeros((NCORE, 128, NWIN * CW * SW), np.float32)
    wext = np.zeros((NCORE, 128, NWIN * NEX * WIN), np.float32)

    c = rows // DPC
    rl = rows - c * DPC
    sub = rl // SW
    w = sub // SPW
    sc = sub - w * SPW
    dloc = rl - sub * SW
    gid = c * (NWIN * SPW) + sub
    order = np.argsort(gid, kind="stable")
    gid_s = gid[order]
    n = len(order)
    ar = np.arange(n)
    first = np.empty(n, bool); first[0] = True; first[1:] = gid_s[1:] != gid_s[:-1]
    gstart = np.maximum.accumulate(np.where(first, ar, 0))
    rank = ar - gstart
    c, rl, w, sc, dloc = (x[order] for x in (c, rl, w, sc, dloc))
    co = cols[order]; v = vals[order]

    k = rank >> 7
    slot = rank & 127
    reg = k < CPS
    col = sc * CPS + k
    fi = (c * 128 + slot) * (NWIN * CH) + w * CH + col
    idx_arr.reshape(-1)[fi[reg]] = co[reg]
    fw = (c * 128 + slot) * (NWIN * CW * SW) + w * (CW * SW) + col * SW + dloc
    wreg.reshape(-1)[fw[reg]] = v[reg]

    ex = ~reg
    if ex.any():
        ce, we_, rle, coe, ve = c[ex], w[ex], rl[ex], co[ex], v[ex]
        key = ce * NWIN + we_
        ne = len(key)
        are = np.arange(ne)
        kf = np.empty(ne, bool); kf[0] = True; kf[1:] = key[1:] != key[:-1]
        ks = np.maximum.accumulate(np.where(kf, are, 0))
        er = are - ks
        assert er.max() < NEX * 128, f"extra chunk overflow: {er.max()}"
        plane = er >> 7
        slot = er & 127
        fie = (ce * 128 + slot) * (NWIN * CH) + we_ * CH + CW + plane
        idx_arr.reshape(-1)[fie] = coe
        off = rle - we_ * WIN
        fxe = (ce * 128 + slot) * (NWIN * NEX * WIN) + we_ * (NEX * WIN) + plane * WIN + off
        wext.reshape(-1)[fxe] = ve
    return idx_arr, wreg.astype(np.float16), wext.astype(np.float16)


def _pack_list(rows, cols, vals, dlist, nwin):
    """Pack edges for a list-addressed SpMM pass: per core, destination d is
    position d of dlist[c] (padded positions use an edge-less row)."""
    ndst = dlist.shape[1]
    idx_arr = np.zeros((NCORE, 128, nwin * CH), np.int32)
    wreg = np.zeros((NCORE, 128, nwin * CW * SW), np.float32)
    wext = np.zeros((NCORE, 128, nwin * NEX * WIN), np.float32)
    order0 = np.argsort(rows, kind="stable")
    rs, cs, vs = rows[order0], cols[order0], vals[order0]
    row_start = np.searchsorted(rs, np.arange(N + 1))
    rr = dlist.reshape(-1).astype(np.int64)
    rrc = np.minimum(rr, N)
    cnt = row_start[np.minimum(rrc + 1, N)] - row_start[rrc]
    cnt[rr >= N] = 0
    ent = np.repeat(np.arange(len(rr)), cnt)
    ofs = np.arange(len(ent)) - np.repeat(np.cumsum(cnt) - cnt, cnt)
    src = row_start[rrc][ent] + ofs
    e_c = ent // ndst
    e_d = ent - e_c * ndst
    e_co = cs[src]; e_v = vs[src]
    sub = e_d // SW
    w = sub // SPW
    sc = sub - w * SPW
    dloc = e_d - sub * SW
    gid = e_c * (nwin * SPW) + sub                      # already sorted (ent asc)
    n = len(gid)
    if n:
        ar = np.arange(n)
        first = np.empty(n, bool); first[0] = True; first[1:] = gid[1:] != gid[:-1]
        rank = ar - np.maximum.accumulate(np.where(first, ar, 0))
        k = rank >> 7
        slot = rank & 127
        reg = k < CPS
        col = sc * CPS + k
        fi = (e_c * 128 + slot) * (nwin * CH) + w * CH + col
        idx_arr.reshape(-1)[fi[reg]] = e_co[reg]
        fw = (e_c * 128 + slot) * (nwin * CW * SW) + w * (CW * SW) + col * SW + dloc
        wreg.reshape(-1)[fw[reg]] = e_v[reg]
        ex = ~reg
        if ex.any():
            ce, we_, de, coe, ve = e_c[ex], w[ex], e_d[ex], e_co[ex], e_v[ex]
            key = ce * nwin + we_
            ne = len(key)
            are = np.arange(ne)
            kf = np.empty(ne, bool); kf[0] = True; kf[1:] = key[1:] != key[:-1]
            ks = np.maximum.accumulate(np.where(kf, are, 0))
            er = are - ks
            assert er.max() < NEX * 128, f"list-pack extra overflow: {er.max()}"
            plane = er >> 7
            slot = er & 127
            fie = (ce * 128 + slot) * (nwin * CH) + we_ * CH + CW + plane
            idx_arr.reshape(-1)[fie] = coe
            off = de - we_ * WIN
            fxe = (ce * 128 + slot) * (nwin * NEX * WIN) + we_ * (NEX * WIN) + plane * WIN + off
            wext.reshape(-1)[fxe] = ve
    return idx_arr, wreg.astype(np.float16), wext.astype(np.float16)


def _prep(inputs):
    p = {}
    users = np.asarray(inputs["users"]);   items = np.asarray(inputs["items"])
    adj = np.asarray(inputs["adj_matrix"])
    rows = np.asarray(inputs["graph_rows"]).astype(np.int64)
    cols = np.asarray(inputs["graph_cols"]).astype(np.int64)
    vals = np.asarray(inputs["graph_vals"]).astype(np.float32)
    W_att = np.asarray(inputs["W_att"]); a_att = np.asarray(inputs["a_att"])
    v1 = W_att @ a_att[:HID, 0]; v2 = W_att @ a_att[HID:, 0]

    p["xu"] = np.ascontiguousarray(np.asarray(inputs["user_semantic_emb"]).astype(np.float16).T)
    p["xi"] = np.ascontiguousarray(np.asarray(inputs["semantic_emb"]).astype(np.float16).T)
    p["wu"] = np.asarray(inputs["W_usem"]).astype(np.float16)
    p["wi"] = np.concatenate([np.asarray(inputs["W_sem"]), v1[:, None], v2[:, None]],
                             axis=1).astype(np.float16)
    p["bu"] = np.asarray(inputs["b_usem"]).reshape(64, 1)
    p["bi"] = np.concatenate([np.asarray(inputs["b_sem"]), np.zeros(2, np.float32)]
                             ).reshape(66, 1).astype(np.float32)
    p["eu"] = np.ascontiguousarray(np.asarray(inputs["emb_user"]).T)
    p["ei"] = np.ascontiguousarray(np.asarray(inputs["emb_item"]).T)

    adj_pad = np.zeros((NCORE, IPAD, K), np.int64)
    for c in range(NCORE):
        adj_pad[c, :IPC] = adj[c * IPC:(c + 1) * IPC]
    # gather col cc = b*K + k, partition pp = item within block:
    # idx[pp, cc] = adj[b*128+pp, k]
    slot_idx = np.transpose(adj_pad.reshape(NCORE, NBLK, 128, K), (0, 2, 1, 3))
    p["adji"] = np.ascontiguousarray(
        slot_idx.reshape(NCORE, 128, NBLK * K).astype(np.int32))

    p["spmm"] = _pack_spmm(rows, cols, vals)
    dlist = np.stack([np.concatenate([
        users[c * BPC:(c + 1) * BPC].astype(np.int64),
        items[c * BPC:(c + 1) * BPC].astype(np.int64) + NUM_USERS])
        for c in range(NCORE)])
    p["spmm3"] = _pack_list(rows, cols, vals, dlist, NWIN3)
    # rows layer 2 must cover: sources of layer-3 edges + K4's fidx rows
    order0 = np.argsort(rows, kind="stable")
    rs, cs = rows[order0], cols[order0]
    row_start = np.searchsorted(rs, np.arange(N + 1))
    rr = dlist.reshape(-1)
    cnt = row_start[rr + 1] - row_start[rr]
    ent = np.repeat(np.arange(len(rr)), cnt)
    ofs = np.arange(len(ent)) - np.repeat(np.cumsum(cnt) - cnt, cnt)
    need = np.zeros(N, bool)
    need[cs[row_start[rr][ent] + ofs]] = True
    need[rr] = True
    R2 = np.nonzero(need)[0]
    Lc = -(-len(R2) // NCORE)
    assert Lc <= L2, f"layer-2 list overflow: {Lc} > {L2}"
    lists2 = np.full((NCORE, L2), N, np.int64)          # pad with edge-less row
    for c in range(NCORE):
        seg = R2[c * Lc:(c + 1) * Lc]
        lists2[c, :len(seg)] = seg
    p["lists2"] = lists2
    p["spmm2"] = _pack_list(rows, cols, vals, lists2, NWIN2)

    p["k4"] = []
    for c in range(NCORE):
        u = users[c * BPC:(c + 1) * BPC].astype(np.int64)
        it = items[c * BPC:(c + 1) * BPC].astype(np.int64) + NUM_USERS
        rws = np.concatenate([u, it])
        p["k4"].append(np.ascontiguousarray(rws.reshape(8, 128).T.astype(np.int32)))
    return p


def _run(name, builder, in_maps):
    if name not in _BUILT:
        _BUILT[name] = builder()
    return bass_utils.run_bass_kernel_spmd(
        _BUILT[name], in_maps, core_ids=list(range(NCORE))).results


def kernel(**inputs):
    p = _prep(inputs)

    # ---------------- K1
    maps = [{
        "xu": p["xu"][:, c * UPC:(c + 1) * UPC],
        "xi": p["xi"][:, c * IPC:(c + 1) * IPC],
        "wu": p["wu"], "wi": p["wi"], "bu": p["bu"], "bi": p["bi"],
        "eu": p["eu"][:, c * UPC:(c + 1) * UPC],
        "ei": p["ei"][:, c * IPC:(c + 1) * IPC],
    } for c in range(NCORE)]
    r1 = _run("k1", _build_k1, maps)
    users_m = np.concatenate([r1[c]["ou"].T for c in range(NCORE)], 0)
    items_m = np.concatenate([r1[c]["oi"].T for c in range(NCORE)], 0)
    s1 = np.concatenate([r1[c]["os12"][0] for c in range(NCORE)])
    s2 = np.concatenate([r1[c]["os12"][1] for c in range(NCORE)])

    # ---------------- K2
    tblA = np.zeros((NUM_ITEMS, TBLW), np.float16)
    tblA[:, 0:64] = items_m
    tblA[:, 64] = s1
    maps = []
    for c in range(NCORE):
        s2c = np.zeros(IPAD, np.float32)
        s2c[:IPC] = s2[c * IPC:(c + 1) * IPC]
        s2r = np.transpose(np.broadcast_to(s2c.reshape(NBLK, 1, 128), (NBLK, K, 128)),
                           (2, 0, 1)).reshape(128, NBLK * K)
        imc = np.zeros((IPAD, 64), np.float32)
        imc[:IPC] = items_m[c * IPC:(c + 1) * IPC]
        itm = np.transpose(imc.reshape(NBLK, 128, 64), (1, 0, 2)).reshape(128, NBLK * 64)
        maps.append({"tbl": tblA, "adji": p["adji"][c],
                     "s2r": np.ascontiguousarray(s2r),
                     "itm": np.ascontiguousarray(itm)})
    r2 = _run("k2", _build_k2, maps)
    items_f = np.zeros((NUM_ITEMS, 64), np.float16)
    for c in range(NCORE):
        o = r2[c]["oit"].reshape(128, NBLK, 64).transpose(1, 0, 2).reshape(IPAD, 64)
        items_f[c * IPC:(c + 1) * IPC] = o[:IPC]

    # ---------------- K3 x3
    idx_arr, wreg, wext = p["spmm"]
    def run_layer(tbl_full):
        maps = [{"tbl": tbl_full, "idx": idx_arr[c],
                 "wreg": wreg[c], "wext": wext[c]} for c in range(NCORE)]
        r = _run("k3", _build_k3, maps)
        res = np.zeros((NPAD, 64), np.float16)
        for c in range(NCORE):
            res[c * DPC:(c + 1) * DPC] = r[c]["out"].T[:DPC]
        return res

    e0 = np.zeros((NPAD, 64), np.float16)
    e0[:NUM_USERS] = users_m.astype(np.float16)
    e0[NUM_USERS:N] = items_f
    e1 = run_layer(e0)

    # list-addressed layer 2: only rows consumed downstream
    idx2, wreg2, wext2 = p["spmm2"]
    maps = [{"tbl": e1, "idx": idx2[c], "wreg": wreg2[c], "wext": wext2[c]}
            for c in range(NCORE)]
    r2b = _run("k3b", _build_k3b, maps)
    e2 = np.zeros((NPAD, 64), np.float16)
    lists2 = p["lists2"]
    for c in range(NCORE):
        outc = r2b[c]["out"].T[:L2]
        sel = lists2[c] < N
        e2[lists2[c][sel]] = outc[sel]

    # compact layer 3: only the batch rows, already in K4's (p, t) layout
    idx3, wreg3, wext3 = p["spmm3"]
    maps = [{"tbl": e2, "idx": idx3[c], "wreg": wreg3[c], "wext": wext3[c]}
            for c in range(NCORE)]
    r3 = _run("k3c", _build_k3c, maps)
    tb3d = []
    for c in range(NCORE):
        rowsc = r3[c]["out"].T[:NDST3]                    # [1024, 64]
        tb3d.append(np.ascontiguousarray(
            rowsc.reshape(8, 128, 64).transpose(1, 0, 2).reshape(128, 8 * 64)))

    # ---------------- K4
    e012 = np.ascontiguousarray(np.concatenate([e0, e1, e2], axis=1))
    maps = [{"tb012": e012, "tb3d": tb3d[c], "fidx": p["k4"][c]}
            for c in range(NCORE)]
    r4 = _run("k4", _build_k4, maps)
    gamma = np.zeros(B, np.float32)
    for c in range(NCORE):
        gamma[c * BPC:(c + 1) * BPC] = r4[c]["out"].T.reshape(BPC)
    return gamma



# revision 8
# speedup vs baseline: 4.1488x; 4.1488x over previous
"""CoLaKG model kernel for 8 Trainium2 NeuronCores (self-contained).

Pipeline (6 bass SPMD launches; host does static prep + mechanical relayout):
  K1 gemm : semantic projections (users+items) + merge, + s1/s2 GAT scalars
  K2 attn : item-neighbor GAT attention (batched dma_gather + softmax + wsum)
  K3 spmm : LightGCN propagation layer 1 (all dests)
  K3b     : layer 2 at the ~87% of rows consumed downstream (list-addressed)
  K3c     : layer 3 at the batch's 2*512 rows per core
  K4 final: gather 4 embedding tables at (user,item) rows, mean, dot product

Perf design: all row gathers use the batched InstDMAGatherAnt (Q7 'mlp'
ucode library) at <=896 indices per instruction, amortizing the ~1us
SWDGE fixed cost ~7x vs per-column indirect DMA. Node tables are stored
at 256B stride with a 128B gathered payload (int16 indices -> 3 table
ranges). The SpMM packs edges per (512-dest window, range) sorted by
dest; each 128-edge column feeds one matmul into a fixed 64-wide psum
span (drift-tolerant placement; stragglers go to window-wide extra
columns; psum initialized by the first wide matmul's start flag).
"""
import copy
import numpy as np

import jax
jax.config.update("jax_compilation_cache_dir", "/tmp/.jax_bass_cache")
jax.config.update("jax_persistent_cache_min_entry_size_bytes", -1)
jax.config.update("jax_persistent_cache_min_compile_time_secs", 0.0)

import concourse.bass as bass
import concourse.mybir as mybir
from concourse.tile import TileContext
from concourse import bass_utils, library_config
import concourse.tile as tile_mod
from concourse.vector_clock import ScopedClock

F32 = mybir.dt.float32
F16 = mybir.dt.float16
I16 = mybir.dt.int16
I32 = mybir.dt.int32
AF = mybir.ActivationFunctionType

# ---------------------------------------------------------------- tile patch
MAX_WAITS = 1

def _split_sync_waits(nc, max_waits=MAX_WAITS):
    template = None
    counter = [0]
    for fn in nc.m.functions:
        for bb in fn.blocks:
            for inst in bb.instructions:
                if type(inst).__name__ == "InstNoOp":
                    template = copy.deepcopy(inst)
                    break
            if template is not None:
                break
        if template is not None:
            break
    for fn in nc.m.functions:
        for bb in fn.blocks:
            il = bb.instructions
            i = 0
            while i < len(il):
                inst = il[i]
                if template is None and type(inst).__name__ == "InstNoOp":
                    template = copy.deepcopy(inst)
                si = inst.sync_info
                if si is not None and si.on_wait is not None and len(si.on_wait) > max_waits:
                    assert template is not None, "no InstNoOp to clone"
                    waits = list(si.on_wait)
                    keep, rest = waits[:max_waits], waits[max_waits:]
                    si.on_wait.clear()
                    for w in keep:
                        si.on_wait.append(w)
                    carriers = []
                    while rest:
                        c = copy.deepcopy(template)
                        counter[0] += 1
                        c.name = f"I-waitsplit-{counter[0]}"
                        c.engine = inst.engine
                        c.sync_info = mybir.SyncInfo(on_wait=list(rest[:max_waits]), on_update=[])
                        carriers.append(c)
                        rest = rest[max_waits:]
                    for k, cinst in enumerate(carriers):
                        try:
                            nc.register_instruction(cinst, overwrite=True)
                        except Exception:
                            pass
                        il.insert(i + k, cinst)
                    i += len(carriers)
                i += 1

def _patched_drain_and_barrier(self, tick_clock, wait_clock):
    nc = self.nc
    nop0 = nc.sync.nop(nofuse=True, hint="predrain_waits")
    wait_clock.add_sem_waits(nop0.ins, ScopedClock({None: tick_clock.global_clock}))
    nc.sync.drain()
    nc.all_engine_barrier()
    assert self.sems is not None
    popped = nc._tile_sem_poison_stack.pop()
    assert popped is self._sem_poison
    nc.clear_and_free_semaphores(list(self.sems.allocated().values()))
    nc.all_engine_barrier()
    _split_sync_waits(nc)

tile_mod.TileContext._drain_and_barrier = _patched_drain_and_barrier

# ---------------------------------------------------------------- constants
NUM_USERS, NUM_ITEMS, D, SEM, HID, K = 60000, 30000, 64, 1024, 32, 32
N = NUM_USERS + NUM_ITEMS             # 90000
NPAD = 90112                          # 704*128
NCORE = 8
DPC = NPAD // NCORE                   # 11264 dest rows per core
WIN = 512                             # dests per window (one psum bank f32)
NWIN1 = DPC // WIN                    # 22 windows, layer-1
SWC = 64                              # psum span per regular matmul column
NRANGE = 3
RS = np.array([0, 32768, 65536, NPAD], dtype=np.int64)   # table range bounds
RROWS = [32768, 32768, 24576]
NI_MAX = 896                          # idx per gather instruction (ring cap)
TSTride = 128                         # table row stride (f16 elems) = 256B

UPC = NUM_USERS // NCORE              # 7500
IPC = NUM_ITEMS // NCORE              # 3750
IPAD = 3840
NBLK = IPAD // 128                    # 30
SUBB = 3                              # item blocks per attention sub-batch
NSUB = NBLK // SUBB                   # 10
GC = SUBB * K                         # 96 gather cols per K2 sub-batch
B = 4096
BPC = B // NCORE                      # 512

_BUILT = {}


def _dma_gather(g, out_ap, in_ap, idxs_ap, num_idxs, elem_size, elem_step,
                regs=None):
    """dma_gather with payload < stride (bass's public API asserts
    elem_size%256B which is only a stride requirement). regs: dict caching
    one GPSIMD register per distinct num_idxs value."""
    _in_ap = g.lower_ap_dma(in_ap, for_custom_bir_dma=True)
    _idxs_ap = g.lower_ap(idxs_ap)
    _out_ap = g.lower_ap(out_ap)
    if regs is None:
        reg = g.to_reg(num_idxs)
    else:
        if num_idxs not in regs:
            regs[num_idxs] = g.to_reg(num_idxs)
        reg = regs[num_idxs]
    stride_bytes = elem_step * mybir.dt.size(in_ap.dtype)
    assert stride_bytes % 256 == 0
    return g.add_instruction(
        mybir.InstDMAGatherAnt(
            name=g.bass.get_next_instruction_name(),
            ins=[*_in_ap, _idxs_ap, g.lower_val_access(reg)],
            outs=[_out_ap],
            transpose=False, num_idxs=num_idxs, elem_size=elem_size,
            stride_bytes_256=stride_bytes // 256, gen_mode=0,
            single_packet=True, queue_num=0,
            sbuf_tokens_per_rank=0, sbuf_free_dim_per_rank=0,
            sbuf_free_dim_pad_per_rank=0, sbuf_byte_offset=0,
        ))


def _elu(nc, pool, out_ap, in_ap, shape, tag):
    """out = elu(in) = max(x,0) + exp(min(x,0)) - 1   (no Elu in ACT table)."""
    mn = pool.tile(shape, F32, tag=tag + "_mn")
    nc.vector.tensor_scalar_min(mn[:], in_ap, 0.0)
    ex = pool.tile(shape, F32, tag=tag + "_ex")
    nc.scalar.activation(ex[:], mn[:], AF.Exp, scale=1.0)
    mx = pool.tile(shape, F32, tag=tag + "_mx")
    nc.vector.tensor_scalar_max(mx[:], in_ap, 0.0)
    nc.vector.tensor_add(out_ap, mx[:], ex[:])
    nc.vector.tensor_scalar_add(out_ap, out_ap, -1.0)


# ================================================================ K1: GEMM
def _build_k1():
    nc = bass.Bass("TRN2", target_bir_lowering=False)
    xu = nc.dram_tensor("xu", [SEM, UPC], F16, kind="ExternalInput")
    xi = nc.dram_tensor("xi", [SEM, IPC], F16, kind="ExternalInput")
    wu = nc.dram_tensor("wu", [SEM, 64], F16, kind="ExternalInput")
    wi = nc.dram_tensor("wi", [SEM, 66], F16, kind="ExternalInput")
    bu = nc.dram_tensor("bu", [64, 1], F32, kind="ExternalInput")
    bi = nc.dram_tensor("bi", [66, 1], F32, kind="ExternalInput")
    eu = nc.dram_tensor("eu", [64, UPC], F32, kind="ExternalInput")
    ei = nc.dram_tensor("ei", [64, IPC], F32, kind="ExternalInput")
    ou = nc.dram_tensor("ou", [64, UPC], F32, kind="ExternalOutput")
    oi = nc.dram_tensor("oi", [64, IPC], F32, kind="ExternalOutput")
    os12 = nc.dram_tensor("os12", [2, IPC], F32, kind="ExternalOutput")

    with TileContext(nc) as tc:
        with tc.tile_pool(name="w", bufs=1) as wp, \
             tc.tile_pool(name="x", bufs=3) as xp, \
             tc.tile_pool(name="o", bufs=2) as op, \
             tc.tile_pool(name="ps", bufs=2, space="PSUM") as pp:
            wu_sb = wp.tile([128, SEM // 128, 64], F16, tag="wu")
            nc.sync.dma_start(wu_sb[:], wu[:].rearrange("(a p) m -> p a m", p=128))
            wi_sb = wp.tile([128, SEM // 128, 66], F16, tag="wi")
            nc.sync.dma_start(wi_sb[:], wi[:].rearrange("(a p) m -> p a m", p=128))
            bu_sb = wp.tile([64, 1], F32, tag="bu")
            nc.sync.dma_start(bu_sb[:], bu[:])
            bi_sb = wp.tile([66, 1], F32, tag="bi")
            nc.sync.dma_start(bi_sb[:], bi[:])

            def gemm(xten, eten, wtile, btile, oten, m, rows, R, s12=None):
                for t in range(rows // R):
                    xt = xp.tile([128, SEM // 128, R], F16, tag="xt")
                    nc.sync.dma_start(
                        xt[:], xten[:, t * R:(t + 1) * R].rearrange("(a p) r -> p a r", p=128))
                    ps = pp.tile([m, R], F32, tag="ps")
                    for kk in range(SEM // 128):
                        nc.tensor.matmul(ps[:], wtile[:, kk, :], xt[:, kk, :],
                                         start=(kk == 0), stop=(kk == SEM // 128 - 1))
                    xb = op.tile([64, R], F32, tag="xb")
                    nc.vector.tensor_scalar_add(xb[:], ps[0:64, :], btile[0:64, :])
                    mg = op.tile([64, R], F32, tag="mg")
                    _elu(nc, op, mg[:], xb[:], [64, R], "e1")
                    et = op.tile([64, R], F32, tag="et")
                    nc.sync.dma_start(et[:], eten[:, t * R:(t + 1) * R])
                    nc.vector.tensor_add(mg[:], mg[:], et[:])
                    nc.scalar.mul(mg[:], mg[:], 0.5)
                    nc.sync.dma_start(oten[:, t * R:(t + 1) * R], mg[:])
                    if s12 is not None:
                        sv = op.tile([2, R], F32, tag="sv")
                        nc.scalar.copy(sv[:], ps[64:66, :])
                        nc.sync.dma_start(s12[:, t * R:(t + 1) * R], sv[:])

            gemm(xu, eu, wu_sb, bu_sb, ou, 64, UPC, 500)
            gemm(xi, ei, wi_sb, bi_sb, oi, 66, IPC, 375, s12=os12)
    return nc


# ================================================================ K2: attention
# Item table rows: 128 f16 (256B): [emb 0:64 | s1 @64 | pad]. Payload 66.
PAY2 = 66

def _build_k2():
    nc = bass.Bass("TRN2", target_bir_lowering=False)
    tbl = nc.dram_tensor("tbl", [NUM_ITEMS, TSTride], F16, kind="ExternalInput")
    adji = nc.dram_tensor("adji", [128, (NBLK * K * 128) // 16], I16, kind="ExternalInput")
    s2r = nc.dram_tensor("s2r", [128, NBLK * K], F32, kind="ExternalInput")
    itm = nc.dram_tensor("itm", [128, NBLK * 64], F32, kind="ExternalInput")
    oit = nc.dram_tensor("oit", [128, NBLK * 64], F16, kind="ExternalOutput")

    with TileContext(nc) as tc:
        with tc.tile_pool(name="g", bufs=2) as gp, \
             tc.tile_pool(name="t", bufs=2) as tp, \
             tc.tile_pool(name="s", bufs=1) as sp, \
             tc.tile_pool(name="m", bufs=2) as mp:
            nc.gpsimd.load_library(library_config.mlp)
            adj_sb = sp.tile([128, (NBLK * K * 128) // 16], I16, tag="adj")
            nc.sync.dma_start(adj_sb[:], adji[:])
            s2_sb = sp.tile([128, NBLK * K], F32, tag="s2")
            nc.sync.dma_start(s2_sb[:], s2r[:])
            itm_sb = sp.tile([128, NBLK * 64], F32, tag="itm")
            nc.sync.dma_start(itm_sb[:], itm[:])
            niregs = {}
            for u in range(NSUB):
                g = gp.tile([128, GC, PAY2], F16, tag="g")
                # GC*128 = 12288 idx in chunks of 896 (7 cols)
                base16 = u * (GC * 128) // 16
                col = 0
                left = GC * 128
                while left > 0:
                    n = min(NI_MAX, left)
                    _dma_gather(nc.gpsimd, g[:, col:col + n // 128, :], tbl[:],
                                adj_sb[:, base16:base16 + n // 16], n, PAY2, TSTride,
                                regs=niregs)
                    base16 += n // 16
                    col += n // 128
                    left -= n
                s1f = mp.tile([128, GC], F32, tag="s1f")
                nc.scalar.copy(s1f[:], g[:, :, 64])
                lg = mp.tile([128, GC], F32, tag="lg")
                nc.vector.tensor_add(lg[:], s1f[:], s2_sb[:, u * GC:(u + 1) * GC])
                lr = mp.tile([128, GC], F32, tag="lr")
                nc.scalar.mul(lr[:], lg[:], 0.2)
                nc.vector.tensor_max(lg[:], lg[:], lr[:])
                ex = mp.tile([128, SUBB, K], F32, tag="ex")
                nc.scalar.activation(ex[:].rearrange("p a b -> p (a b)"), lg[:],
                                     AF.Exp, scale=1.0)
                sm = mp.tile([128, SUBB], F32, tag="sm")
                nc.vector.reduce_sum(sm[:], ex[:], axis=mybir.AxisListType.X)
                nc.vector.reciprocal(sm[:], sm[:])
                att = mp.tile([128, SUBB, K], F16, tag="att")
                for bb in range(SUBB):
                    nc.vector.tensor_scalar_mul(att[:, bb, :], ex[:, bb, :], sm[:, bb:bb + 1])
                tmp = tp.tile([128, SUBB, K, 64], F16, tag="tmp")
                av = att[:]
                att_b = bass.AP(av.tensor, av.offset, list(av.ap) + [[0, 64]])
                nc.vector.tensor_mul(
                    tmp[:], g[:].rearrange("p (b k) d -> p b k d", b=SUBB)[:, :, :, 0:64],
                    att_b)
                hp = mp.tile([128, SUBB, 64], F32, tag="hp")
                nc.vector.reduce_sum(hp[:], tmp[:].rearrange("p b k d -> p b d k"),
                                     axis=mybir.AxisListType.X)
                he = mp.tile([128, SUBB * 64], F32, tag="he")
                _elu(nc, mp, he[:], hp[:].rearrange("p b d -> p (b d)"),
                     [128, SUBB * 64], "e2")
                fo = mp.tile([128, SUBB * 64], F32, tag="fo")
                nc.vector.tensor_add(fo[:], he[:],
                                     itm_sb[:, u * SUBB * 64:(u + 1) * SUBB * 64])
                fo16 = mp.tile([128, SUBB * 64], F16, tag="fo16")
                nc.scalar.mul(fo16[:], fo[:], 0.5)
                nc.sync.dma_start(oit[:, u * SUBB * 64:(u + 1) * SUBB * 64], fo16[:])
    return nc


# ================================================================ K3: spmm
def _build_k3(meta):
    """SpMM layer kernel from packing metadata.

    meta: nwin, cap[w][r] (regular slots), ex[w][r] (extra cols),
          off[w][r] = list of per-column psum offsets.
    Stream layout per (w, r): [extra cols | regular cols]; per window the
    first extra of r0 initializes psum (start=True); last regular matmul
    of the last nonempty range carries stop=True.
    """
    nwin = meta["nwin"]
    cap = meta["cap"]; ex = meta["ex"]; off = meta["off"]
    totslots = int(sum(cap[w][r] + 128 * ex[w][r]
                       for w in range(nwin) for r in range(NRANGE)))
    totregcol = int(sum(cap[w][r] // 128 for w in range(nwin) for r in range(NRANGE)))
    totexcol = int(sum(ex[w][r] for w in range(nwin) for r in range(NRANGE)))
    maxwcol = max(sum(cap[w][r] // 128 + ex[w][r] for r in range(NRANGE))
                  for w in range(nwin))
    maxwreg = max(sum(cap[w][r] // 128 for r in range(NRANGE)) for w in range(nwin))
    maxwex = max(sum(ex[w][r] for r in range(NRANGE)) for w in range(nwin))

    nc = bass.Bass("TRN2", target_bir_lowering=False)
    tbls = [nc.dram_tensor(f"tbl{r}", [RROWS[r], TSTride], F16, kind="ExternalInput")
            for r in range(NRANGE)]
    idx = nc.dram_tensor("idx", [128, totslots // 16], I16, kind="ExternalInput")
    wreg = nc.dram_tensor("wreg", [128, totregcol * SWC], F16, kind="ExternalInput")
    wext = nc.dram_tensor("wext", [128, max(totexcol, 1) * WIN], F16, kind="ExternalInput")
    out = nc.dram_tensor("out", [64, nwin * WIN], F16, kind="ExternalOutput")

    with TileContext(nc) as tc:
        with tc.tile_pool(name="s", bufs=1) as sp, \
             tc.tile_pool(name="g", bufs=3) as gp, \
             tc.tile_pool(name="w", bufs=3) as wp, \
             tc.tile_pool(name="o", bufs=3) as op, \
             tc.tile_pool(name="ps", bufs=4, space="PSUM") as pp:
            nc.gpsimd.load_library(library_config.mlp)
            idx_sb = sp.tile([128, totslots // 16], I16, tag="idx")
            nc.sync.dma_start(idx_sb[:], idx[:])
            niregs = {}
            i16 = 0          # cursor into idx (units of 16 slots)
            rcol = 0         # cursor into wreg (regular col index)
            ecol = 0         # cursor into wext (extra col index)
            for w in range(nwin):
                wcols = sum(cap[w][r] // 128 + ex[w][r] for r in range(NRANGE))
                wregc = sum(cap[w][r] // 128 for r in range(NRANGE))
                wexc = sum(ex[w][r] for r in range(NRANGE))
                gt = gp.tile([128, maxwcol, SWC], F16, tag="g")
                wr = wp.tile([128, maxwreg * SWC], F16, tag="wr")
                nc.sync.dma_start(wr[:, 0:wregc * SWC],
                                  wreg[:, rcol * SWC:(rcol + wregc) * SWC])
                if maxwex:
                    we = wp.tile([128, max(maxwex, 1) * WIN], F16, tag="we")
                    if wexc:
                        nc.sync.dma_start(we[:, 0:wexc * WIN],
                                          wext[:, ecol * WIN:(ecol + wexc) * WIN])
                # gathers for the whole window (extras first per range)
                col = 0
                colmap = []   # per range: (excolbase, regcolbase)
                for r in range(NRANGE):
                    nsl = cap[w][r] + 128 * ex[w][r]
                    colmap.append((col, col + ex[w][r]))
                    left = nsl
                    while left > 0:
                        n = min(NI_MAX, left)
                        _dma_gather(nc.gpsimd, gt[:, col:col + n // 128, :], tbls[r][:],
                                    idx_sb[:, i16:i16 + n // 16], n, SWC, TSTride,
                                    regs=niregs)
                        i16 += n // 16
                        col += n // 128
                        left -= n
                # matmuls
                ps = pp.tile([64, WIN], F32, tag="ps")
                first = True
                wrc = 0
                wec = 0
                last_r = max(r for r in range(NRANGE) if cap[w][r] > 0)
                for r in range(NRANGE):
                    exbase, regbase = colmap[r]
                    for e in range(ex[w][r]):
                        nc.tensor.matmul(ps[:], gt[:, exbase + e, :],
                                         we[:, wec * WIN:(wec + 1) * WIN],
                                         start=first, stop=False)
                        first = False
                        wec += 1
                    ncols = cap[w][r] // 128
                    for j in range(ncols):
                        o = off[w][r][j]
                        stop = (r == last_r and j == ncols - 1)
                        nc.tensor.matmul(ps[:, o:o + SWC], gt[:, regbase + j, :],
                                         wr[:, wrc * SWC:(wrc + 1) * SWC],
                                         start=first, stop=stop)
                        first = False
                        wrc += 1
                rcol += wregc
                ecol += wexc
                ot = op.tile([64, WIN], F16, tag="ot")
                nc.scalar.copy(ot[:], ps[:])
                nc.sync.dma_start(out[:, w * WIN:(w + 1) * WIN], ot[:])
    return nc


# ================================================================ host packing
def _pack_edges(core, pos, src, val, nwin):
    """Pack edges (dest position pos within core, source node src) into the
    per-(window, range) gather/weight layout. Returns per-core arrays + meta."""
    w = pos // WIN
    drel = (pos - w * WIN).astype(np.int64)
    rg = np.searchsorted(RS, src, side="right") - 1
    src_rel = (src - RS[rg]).astype(np.int64)
    order = np.lexsort((drel, rg, w, core))
    core, w, drel, rg, src_rel, val = (a[order] for a in (core, w, drel, rg, src_rel, val))

    key = (core * nwin + w) * NRANGE + rg
    cnt = np.bincount(key, minlength=NCORE * nwin * NRANGE).reshape(NCORE, nwin, NRANGE)
    cap = ((cnt.max(axis=0) + 127) // 128) * 128          # [nwin, NRANGE]
    cap = np.maximum(cap, 128)                            # every cell >= 1 col

    n = len(key)
    ar = np.arange(n)
    first = np.empty(n, bool); first[0] = True; first[1:] = key[1:] != key[:-1]
    slot = ar - np.maximum.accumulate(np.where(first, ar, 0))

    ncol = cap >> 7
    regcol_base = np.concatenate([[0], np.cumsum(ncol.reshape(-1))])[:-1]\
        .reshape(nwin, NRANGE)
    totregcol = int(ncol.sum())

    # data-driven column offsets: cover the across-core [min, max] dest range
    colj = slot >> 7
    gcol = regcol_base[w, rg] + colj
    lo = np.full(totregcol, WIN, np.int64)
    hi = np.full(totregcol, -1, np.int64)
    np.minimum.at(lo, gcol, drel)
    np.maximum.at(hi, gcol, drel)
    lo = np.minimum(lo, hi)                               # empty cols -> [hi,hi]
    offcol = np.clip((lo + hi + 1 - SWC) // 2, 0, WIN - SWC)
    off_e = offcol[gcol]
    spill = (drel < off_e) | (drel >= off_e + SWC)

    # extra column counts (same for all cores)
    skey = key[spill]
    scnt = np.bincount(skey, minlength=NCORE * nwin * NRANGE).reshape(NCORE, nwin, NRANGE)
    exc = (scnt.max(axis=0) + 127) // 128                 # [nwin, NRANGE] cols
    exc[:, 0] = np.maximum(exc[:, 0], 1)                  # psum initializer

    # per-(w,r) stream slot count and bases (same all cores)
    cell_slots = cap + 128 * exc                          # [nwin, NRANGE]
    cell_base = np.concatenate([[0], np.cumsum(cell_slots.reshape(-1))])[:-1]\
        .reshape(nwin, NRANGE)                            # base within core stream
    tot = int(cell_slots.sum())
    excol_base = np.concatenate([[0], np.cumsum(exc.reshape(-1))])[:-1]\
        .reshape(nwin, NRANGE)
    totexcol = int(exc.sum())

    idx_flat = np.zeros((NCORE, tot), np.int16)
    wreg = np.zeros((NCORE, 128, totregcol * SWC), np.float16)
    wext = np.zeros((NCORE, 128, max(totexcol, 1) * WIN), np.float16)

    # regular slots: stream position = cell_base + 128*exc (extras first) + slot
    spos = cell_base[w, rg] + 128 * exc[w, rg] + slot
    idx_flat[core, spos] = src_rel.astype(np.int16)
    reg = ~spill
    fw = (core[reg] * 128 + (slot[reg] & 127)) * (totregcol * SWC) \
        + (regcol_base[w[reg], rg[reg]] + colj[reg]) * SWC + (drel[reg] - off_e[reg])
    wreg.reshape(-1)[fw] = val[reg]

    # spilled edges -> extra slots (their regular slot stays as weight-0 pad)
    if spill.any():
        sc, sw_, srg, ssrc, sdrel, sval = (a[spill] for a in (core, w, rg, src_rel, drel, val))
        ns = len(sc)
        ars = np.arange(ns)
        sfirst = np.empty(ns, bool); sfirst[0] = True; sfirst[1:] = skey[1:] != skey[:-1]
        eslot = ars - np.maximum.accumulate(np.where(sfirst, ars, 0))
        espos = cell_base[sw_, srg] + eslot
        idx_flat[sc, espos] = ssrc.astype(np.int16)
        fx = (sc * 128 + (eslot & 127)) * (max(totexcol, 1) * WIN) \
            + (excol_base[sw_, srg] + (eslot >> 7)) * WIN + sdrel
        wext.reshape(-1)[fx] = sval

    # idx stream -> [128, tot/16] wrapped+replicated layout
    idx_arr = np.tile(idx_flat.reshape(NCORE, tot // 16, 16).transpose(0, 2, 1),
                      (1, 8, 1))

    off_tab = [[list(int(offcol[regcol_base[w_, r_] + j_])
                     for j_ in range(ncol[w_][r_]))
                for r_ in range(NRANGE)] for w_ in range(nwin)]
    meta = {"nwin": nwin,
            "cap": [[int(cap[w_][r_]) for r_ in range(NRANGE)] for w_ in range(nwin)],
            "ex": [[int(exc[w_][r_]) for r_ in range(NRANGE)] for w_ in range(nwin)],
            "off": off_tab}
    return idx_arr, wreg, wext, meta


def _edges_for_lists(rows, cols, vals, dlist):
    """Expand: for each core and each listed dest (position p in dlist[c]),
    all incoming edges. Returns (core, pos, src, val). dlist entries >= N are
    edgeless sentinels."""
    order0 = np.argsort(rows, kind="stable")
    rs, cs, vs = rows[order0], cols[order0], vals[order0]
    row_start = np.searchsorted(rs, np.arange(N + 1))
    rr = dlist.reshape(-1).astype(np.int64)
    rrc = np.minimum(rr, N)
    cnt = row_start[np.minimum(rrc + 1, N)] - row_start[rrc]
    cnt[rr >= N] = 0
    ent = np.repeat(np.arange(len(rr)), cnt)
    ofs = np.arange(len(ent)) - np.repeat(np.cumsum(cnt) - cnt, cnt)
    srcidx = row_start[rrc][ent] + ofs
    ndst = dlist.shape[1]
    e_core = ent // ndst
    e_pos = ent - e_core * ndst
    return e_core, e_pos, cs[srcidx], vs[srcidx]


def _prep(inputs):
    p = {}
    users = np.asarray(inputs["users"]);   items = np.asarray(inputs["items"])
    adj = np.asarray(inputs["adj_matrix"])
    rows = np.asarray(inputs["graph_rows"]).astype(np.int64)
    cols = np.asarray(inputs["graph_cols"]).astype(np.int64)
    vals = np.asarray(inputs["graph_vals"]).astype(np.float32)
    W_att = np.asarray(inputs["W_att"]); a_att = np.asarray(inputs["a_att"])
    v1 = W_att @ a_att[:HID, 0]; v2 = W_att @ a_att[HID:, 0]

    p["xu"] = np.ascontiguousarray(np.asarray(inputs["user_semantic_emb"]).astype(np.float16).T)
    p["xi"] = np.ascontiguousarray(np.asarray(inputs["semantic_emb"]).astype(np.float16).T)
    p["wu"] = np.asarray(inputs["W_usem"]).astype(np.float16)
    p["wi"] = np.concatenate([np.asarray(inputs["W_sem"]), v1[:, None], v2[:, None]],
                             axis=1).astype(np.float16)
    p["bu"] = np.asarray(inputs["b_usem"]).reshape(64, 1)
    p["bi"] = np.concatenate([np.asarray(inputs["b_sem"]), np.zeros(2, np.float32)]
                             ).reshape(66, 1).astype(np.float32)
    p["eu"] = np.ascontiguousarray(np.asarray(inputs["emb_user"]).T)
    p["ei"] = np.ascontiguousarray(np.asarray(inputs["emb_item"]).T)

    # K2 neighbor indices: gather slot i (of GC*128 per sub-batch) = col*128+p
    # -> adj[block b = (u*SUBB + col//K), item p, k = col%K]
    adj_pad = np.zeros((NCORE, IPAD, K), np.int64)
    for c in range(NCORE):
        adj_pad[c, :IPC] = adj[c * IPC:(c + 1) * IPC]
    slot_idx = np.transpose(adj_pad.reshape(NCORE, NBLK, 128, K), (0, 2, 1, 3))
    # flat stream per core: for sub-batch u, col cc, slot p: value adj[..]
    flat = np.transpose(adj_pad.reshape(NCORE, NBLK, 128, K), (0, 1, 3, 2))\
        .reshape(NCORE, NBLK * K * 128)            # (b, k) cols x 128 slots
    p["adji"] = np.tile(flat.reshape(NCORE, -1, 16).transpose(0, 2, 1),
                        (1, 8, 1)).astype(np.int16)

    # layer 1: all NPAD dests
    e_core = rows // DPC
    e_pos = rows - e_core * DPC
    p["l1"] = _pack_edges(e_core, e_pos, cols, vals, NWIN1)

    # batch dest list (layer 3 + K4)
    dlist = np.stack([np.concatenate([
        users[c * BPC:(c + 1) * BPC].astype(np.int64),
        items[c * BPC:(c + 1) * BPC].astype(np.int64) + NUM_USERS])
        for c in range(NCORE)])
    NWIN3 = (2 * BPC) // WIN                       # 2
    ec, ep, es, ev = _edges_for_lists(rows, cols, vals, dlist)
    p["l3"] = _pack_edges(ec, ep, es, ev, NWIN3)
    p["nwin3"] = NWIN3

    # layer 2 rows needed: sources of layer-3 edges + K4's rows
    need = np.zeros(N, bool)
    need[es] = True
    need[dlist.reshape(-1)] = True
    R2 = np.nonzero(need)[0]
    Lc = -(-len(R2) // NCORE)
    L2 = -(-Lc // WIN) * WIN
    NWIN2 = L2 // WIN
    lists2 = np.full((NCORE, L2), N, np.int64)
    for c in range(NCORE):
        seg = R2[c * Lc:(c + 1) * Lc]
        lists2[c, :len(seg)] = seg
    p["lists2"] = lists2
    ec, ep, es2, ev2 = _edges_for_lists(rows, cols, vals, lists2)
    p["l2"] = _pack_edges(ec, ep, es2, ev2, NWIN2)
    p["nwin2"] = NWIN2

    p["k4"] = []
    for c in range(NCORE):
        u = users[c * BPC:(c + 1) * BPC].astype(np.int64)
        it = items[c * BPC:(c + 1) * BPC].astype(np.int64) + NUM_USERS
        rws = np.concatenate([u, it])
        p["k4"].append(np.ascontiguousarray(rws.reshape(8, 128).T.astype(np.int32)))
    return p


# ================================================================ K4: final
def _build_k4():
    nc = bass.Bass("TRN2", target_bir_lowering=False)
    tb012 = nc.dram_tensor("tb012", [NPAD, 192], F16, kind="ExternalInput")
    tb3d = nc.dram_tensor("tb3d", [128, 8 * 64], F16, kind="ExternalInput")
    fidx = nc.dram_tensor("fidx", [128, 8], I32, kind="ExternalInput")
    out = nc.dram_tensor("out", [128, 4], F32, kind="ExternalOutput")

    with TileContext(nc) as tc:
        with tc.tile_pool(name="g", bufs=2) as gp, \
             tc.tile_pool(name="m", bufs=1) as mp:
            it = mp.tile([128, 8], I32, tag="it")
            nc.sync.dma_start(it[:], fidx[:])
            t3 = mp.tile([128, 8, 64], F16, tag="t3")
            nc.sync.dma_start(t3[:].rearrange("p a b -> p (a b)"), tb3d[:])
            acc = mp.tile([128, 8, 64], F32, tag="acc")
            nc.scalar.copy(acc[:], t3[:])
            g = gp.tile([128, 8, 192], F16, tag="g")
            for t in range(8):
                nc.gpsimd.indirect_dma_start(
                    out=g[:, t, :], out_offset=None, in_=tb012[:],
                    in_offset=bass.IndirectOffsetOnAxis(ap=it[:, t:t + 1], axis=0))
            for sl in range(3):
                gf = gp.tile([128, 8, 64], F32, tag="gf")
                nc.scalar.copy(gf[:], g[:].rearrange("p a (s b) -> p a s b", s=3)[:, :, sl, :])
                nc.vector.tensor_add(acc[:], acc[:], gf[:])
            nc.scalar.mul(acc[:], acc[:], 0.25)
            prod = mp.tile([128, 4, 64], F32, tag="prod")
            nc.vector.tensor_mul(prod[:], acc[:, 0:4, :], acc[:, 4:8, :])
            res = mp.tile([128, 4], F32, tag="res")
            nc.vector.reduce_sum(res[:], prod[:], axis=mybir.AxisListType.X)
            nc.sync.dma_start(out[:], res[:])
    return nc


_META = {}

def _run(name, builder, in_maps, meta_key=None):
    if name not in _BUILT or _META.get(name) != meta_key:
        nc = builder()
        mybir.codegen_inst_isa_subclasses(nc)
        _BUILT[name] = nc
        _META[name] = meta_key
    return bass_utils.run_bass_kernel_spmd(
        _BUILT[name], in_maps, core_ids=list(range(NCORE))).results


def _tables_from_nodes(node_tbl):
    """node_tbl [NPAD, 64] f16 -> 3 range tables [rows, 128] f16."""
    full = np.zeros((NPAD, TSTride), np.float16)
    full[:, 0:64] = node_tbl
    return [np.ascontiguousarray(full[RS[r]:RS[r + 1]]) for r in range(NRANGE)]


def kernel(**inputs):
    p = _prep(inputs)

    # ---------------- K1
    maps = [{
        "xu": p["xu"][:, c * UPC:(c + 1) * UPC],
        "xi": p["xi"][:, c * IPC:(c + 1) * IPC],
        "wu": p["wu"], "wi": p["wi"], "bu": p["bu"], "bi": p["bi"],
        "eu": p["eu"][:, c * UPC:(c + 1) * UPC],
        "ei": p["ei"][:, c * IPC:(c + 1) * IPC],
    } for c in range(NCORE)]
    r1 = _run("k1", _build_k1, maps)
    users_m = np.concatenate([r1[c]["ou"].T for c in range(NCORE)], 0)
    items_m = np.concatenate([r1[c]["oi"].T for c in range(NCORE)], 0)
    s1 = np.concatenate([r1[c]["os12"][0] for c in range(NCORE)])
    s2 = np.concatenate([r1[c]["os12"][1] for c in range(NCORE)])

    # ---------------- K2
    tblA = np.zeros((NUM_ITEMS, TSTride), np.float16)
    tblA[:, 0:64] = items_m
    tblA[:, 64] = s1
    maps = []
    for c in range(NCORE):
        s2c = np.zeros(IPAD, np.float32)
        s2c[:IPC] = s2[c * IPC:(c + 1) * IPC]
        s2r = np.transpose(np.broadcast_to(s2c.reshape(NBLK, 1, 128), (NBLK, K, 128)),
                           (2, 0, 1)).reshape(128, NBLK * K)
        imc = np.zeros((IPAD, 64), np.float32)
        imc[:IPC] = items_m[c * IPC:(c + 1) * IPC]
        itm = np.transpose(imc.reshape(NBLK, 128, 64), (1, 0, 2)).reshape(128, NBLK * 64)
        maps.append({"tbl": tblA, "adji": p["adji"][c],
                     "s2r": np.ascontiguousarray(s2r),
                     "itm": np.ascontiguousarray(itm)})
    r2 = _run("k2", _build_k2, maps)
    items_f = np.zeros((NUM_ITEMS, 64), np.float16)
    for c in range(NCORE):
        o = r2[c]["oit"].reshape(128, NBLK, 64).transpose(1, 0, 2).reshape(IPAD, 64)
        items_f[c * IPC:(c + 1) * IPC] = o[:IPC]

    # ---------------- K3 layers
    def run_spmm(name, pack, tbl_nodes):
        idx_arr, wreg, wext, meta = pack
        tbls = _tables_from_nodes(tbl_nodes)
        maps = [dict({f"tbl{r}": tbls[r] for r in range(NRANGE)},
                     idx=idx_arr[c], wreg=wreg[c], wext=wext[c])
                for c in range(NCORE)]
        mk = (meta["nwin"], tuple(map(tuple, meta["cap"])), tuple(map(tuple, meta["ex"])))
        r = _run(name, lambda: _build_k3(meta), maps, meta_key=mk)
        return r

    e0 = np.zeros((NPAD, 64), np.float16)
    e0[:NUM_USERS] = users_m.astype(np.float16)
    e0[NUM_USERS:N] = items_f

    r = run_spmm("k3", p["l1"], e0)
    e1 = np.zeros((NPAD, 64), np.float16)
    for c in range(NCORE):
        e1[c * DPC:(c + 1) * DPC] = r[c]["out"].T

    r = run_spmm("k3b", p["l2"], e1)
    e2 = np.zeros((NPAD, 64), np.float16)
    lists2 = p["lists2"]
    for c in range(NCORE):
        outc = r[c]["out"].T
        sel = lists2[c] < N
        e2[lists2[c][sel]] = outc[:len(lists2[c])][sel]

    r = run_spmm("k3c", p["l3"], e2)
    tb3d = []
    for c in range(NCORE):
        rowsc = r[c]["out"].T[:2 * BPC]                  # [1024, 64]
        tb3d.append(np.ascontiguousarray(
            rowsc.reshape(8, 128, 64).transpose(1, 0, 2).reshape(128, 8 * 64)))

    # ---------------- K4
    e012 = np.ascontiguousarray(np.concatenate([e0, e1, e2], axis=1))
    maps = [{"tb012": e012, "tb3d": tb3d[c], "fidx": p["k4"][c]}
            for c in range(NCORE)]
    r4 = _run("k4", _build_k4, maps)
    gamma = np.zeros(B, np.float32)
    for c in range(NCORE):
        gamma[c * BPC:(c + 1) * BPC] = r4[c]["out"].T.reshape(BPC)
    return gamma


# revision 11
# speedup vs baseline: 4.2021x; 1.0128x over previous
"""CoLaKG model kernel for 8 Trainium2 NeuronCores (self-contained).

Pipeline (6 bass SPMD launches; host does static prep + mechanical relayout):
  K1 gemm : semantic projections (users+items) + merge, + s1/s2 GAT scalars
  K2 attn : item-neighbor GAT attention (batched dma_gather + softmax + wsum)
  K3 spmm : LightGCN propagation layer 1 (all dests)
  K3b     : layer 2 at the ~87% of rows consumed downstream (list-addressed)
  K3c     : layer 3 at the batch's 2*512 rows per core
  K4 final: gather 4 embedding tables at (user,item) rows, mean, dot product

Perf design: all row gathers use the batched InstDMAGatherAnt (Q7 'mlp'
ucode library) at <=896 indices per instruction, amortizing the ~1us
SWDGE fixed cost ~7x vs per-column indirect DMA. Node tables are stored
at 256B stride with a 128B gathered payload (int16 indices -> 3 table
ranges). The SpMM packs edges per (512-dest window, range) sorted by
dest; each 128-edge column feeds one matmul into a fixed 64-wide psum
span (drift-tolerant placement; stragglers go to window-wide extra
columns; psum initialized by the first wide matmul's start flag).
"""
import copy
import numpy as np

import jax
jax.config.update("jax_compilation_cache_dir", "/tmp/.jax_bass_cache")
jax.config.update("jax_persistent_cache_min_entry_size_bytes", -1)
jax.config.update("jax_persistent_cache_min_compile_time_secs", 0.0)

import concourse.bass as bass
import concourse.mybir as mybir
from concourse.tile import TileContext
from concourse import bass_utils, library_config
import concourse.tile as tile_mod
from concourse.vector_clock import ScopedClock

F32 = mybir.dt.float32
F16 = mybir.dt.float16
I16 = mybir.dt.int16
I32 = mybir.dt.int32
AF = mybir.ActivationFunctionType

# ---------------------------------------------------------------- tile patch
MAX_WAITS = 1

def _split_sync_waits(nc, max_waits=MAX_WAITS):
    template = None
    counter = [0]
    for fn in nc.m.functions:
        for bb in fn.blocks:
            for inst in bb.instructions:
                if type(inst).__name__ == "InstNoOp":
                    template = copy.deepcopy(inst)
                    break
            if template is not None:
                break
        if template is not None:
            break
    for fn in nc.m.functions:
        for bb in fn.blocks:
            il = bb.instructions
            i = 0
            while i < len(il):
                inst = il[i]
                if template is None and type(inst).__name__ == "InstNoOp":
                    template = copy.deepcopy(inst)
                si = inst.sync_info
                if si is not None and si.on_wait is not None and len(si.on_wait) > max_waits:
                    assert template is not None, "no InstNoOp to clone"
                    waits = list(si.on_wait)
                    keep, rest = waits[:max_waits], waits[max_waits:]
                    si.on_wait.clear()
                    for w in keep:
                        si.on_wait.append(w)
                    carriers = []
                    while rest:
                        c = copy.deepcopy(template)
                        counter[0] += 1
                        c.name = f"I-waitsplit-{counter[0]}"
                        c.engine = inst.engine
                        c.sync_info = mybir.SyncInfo(on_wait=list(rest[:max_waits]), on_update=[])
                        carriers.append(c)
                        rest = rest[max_waits:]
                    for k, cinst in enumerate(carriers):
                        try:
                            nc.register_instruction(cinst, overwrite=True)
                        except Exception:
                            pass
                        il.insert(i + k, cinst)
                    i += len(carriers)
                i += 1

def _patched_drain_and_barrier(self, tick_clock, wait_clock):
    nc = self.nc
    nop0 = nc.sync.nop(nofuse=True, hint="predrain_waits")
    wait_clock.add_sem_waits(nop0.ins, ScopedClock({None: tick_clock.global_clock}))
    nc.sync.drain()
    nc.all_engine_barrier()
    assert self.sems is not None
    popped = nc._tile_sem_poison_stack.pop()
    assert popped is self._sem_poison
    nc.clear_and_free_semaphores(list(self.sems.allocated().values()))
    nc.all_engine_barrier()
    _split_sync_waits(nc)

tile_mod.TileContext._drain_and_barrier = _patched_drain_and_barrier

# ---------------------------------------------------------------- constants
NUM_USERS, NUM_ITEMS, D, SEM, HID, K = 60000, 30000, 64, 1024, 32, 32
N = NUM_USERS + NUM_ITEMS             # 90000
NPAD = 90112                          # 704*128
NCORE = 8
DPC = NPAD // NCORE                   # 11264 dest rows per core
WIN = 512                             # dests per window (one psum bank f32)
NWIN1 = DPC // WIN                    # 22 windows, layer-1
SWC = 64                              # psum span per regular matmul column
NRANGE = 3
RS = np.array([0, 32768, 65536, NPAD], dtype=np.int64)   # table range bounds
RROWS = [32768, 32768, 24576]
NI_MAX = 896                          # idx per gather instruction (ring cap)
TSTride = 128                         # table row stride (f16 elems) = 256B

UPC = NUM_USERS // NCORE              # 7500
IPC = NUM_ITEMS // NCORE              # 3750
IPAD = 3840
NBLK = IPAD // 128                    # 30
SUBB = 3                              # item blocks per attention sub-batch
NSUB = NBLK // SUBB                   # 10
GC = SUBB * K                         # 96 gather cols per K2 sub-batch
B = 4096
BPC = B // NCORE                      # 512

_BUILT = {}


def _dma_gather(g, out_ap, in_ap, idxs_ap, num_idxs, elem_size, elem_step,
                regs=None):
    """dma_gather with payload < stride (bass's public API asserts
    elem_size%256B which is only a stride requirement). regs: dict caching
    one GPSIMD register per distinct num_idxs value."""
    _in_ap = g.lower_ap_dma(in_ap, for_custom_bir_dma=True)
    _idxs_ap = g.lower_ap(idxs_ap)
    _out_ap = g.lower_ap(out_ap)
    if regs is None:
        reg = g.to_reg(num_idxs)
    else:
        if num_idxs not in regs:
            regs[num_idxs] = g.to_reg(num_idxs)
        reg = regs[num_idxs]
    stride_bytes = elem_step * mybir.dt.size(in_ap.dtype)
    assert stride_bytes % 256 == 0
    return g.add_instruction(
        mybir.InstDMAGatherAnt(
            name=g.bass.get_next_instruction_name(),
            ins=[*_in_ap, _idxs_ap, g.lower_val_access(reg)],
            outs=[_out_ap],
            transpose=False, num_idxs=num_idxs, elem_size=elem_size,
            stride_bytes_256=stride_bytes // 256, gen_mode=0,
            single_packet=True, queue_num=0,
            sbuf_tokens_per_rank=0, sbuf_free_dim_per_rank=0,
            sbuf_free_dim_pad_per_rank=0, sbuf_byte_offset=0,
        ))


def _elu(nc, pool, out_ap, in_ap, shape, tag):
    """out = elu(in) = max(x,0) + exp(min(x,0)) - 1   (no Elu in ACT table)."""
    mn = pool.tile(shape, F32, tag=tag + "_mn")
    nc.vector.tensor_scalar_min(mn[:], in_ap, 0.0)
    ex = pool.tile(shape, F32, tag=tag + "_ex")
    nc.scalar.activation(ex[:], mn[:], AF.Exp, scale=1.0)
    mx = pool.tile(shape, F32, tag=tag + "_mx")
    nc.vector.tensor_scalar_max(mx[:], in_ap, 0.0)
    nc.vector.tensor_add(out_ap, mx[:], ex[:])
    nc.vector.tensor_scalar_add(out_ap, out_ap, -1.0)


# ================================================================ K1: GEMM
def _build_k1():
    nc = bass.Bass("TRN2", target_bir_lowering=False)
    xu = nc.dram_tensor("xu", [SEM, UPC], F16, kind="ExternalInput")
    xi = nc.dram_tensor("xi", [SEM, IPC], F16, kind="ExternalInput")
    wu = nc.dram_tensor("wu", [SEM, 64], F16, kind="ExternalInput")
    wi = nc.dram_tensor("wi", [SEM, 66], F16, kind="ExternalInput")
    bu = nc.dram_tensor("bu", [64, 1], F32, kind="ExternalInput")
    bi = nc.dram_tensor("bi", [66, 1], F32, kind="ExternalInput")
    eu = nc.dram_tensor("eu", [64, UPC], F32, kind="ExternalInput")
    ei = nc.dram_tensor("ei", [64, IPC], F32, kind="ExternalInput")
    ou = nc.dram_tensor("ou", [64, UPC], F32, kind="ExternalOutput")
    oi = nc.dram_tensor("oi", [64, IPC], F32, kind="ExternalOutput")
    os12 = nc.dram_tensor("os12", [2, IPC], F32, kind="ExternalOutput")

    with TileContext(nc) as tc:
        with tc.tile_pool(name="w", bufs=1) as wp, \
             tc.tile_pool(name="x", bufs=3) as xp, \
             tc.tile_pool(name="o", bufs=2) as op, \
             tc.tile_pool(name="ps", bufs=2, space="PSUM") as pp:
            wu_sb = wp.tile([128, SEM // 128, 64], F16, tag="wu")
            nc.sync.dma_start(wu_sb[:], wu[:].rearrange("(a p) m -> p a m", p=128))
            wi_sb = wp.tile([128, SEM // 128, 66], F16, tag="wi")
            nc.sync.dma_start(wi_sb[:], wi[:].rearrange("(a p) m -> p a m", p=128))
            bu_sb = wp.tile([64, 1], F32, tag="bu")
            nc.sync.dma_start(bu_sb[:], bu[:])
            bi_sb = wp.tile([66, 1], F32, tag="bi")
            nc.sync.dma_start(bi_sb[:], bi[:])

            def gemm(xten, eten, wtile, btile, oten, m, rows, RL, RM, s12=None):
                # RL: DMA load tile; RM: matmul tile (psum bank limit 512 f32)
                for t in range(rows // RL):
                    xt = xp.tile([128, SEM // 128, RL], F16, tag="xt")
                    nc.sync.dma_start(
                        xt[:], xten[:, t * RL:(t + 1) * RL].rearrange("(a p) r -> p a r", p=128))
                    et = op.tile([64, RL], F32, tag="et")
                    nc.sync.dma_start(et[:], eten[:, t * RL:(t + 1) * RL])
                    mg = op.tile([64, RL], F32, tag="mg")
                    if s12 is not None:
                        sv = op.tile([2, RL], F32, tag="sv")
                    else:
                        sv = None
                    for q in range(RL // RM):
                        ps = pp.tile([m, RM], F32, tag="ps")
                        for kk in range(SEM // 128):
                            nc.tensor.matmul(ps[:], wtile[:, kk, :],
                                             xt[:, kk, q * RM:(q + 1) * RM],
                                             start=(kk == 0), stop=(kk == SEM // 128 - 1))
                        xb = op.tile([64, RM], F32, tag="xb")
                        nc.vector.tensor_scalar_add(xb[:], ps[0:64, :], btile[0:64, :])
                        _elu(nc, op, mg[:, q * RM:(q + 1) * RM], xb[:], [64, RM], "e1")
                        if s12 is not None:
                            nc.scalar.copy(sv[:, q * RM:(q + 1) * RM], ps[64:66, :])
                    nc.vector.tensor_add(mg[:], mg[:], et[:])
                    nc.scalar.mul(mg[:], mg[:], 0.5)
                    nc.sync.dma_start(oten[:, t * RL:(t + 1) * RL], mg[:])
                    if s12 is not None:
                        nc.sync.dma_start(s12[:, t * RL:(t + 1) * RL], sv[:])

            gemm(xu, eu, wu_sb, bu_sb, ou, 64, UPC, 1500, 500)
            gemm(xi, ei, wi_sb, bi_sb, oi, 66, IPC, 750, 375, s12=os12)
    return nc


# ================================================================ K2: attention
# Item table rows: 128 f16 (256B): [emb 0:64 | s1 @64 | pad]. Payload 66.
PAY2 = 66

def _build_k2():
    nc = bass.Bass("TRN2", target_bir_lowering=False)
    tbl = nc.dram_tensor("tbl", [NUM_ITEMS, TSTride], F16, kind="ExternalInput")
    adji = nc.dram_tensor("adji", [128, (NBLK * K * 128) // 16], I16, kind="ExternalInput")
    s2r = nc.dram_tensor("s2r", [128, NBLK * K], F32, kind="ExternalInput")
    itm = nc.dram_tensor("itm", [128, NBLK * 64], F32, kind="ExternalInput")
    oit = nc.dram_tensor("oit", [128, NBLK * 64], F16, kind="ExternalOutput")

    with TileContext(nc) as tc:
        with tc.tile_pool(name="g", bufs=3) as gp, \
             tc.tile_pool(name="t", bufs=3) as tp, \
             tc.tile_pool(name="s", bufs=1) as sp, \
             tc.tile_pool(name="m", bufs=3) as mp:
            nc.gpsimd.load_library(library_config.mlp)
            adj_sb = sp.tile([128, (NBLK * K * 128) // 16], I16, tag="adj")
            nc.sync.dma_start(adj_sb[:], adji[:])
            s2_sb = sp.tile([128, NBLK * K], F32, tag="s2")
            nc.sync.dma_start(s2_sb[:], s2r[:])
            itm_sb = sp.tile([128, NBLK * 64], F32, tag="itm")
            nc.sync.dma_start(itm_sb[:], itm[:])
            niregs = {}
            for u in range(NSUB):
                g = gp.tile([128, GC, PAY2], F16, tag="g")
                # GC*128 = 12288 idx in chunks of 896 (7 cols)
                base16 = u * (GC * 128) // 16
                col = 0
                left = GC * 128
                while left > 0:
                    n = min(NI_MAX, left)
                    _dma_gather(nc.gpsimd, g[:, col:col + n // 128, :], tbl[:],
                                adj_sb[:, base16:base16 + n // 16], n, PAY2, TSTride,
                                regs=niregs)
                    base16 += n // 16
                    col += n // 128
                    left -= n
                s1f = mp.tile([128, GC], F32, tag="s1f")
                nc.scalar.copy(s1f[:], g[:, :, 64])
                lg = mp.tile([128, GC], F32, tag="lg")
                nc.vector.tensor_add(lg[:], s1f[:], s2_sb[:, u * GC:(u + 1) * GC])
                lr = mp.tile([128, GC], F32, tag="lr")
                nc.scalar.mul(lr[:], lg[:], 0.2)
                nc.vector.tensor_max(lg[:], lg[:], lr[:])
                ex = mp.tile([128, SUBB, K], F32, tag="ex")
                nc.scalar.activation(ex[:].rearrange("p a b -> p (a b)"), lg[:],
                                     AF.Exp, scale=1.0)
                sm = mp.tile([128, SUBB], F32, tag="sm")
                nc.vector.reduce_sum(sm[:], ex[:], axis=mybir.AxisListType.X)
                nc.vector.reciprocal(sm[:], sm[:])
                att = mp.tile([128, SUBB, K], F16, tag="att")
                for bb in range(SUBB):
                    nc.vector.tensor_scalar_mul(att[:, bb, :], ex[:, bb, :], sm[:, bb:bb + 1])
                tmp = tp.tile([128, SUBB, K, 64], F16, tag="tmp")
                av = att[:]
                att_b = bass.AP(av.tensor, av.offset, list(av.ap) + [[0, 64]])
                nc.vector.tensor_mul(
                    tmp[:], g[:].rearrange("p (b k) d -> p b k d", b=SUBB)[:, :, :, 0:64],
                    att_b)
                hp = mp.tile([128, SUBB, 64], F32, tag="hp")
                nc.vector.reduce_sum(hp[:], tmp[:].rearrange("p b k d -> p b d k"),
                                     axis=mybir.AxisListType.X)
                he = mp.tile([128, SUBB * 64], F32, tag="he")
                _elu(nc, mp, he[:], hp[:].rearrange("p b d -> p (b d)"),
                     [128, SUBB * 64], "e2")
                fo = mp.tile([128, SUBB * 64], F32, tag="fo")
                nc.vector.tensor_add(fo[:], he[:],
                                     itm_sb[:, u * SUBB * 64:(u + 1) * SUBB * 64])
                fo16 = mp.tile([128, SUBB * 64], F16, tag="fo16")
                nc.scalar.mul(fo16[:], fo[:], 0.5)
                nc.sync.dma_start(oit[:, u * SUBB * 64:(u + 1) * SUBB * 64], fo16[:])
    return nc


# ================================================================ K3: spmm
def _build_k3(meta):
    """SpMM layer kernel from packing metadata.

    meta: nwin, cap[w][r] (regular slots), ex[w][r] (extra cols),
          off[w][r] = list of per-column psum offsets.
    Stream layout per (w, r): [extra cols | regular cols]; per window the
    first extra of r0 initializes psum (start=True); last regular matmul
    of the last nonempty range carries stop=True.
    """
    nwin = meta["nwin"]
    cap = meta["cap"]; ex = meta["ex"]; off = meta["off"]
    totslots = int(sum(cap[w][r] + 128 * ex[w][r]
                       for w in range(nwin) for r in range(NRANGE)))
    totregcol = int(sum(cap[w][r] // 128 for w in range(nwin) for r in range(NRANGE)))
    totexcol = int(sum(ex[w][r] for w in range(nwin) for r in range(NRANGE)))
    maxwcol = max(sum(cap[w][r] // 128 + ex[w][r] for r in range(NRANGE))
                  for w in range(nwin))
    maxwreg = max(sum(cap[w][r] // 128 for r in range(NRANGE)) for w in range(nwin))
    maxwex = max(sum(ex[w][r] for r in range(NRANGE)) for w in range(nwin))

    nc = bass.Bass("TRN2", target_bir_lowering=False)
    tbls = [nc.dram_tensor(f"tbl{r}", [RROWS[r], TSTride], F16, kind="ExternalInput")
            for r in range(NRANGE)]
    idx = nc.dram_tensor("idx", [128, totslots // 16], I16, kind="ExternalInput")
    wreg = nc.dram_tensor("wreg", [128, totregcol * SWC], F16, kind="ExternalInput")
    wext = nc.dram_tensor("wext", [128, max(totexcol, 1) * WIN], F16, kind="ExternalInput")
    out = nc.dram_tensor("out", [64, nwin * WIN], F16, kind="ExternalOutput")

    with TileContext(nc) as tc:
        with tc.tile_pool(name="s", bufs=1) as sp, \
             tc.tile_pool(name="g", bufs=3) as gp, \
             tc.tile_pool(name="w", bufs=3) as wp, \
             tc.tile_pool(name="o", bufs=3) as op, \
             tc.tile_pool(name="ps", bufs=4, space="PSUM") as pp:
            nc.gpsimd.load_library(library_config.mlp)
            idx_sb = sp.tile([128, totslots // 16], I16, tag="idx")
            nc.sync.dma_start(idx_sb[:], idx[:])
            niregs = {}
            i16 = 0          # cursor into idx (units of 16 slots)
            rcol = 0         # cursor into wreg (regular col index)
            ecol = 0         # cursor into wext (extra col index)
            for w in range(nwin):
                wcols = sum(cap[w][r] // 128 + ex[w][r] for r in range(NRANGE))
                wregc = sum(cap[w][r] // 128 for r in range(NRANGE))
                wexc = sum(ex[w][r] for r in range(NRANGE))
                gt = gp.tile([128, maxwcol, SWC], F16, tag="g")
                wr = wp.tile([128, maxwreg * SWC], F16, tag="wr")
                nc.sync.dma_start(wr[:, 0:wregc * SWC],
                                  wreg[:, rcol * SWC:(rcol + wregc) * SWC])
                if maxwex:
                    we = wp.tile([128, max(maxwex, 1) * WIN], F16, tag="we")
                    if wexc:
                        nc.sync.dma_start(we[:, 0:wexc * WIN],
                                          wext[:, ecol * WIN:(ecol + wexc) * WIN])
                # gathers for the whole window (extras first per range)
                col = 0
                colmap = []   # per range: (excolbase, regcolbase)
                for r in range(NRANGE):
                    nsl = cap[w][r] + 128 * ex[w][r]
                    colmap.append((col, col + ex[w][r]))
                    left = nsl
                    while left > 0:
                        n = min(NI_MAX, left)
                        _dma_gather(nc.gpsimd, gt[:, col:col + n // 128, :], tbls[r][:],
                                    idx_sb[:, i16:i16 + n // 16], n, SWC, TSTride,
                                    regs=niregs)
                        i16 += n // 16
                        col += n // 128
                        left -= n
                # matmuls
                ps = pp.tile([64, WIN], F32, tag="ps")
                first = True
                wrc = 0
                wec = 0
                last_r = max(r for r in range(NRANGE) if cap[w][r] > 0)
                for r in range(NRANGE):
                    exbase, regbase = colmap[r]
                    for e in range(ex[w][r]):
                        nc.tensor.matmul(ps[:], gt[:, exbase + e, :],
                                         we[:, wec * WIN:(wec + 1) * WIN],
                                         start=first, stop=False)
                        first = False
                        wec += 1
                    ncols = cap[w][r] // 128
                    for j in range(ncols):
                        o = off[w][r][j]
                        stop = (r == last_r and j == ncols - 1)
                        nc.tensor.matmul(ps[:, o:o + SWC], gt[:, regbase + j, :],
                                         wr[:, wrc * SWC:(wrc + 1) * SWC],
                                         start=first, stop=stop)
                        first = False
                        wrc += 1
                rcol += wregc
                ecol += wexc
                ot = op.tile([64, WIN], F16, tag="ot")
                nc.scalar.copy(ot[:], ps[:])
                nc.sync.dma_start(out[:, w * WIN:(w + 1) * WIN], ot[:])
    return nc


# ================================================================ host packing
def _pack_edges(core, pos, src, val, nwin):
    """Pack edges (dest position pos within core, source node src) into the
    per-(window, range) gather/weight layout. Returns per-core arrays + meta."""
    w = pos // WIN
    drel = (pos - w * WIN).astype(np.int64)
    rg = np.searchsorted(RS, src, side="right") - 1
    src_rel = (src - RS[rg]).astype(np.int64)
    order = np.lexsort((drel, rg, w, core))
    core, w, drel, rg, src_rel, val = (a[order] for a in (core, w, drel, rg, src_rel, val))

    key = (core * nwin + w) * NRANGE + rg
    cnt = np.bincount(key, minlength=NCORE * nwin * NRANGE).reshape(NCORE, nwin, NRANGE)
    cap = ((cnt.max(axis=0) + 127) // 128) * 128          # [nwin, NRANGE]
    cap = np.maximum(cap, 128)                            # every cell >= 1 col

    n = len(key)
    ar = np.arange(n)
    first = np.empty(n, bool); first[0] = True; first[1:] = key[1:] != key[:-1]
    slot = ar - np.maximum.accumulate(np.where(first, ar, 0))

    ncol = cap >> 7
    regcol_base = np.concatenate([[0], np.cumsum(ncol.reshape(-1))])[:-1]\
        .reshape(nwin, NRANGE)
    totregcol = int(ncol.sum())

    # data-driven column offsets: cover the across-core [min, max] dest range
    colj = slot >> 7
    gcol = regcol_base[w, rg] + colj
    lo = np.full(totregcol, WIN, np.int64)
    hi = np.full(totregcol, -1, np.int64)
    np.minimum.at(lo, gcol, drel)
    np.maximum.at(hi, gcol, drel)
    lo = np.minimum(lo, hi)                               # empty cols -> [hi,hi]
    offcol = np.clip((lo + hi + 1 - SWC) // 2, 0, WIN - SWC)
    off_e = offcol[gcol]
    spill = (drel < off_e) | (drel >= off_e + SWC)

    # extra column counts (same for all cores)
    skey = key[spill]
    scnt = np.bincount(skey, minlength=NCORE * nwin * NRANGE).reshape(NCORE, nwin, NRANGE)
    exc = (scnt.max(axis=0) + 127) // 128                 # [nwin, NRANGE] cols
    exc[:, 0] = np.maximum(exc[:, 0], 1)                  # psum initializer

    # per-(w,r) stream slot count and bases (same all cores)
    cell_slots = cap + 128 * exc                          # [nwin, NRANGE]
    cell_base = np.concatenate([[0], np.cumsum(cell_slots.reshape(-1))])[:-1]\
        .reshape(nwin, NRANGE)                            # base within core stream
    tot = int(cell_slots.sum())
    excol_base = np.concatenate([[0], np.cumsum(exc.reshape(-1))])[:-1]\
        .reshape(nwin, NRANGE)
    totexcol = int(exc.sum())

    idx_flat = np.zeros((NCORE, tot), np.int16)
    wreg = np.zeros((NCORE, 128, totregcol * SWC), np.float16)
    wext = np.zeros((NCORE, 128, max(totexcol, 1) * WIN), np.float16)

    # regular slots: stream position = cell_base + 128*exc (extras first) + slot
    spos = cell_base[w, rg] + 128 * exc[w, rg] + slot
    idx_flat[core, spos] = src_rel.astype(np.int16)
    reg = ~spill
    fw = (core[reg] * 128 + (slot[reg] & 127)) * (totregcol * SWC) \
        + (regcol_base[w[reg], rg[reg]] + colj[reg]) * SWC + (drel[reg] - off_e[reg])
    wreg.reshape(-1)[fw] = val[reg]

    # spilled edges -> extra slots (their regular slot stays as weight-0 pad)
    if spill.any():
        sc, sw_, srg, ssrc, sdrel, sval = (a[spill] for a in (core, w, rg, src_rel, drel, val))
        ns = len(sc)
        ars = np.arange(ns)
        sfirst = np.empty(ns, bool); sfirst[0] = True; sfirst[1:] = skey[1:] != skey[:-1]
        eslot = ars - np.maximum.accumulate(np.where(sfirst, ars, 0))
        espos = cell_base[sw_, srg] + eslot
        idx_flat[sc, espos] = ssrc.astype(np.int16)
        fx = (sc * 128 + (eslot & 127)) * (max(totexcol, 1) * WIN) \
            + (excol_base[sw_, srg] + (eslot >> 7)) * WIN + sdrel
        wext.reshape(-1)[fx] = sval

    # idx stream -> [128, tot/16] wrapped+replicated layout
    idx_arr = np.tile(idx_flat.reshape(NCORE, tot // 16, 16).transpose(0, 2, 1),
                      (1, 8, 1))

    off_tab = [[list(int(offcol[regcol_base[w_, r_] + j_])
                     for j_ in range(ncol[w_][r_]))
                for r_ in range(NRANGE)] for w_ in range(nwin)]
    meta = {"nwin": nwin,
            "cap": [[int(cap[w_][r_]) for r_ in range(NRANGE)] for w_ in range(nwin)],
            "ex": [[int(exc[w_][r_]) for r_ in range(NRANGE)] for w_ in range(nwin)],
            "off": off_tab}
    return idx_arr, wreg, wext, meta


def _edges_for_lists(rows, cols, vals, dlist):
    """Expand: for each core and each listed dest (position p in dlist[c]),
    all incoming edges. Returns (core, pos, src, val). dlist entries >= N are
    edgeless sentinels."""
    order0 = np.argsort(rows, kind="stable")
    rs, cs, vs = rows[order0], cols[order0], vals[order0]
    row_start = np.searchsorted(rs, np.arange(N + 1))
    rr = dlist.reshape(-1).astype(np.int64)
    rrc = np.minimum(rr, N)
    cnt = row_start[np.minimum(rrc + 1, N)] - row_start[rrc]
    cnt[rr >= N] = 0
    ent = np.repeat(np.arange(len(rr)), cnt)
    ofs = np.arange(len(ent)) - np.repeat(np.cumsum(cnt) - cnt, cnt)
    srcidx = row_start[rrc][ent] + ofs
    ndst = dlist.shape[1]
    e_core = ent // ndst
    e_pos = ent - e_core * ndst
    return e_core, e_pos, cs[srcidx], vs[srcidx]


def _prep(inputs):
    p = {}
    users = np.asarray(inputs["users"]);   items = np.asarray(inputs["items"])
    adj = np.asarray(inputs["adj_matrix"])
    rows = np.asarray(inputs["graph_rows"]).astype(np.int64)
    cols = np.asarray(inputs["graph_cols"]).astype(np.int64)
    vals = np.asarray(inputs["graph_vals"]).astype(np.float32)
    W_att = np.asarray(inputs["W_att"]); a_att = np.asarray(inputs["a_att"])
    v1 = W_att @ a_att[:HID, 0]; v2 = W_att @ a_att[HID:, 0]

    p["xu"] = np.ascontiguousarray(np.asarray(inputs["user_semantic_emb"]).astype(np.float16).T)
    p["xi"] = np.ascontiguousarray(np.asarray(inputs["semantic_emb"]).astype(np.float16).T)
    p["wu"] = np.asarray(inputs["W_usem"]).astype(np.float16)
    p["wi"] = np.concatenate([np.asarray(inputs["W_sem"]), v1[:, None], v2[:, None]],
                             axis=1).astype(np.float16)
    p["bu"] = np.asarray(inputs["b_usem"]).reshape(64, 1)
    p["bi"] = np.concatenate([np.asarray(inputs["b_sem"]), np.zeros(2, np.float32)]
                             ).reshape(66, 1).astype(np.float32)
    p["eu"] = np.ascontiguousarray(np.asarray(inputs["emb_user"]).T)
    p["ei"] = np.ascontiguousarray(np.asarray(inputs["emb_item"]).T)

    # K2 neighbor indices: gather slot i (of GC*128 per sub-batch) = col*128+p
    # -> adj[block b = (u*SUBB + col//K), item p, k = col%K]
    adj_pad = np.zeros((NCORE, IPAD, K), np.int64)
    for c in range(NCORE):
        adj_pad[c, :IPC] = adj[c * IPC:(c + 1) * IPC]
    slot_idx = np.transpose(adj_pad.reshape(NCORE, NBLK, 128, K), (0, 2, 1, 3))
    # flat stream per core: for sub-batch u, col cc, slot p: value adj[..]
    flat = np.transpose(adj_pad.reshape(NCORE, NBLK, 128, K), (0, 1, 3, 2))\
        .reshape(NCORE, NBLK * K * 128)            # (b, k) cols x 128 slots
    p["adji"] = np.tile(flat.reshape(NCORE, -1, 16).transpose(0, 2, 1),
                        (1, 8, 1)).astype(np.int16)

    # layer 1: all NPAD dests
    e_core = rows // DPC
    e_pos = rows - e_core * DPC
    p["l1"] = _pack_edges(e_core, e_pos, cols, vals, NWIN1)

    # batch dest list (layer 3 + K4)
    dlist = np.stack([np.concatenate([
        users[c * BPC:(c + 1) * BPC].astype(np.int64),
        items[c * BPC:(c + 1) * BPC].astype(np.int64) + NUM_USERS])
        for c in range(NCORE)])
    NWIN3 = (2 * BPC) // WIN                       # 2
    ec, ep, es, ev = _edges_for_lists(rows, cols, vals, dlist)
    p["l3"] = _pack_edges(ec, ep, es, ev, NWIN3)
    p["nwin3"] = NWIN3

    # layer 2 rows needed: sources of layer-3 edges + K4's rows
    need = np.zeros(N, bool)
    need[es] = True
    need[dlist.reshape(-1)] = True
    R2 = np.nonzero(need)[0]
    Lc = -(-len(R2) // NCORE)
    L2 = -(-Lc // WIN) * WIN
    NWIN2 = L2 // WIN
    lists2 = np.full((NCORE, L2), N, np.int64)
    for c in range(NCORE):
        seg = R2[c * Lc:(c + 1) * Lc]
        lists2[c, :len(seg)] = seg
    p["lists2"] = lists2
    ec, ep, es2, ev2 = _edges_for_lists(rows, cols, vals, lists2)
    p["l2"] = _pack_edges(ec, ep, es2, ev2, NWIN2)
    p["nwin2"] = NWIN2

    p["k4"] = []
    for c in range(NCORE):
        u = users[c * BPC:(c + 1) * BPC].astype(np.int64)
        it = items[c * BPC:(c + 1) * BPC].astype(np.int64) + NUM_USERS
        rws = np.concatenate([u, it])
        p["k4"].append(np.ascontiguousarray(rws.reshape(8, 128).T.astype(np.int32)))
    return p


# ================================================================ K4: final
def _build_k4():
    nc = bass.Bass("TRN2", target_bir_lowering=False)
    tb012 = nc.dram_tensor("tb012", [NPAD, 192], F16, kind="ExternalInput")
    tb3d = nc.dram_tensor("tb3d", [128, 8 * 64], F16, kind="ExternalInput")
    fidx = nc.dram_tensor("fidx", [128, 8], I32, kind="ExternalInput")
    out = nc.dram_tensor("out", [128, 4], F32, kind="ExternalOutput")

    with TileContext(nc) as tc:
        with tc.tile_pool(name="g", bufs=2) as gp, \
             tc.tile_pool(name="m", bufs=1) as mp:
            it = mp.tile([128, 8], I32, tag="it")
            nc.sync.dma_start(it[:], fidx[:])
            t3 = mp.tile([128, 8, 64], F16, tag="t3")
            nc.sync.dma_start(t3[:].rearrange("p a b -> p (a b)"), tb3d[:])
            acc = mp.tile([128, 8, 64], F32, tag="acc")
            nc.scalar.copy(acc[:], t3[:])
            g = gp.tile([128, 8, 192], F16, tag="g")
            for t in range(8):
                nc.gpsimd.indirect_dma_start(
                    out=g[:, t, :], out_offset=None, in_=tb012[:],
                    in_offset=bass.IndirectOffsetOnAxis(ap=it[:, t:t + 1], axis=0))
            for sl in range(3):
                gf = gp.tile([128, 8, 64], F32, tag="gf")
                nc.scalar.copy(gf[:], g[:].rearrange("p a (s b) -> p a s b", s=3)[:, :, sl, :])
                nc.vector.tensor_add(acc[:], acc[:], gf[:])
            nc.scalar.mul(acc[:], acc[:], 0.25)
            prod = mp.tile([128, 4, 64], F32, tag="prod")
            nc.vector.tensor_mul(prod[:], acc[:, 0:4, :], acc[:, 4:8, :])
            res = mp.tile([128, 4], F32, tag="res")
            nc.vector.reduce_sum(res[:], prod[:], axis=mybir.AxisListType.X)
            nc.sync.dma_start(out[:], res[:])
    return nc


_META = {}

def _run(name, builder, in_maps, meta_key=None):
    if name not in _BUILT or _META.get(name) != meta_key:
        nc = builder()
        mybir.codegen_inst_isa_subclasses(nc)
        _BUILT[name] = nc
        _META[name] = meta_key
    return bass_utils.run_bass_kernel_spmd(
        _BUILT[name], in_maps, core_ids=list(range(NCORE))).results


def _tables_from_nodes(node_tbl):
    """node_tbl [NPAD, 64] f16 -> 3 range tables [rows, 128] f16."""
    full = np.zeros((NPAD, TSTride), np.float16)
    full[:, 0:64] = node_tbl
    return [np.ascontiguousarray(full[RS[r]:RS[r + 1]]) for r in range(NRANGE)]


def kernel(**inputs):
    p = _prep(inputs)

    # ---------------- K1
    maps = [{
        "xu": p["xu"][:, c * UPC:(c + 1) * UPC],
        "xi": p["xi"][:, c * IPC:(c + 1) * IPC],
        "wu": p["wu"], "wi": p["wi"], "bu": p["bu"], "bi": p["bi"],
        "eu": p["eu"][:, c * UPC:(c + 1) * UPC],
        "ei": p["ei"][:, c * IPC:(c + 1) * IPC],
    } for c in range(NCORE)]
    r1 = _run("k1", _build_k1, maps)
    users_m = np.concatenate([r1[c]["ou"].T for c in range(NCORE)], 0)
    items_m = np.concatenate([r1[c]["oi"].T for c in range(NCORE)], 0)
    s1 = np.concatenate([r1[c]["os12"][0] for c in range(NCORE)])
    s2 = np.concatenate([r1[c]["os12"][1] for c in range(NCORE)])

    # ---------------- K2
    tblA = np.zeros((NUM_ITEMS, TSTride), np.float16)
    tblA[:, 0:64] = items_m
    tblA[:, 64] = s1
    maps = []
    for c in range(NCORE):
        s2c = np.zeros(IPAD, np.float32)
        s2c[:IPC] = s2[c * IPC:(c + 1) * IPC]
        s2r = np.transpose(np.broadcast_to(s2c.reshape(NBLK, 1, 128), (NBLK, K, 128)),
                           (2, 0, 1)).reshape(128, NBLK * K)
        imc = np.zeros((IPAD, 64), np.float32)
        imc[:IPC] = items_m[c * IPC:(c + 1) * IPC]
        itm = np.transpose(imc.reshape(NBLK, 128, 64), (1, 0, 2)).reshape(128, NBLK * 64)
        maps.append({"tbl": tblA, "adji": p["adji"][c],
                     "s2r": np.ascontiguousarray(s2r),
                     "itm": np.ascontiguousarray(itm)})
    r2 = _run("k2", _build_k2, maps)
    items_f = np.zeros((NUM_ITEMS, 64), np.float16)
    for c in range(NCORE):
        o = r2[c]["oit"].reshape(128, NBLK, 64).transpose(1, 0, 2).reshape(IPAD, 64)
        items_f[c * IPC:(c + 1) * IPC] = o[:IPC]

    # ---------------- K3 layers
    def run_spmm(name, pack, tbl_nodes):
        idx_arr, wreg, wext, meta = pack
        tbls = _tables_from_nodes(tbl_nodes)
        maps = [dict({f"tbl{r}": tbls[r] for r in range(NRANGE)},
                     idx=idx_arr[c], wreg=wreg[c], wext=wext[c])
                for c in range(NCORE)]
        mk = (meta["nwin"], tuple(map(tuple, meta["cap"])), tuple(map(tuple, meta["ex"])))
        r = _run(name, lambda: _build_k3(meta), maps, meta_key=mk)
        return r

    e0 = np.zeros((NPAD, 64), np.float16)
    e0[:NUM_USERS] = users_m.astype(np.float16)
    e0[NUM_USERS:N] = items_f

    r = run_spmm("k3", p["l1"], e0)
    e1 = np.zeros((NPAD, 64), np.float16)
    for c in range(NCORE):
        e1[c * DPC:(c + 1) * DPC] = r[c]["out"].T

    r = run_spmm("k3b", p["l2"], e1)
    e2 = np.zeros((NPAD, 64), np.float16)
    lists2 = p["lists2"]
    for c in range(NCORE):
        outc = r[c]["out"].T
        sel = lists2[c] < N
        e2[lists2[c][sel]] = outc[:len(lists2[c])][sel]

    r = run_spmm("k3c", p["l3"], e2)
    tb3d = []
    for c in range(NCORE):
        rowsc = r[c]["out"].T[:2 * BPC]                  # [1024, 64]
        tb3d.append(np.ascontiguousarray(
            rowsc.reshape(8, 128, 64).transpose(1, 0, 2).reshape(128, 8 * 64)))

    # ---------------- K4
    e012 = np.ascontiguousarray(np.concatenate([e0, e1, e2], axis=1))
    maps = [{"tb012": e012, "tb3d": tb3d[c], "fidx": p["k4"][c]}
            for c in range(NCORE)]
    r4 = _run("k4", _build_k4, maps)
    gamma = np.zeros(B, np.float32)
    for c in range(NCORE):
        gamma[c * BPC:(c + 1) * BPC] = r4[c]["out"].T.reshape(BPC)
    return gamma


# revision 12
# speedup vs baseline: 4.2473x; 1.0107x over previous
"""CoLaKG model kernel for 8 Trainium2 NeuronCores (self-contained).

Pipeline (6 bass SPMD launches; host does static prep + mechanical relayout):
  K1 gemm : semantic projections (users+items) + merge, + s1/s2 GAT scalars
  K2 attn : item-neighbor GAT attention (batched dma_gather + softmax + wsum)
  K3 spmm : LightGCN propagation layer 1 (all dests)
  K3b     : layer 2 at the ~87% of rows consumed downstream (list-addressed)
  K3c     : layer 3 at the batch's 2*512 rows per core
  K4 final: gather 4 embedding tables at (user,item) rows, mean, dot product

Perf design: all row gathers use the batched InstDMAGatherAnt (Q7 'mlp'
ucode library) at <=896 indices per instruction, amortizing the ~1us
SWDGE fixed cost ~7x vs per-column indirect DMA. Node tables are stored
at 256B stride with a 128B gathered payload (int16 indices -> 3 table
ranges). The SpMM packs edges per (512-dest window, range) sorted by
dest; each 128-edge column feeds one matmul into a fixed 64-wide psum
span (drift-tolerant placement; stragglers go to window-wide extra
columns; psum initialized by the first wide matmul's start flag).
"""
import copy
import numpy as np

import jax
jax.config.update("jax_compilation_cache_dir", "/tmp/.jax_bass_cache")
jax.config.update("jax_persistent_cache_min_entry_size_bytes", -1)
jax.config.update("jax_persistent_cache_min_compile_time_secs", 0.0)

import concourse.bass as bass
import concourse.mybir as mybir
from concourse.tile import TileContext
from concourse import bass_utils, library_config
import concourse.tile as tile_mod
from concourse.vector_clock import ScopedClock

F32 = mybir.dt.float32
F16 = mybir.dt.float16
I16 = mybir.dt.int16
I32 = mybir.dt.int32
AF = mybir.ActivationFunctionType

# ---------------------------------------------------------------- tile patch
MAX_WAITS = 1

def _split_sync_waits(nc, max_waits=MAX_WAITS):
    template = None
    counter = [0]
    for fn in nc.m.functions:
        for bb in fn.blocks:
            for inst in bb.instructions:
                if type(inst).__name__ == "InstNoOp":
                    template = copy.deepcopy(inst)
                    break
            if template is not None:
                break
        if template is not None:
            break
    for fn in nc.m.functions:
        for bb in fn.blocks:
            il = bb.instructions
            i = 0
            while i < len(il):
                inst = il[i]
                if template is None and type(inst).__name__ == "InstNoOp":
                    template = copy.deepcopy(inst)
                si = inst.sync_info
                if si is not None and si.on_wait is not None and len(si.on_wait) > max_waits:
                    assert template is not None, "no InstNoOp to clone"
                    waits = list(si.on_wait)
                    keep, rest = waits[:max_waits], waits[max_waits:]
                    si.on_wait.clear()
                    for w in keep:
                        si.on_wait.append(w)
                    carriers = []
                    while rest:
                        c = copy.deepcopy(template)
                        counter[0] += 1
                        c.name = f"I-waitsplit-{counter[0]}"
                        c.engine = inst.engine
                        c.sync_info = mybir.SyncInfo(on_wait=list(rest[:max_waits]), on_update=[])
                        carriers.append(c)
                        rest = rest[max_waits:]
                    for k, cinst in enumerate(carriers):
                        try:
                            nc.register_instruction(cinst, overwrite=True)
                        except Exception:
                            pass
                        il.insert(i + k, cinst)
                    i += len(carriers)
                i += 1

def _patched_drain_and_barrier(self, tick_clock, wait_clock):
    nc = self.nc
    nop0 = nc.sync.nop(nofuse=True, hint="predrain_waits")
    wait_clock.add_sem_waits(nop0.ins, ScopedClock({None: tick_clock.global_clock}))
    nc.sync.drain()
    nc.all_engine_barrier()
    assert self.sems is not None
    popped = nc._tile_sem_poison_stack.pop()
    assert popped is self._sem_poison
    nc.clear_and_free_semaphores(list(self.sems.allocated().values()))
    nc.all_engine_barrier()
    _split_sync_waits(nc)

tile_mod.TileContext._drain_and_barrier = _patched_drain_and_barrier

# ---------------------------------------------------------------- constants
NUM_USERS, NUM_ITEMS, D, SEM, HID, K = 60000, 30000, 64, 1024, 32, 32
N = NUM_USERS + NUM_ITEMS             # 90000
NPAD = 90112                          # 704*128
NCORE = 8
DPC = NPAD // NCORE                   # 11264 dest rows per core
WIN = 512                             # dests per window (one psum bank f32)
NWIN1 = DPC // WIN                    # 22 windows, layer-1
SWC = 64                              # psum span per regular matmul column
NRANGE = 3
RS = np.array([0, 32768, 65536, NPAD], dtype=np.int64)   # table range bounds
RROWS = [32768, 32768, 24576]
NI_MAX = 896                          # idx per gather instruction (ring cap)
TSTride = 128                         # table row stride (f16 elems) = 256B

UPC = NUM_USERS // NCORE              # 7500
IPC = NUM_ITEMS // NCORE              # 3750
IPAD = 3840
NBLK = IPAD // 128                    # 30
SUBB = 6                              # item blocks per attention sub-batch
NSUB = NBLK // SUBB                   # 10
GC = SUBB * K                         # 96 gather cols per K2 sub-batch
B = 4096
BPC = B // NCORE                      # 512

_BUILT = {}


def _dma_gather(g, out_ap, in_ap, idxs_ap, num_idxs, elem_size, elem_step,
                regs=None):
    """dma_gather with payload < stride (bass's public API asserts
    elem_size%256B which is only a stride requirement). regs: dict caching
    one GPSIMD register per distinct num_idxs value."""
    _in_ap = g.lower_ap_dma(in_ap, for_custom_bir_dma=True)
    _idxs_ap = g.lower_ap(idxs_ap)
    _out_ap = g.lower_ap(out_ap)
    if regs is None:
        reg = g.to_reg(num_idxs)
    else:
        if num_idxs not in regs:
            regs[num_idxs] = g.to_reg(num_idxs)
        reg = regs[num_idxs]
    stride_bytes = elem_step * mybir.dt.size(in_ap.dtype)
    assert stride_bytes % 256 == 0
    return g.add_instruction(
        mybir.InstDMAGatherAnt(
            name=g.bass.get_next_instruction_name(),
            ins=[*_in_ap, _idxs_ap, g.lower_val_access(reg)],
            outs=[_out_ap],
            transpose=False, num_idxs=num_idxs, elem_size=elem_size,
            stride_bytes_256=stride_bytes // 256, gen_mode=0,
            single_packet=True, queue_num=0,
            sbuf_tokens_per_rank=0, sbuf_free_dim_per_rank=0,
            sbuf_free_dim_pad_per_rank=0, sbuf_byte_offset=0,
        ))


def _elu(nc, pool, out_ap, in_ap, shape, tag):
    """out = elu(in) = max(x,0) + exp(min(x,0)) - 1   (no Elu in ACT table)."""
    mn = pool.tile(shape, F32, tag=tag + "_mn")
    nc.vector.tensor_scalar_min(mn[:], in_ap, 0.0)
    ex = pool.tile(shape, F32, tag=tag + "_ex")
    nc.scalar.activation(ex[:], mn[:], AF.Exp, scale=1.0)
    mx = pool.tile(shape, F32, tag=tag + "_mx")
    nc.vector.tensor_scalar_max(mx[:], in_ap, 0.0)
    nc.vector.tensor_add(out_ap, mx[:], ex[:])
    nc.vector.tensor_scalar_add(out_ap, out_ap, -1.0)


# ================================================================ K1: GEMM
def _build_k1():
    nc = bass.Bass("TRN2", target_bir_lowering=False)
    xu = nc.dram_tensor("xu", [SEM, UPC], F16, kind="ExternalInput")
    xi = nc.dram_tensor("xi", [SEM, IPC], F16, kind="ExternalInput")
    wu = nc.dram_tensor("wu", [SEM, 64], F16, kind="ExternalInput")
    wi = nc.dram_tensor("wi", [SEM, 66], F16, kind="ExternalInput")
    bu = nc.dram_tensor("bu", [64, 1], F32, kind="ExternalInput")
    bi = nc.dram_tensor("bi", [66, 1], F32, kind="ExternalInput")
    eu = nc.dram_tensor("eu", [64, UPC], F32, kind="ExternalInput")
    ei = nc.dram_tensor("ei", [64, IPC], F32, kind="ExternalInput")
    ou = nc.dram_tensor("ou", [64, UPC], F32, kind="ExternalOutput")
    oi = nc.dram_tensor("oi", [64, IPC], F32, kind="ExternalOutput")
    os12 = nc.dram_tensor("os12", [2, IPC], F32, kind="ExternalOutput")

    with TileContext(nc) as tc:
        with tc.tile_pool(name="w", bufs=1) as wp, \
             tc.tile_pool(name="x", bufs=3) as xp, \
             tc.tile_pool(name="o", bufs=2) as op, \
             tc.tile_pool(name="ps", bufs=2, space="PSUM") as pp:
            wu_sb = wp.tile([128, SEM // 128, 64], F16, tag="wu")
            nc.sync.dma_start(wu_sb[:], wu[:].rearrange("(a p) m -> p a m", p=128))
            wi_sb = wp.tile([128, SEM // 128, 66], F16, tag="wi")
            nc.sync.dma_start(wi_sb[:], wi[:].rearrange("(a p) m -> p a m", p=128))
            bu_sb = wp.tile([64, 1], F32, tag="bu")
            nc.sync.dma_start(bu_sb[:], bu[:])
            bi_sb = wp.tile([66, 1], F32, tag="bi")
            nc.sync.dma_start(bi_sb[:], bi[:])

            def gemm(xten, eten, wtile, btile, oten, m, rows, RL, RM, s12=None):
                # RL: DMA load tile; RM: matmul tile (psum bank limit 512 f32)
                for t in range(rows // RL):
                    xt = xp.tile([128, SEM // 128, RL], F16, tag="xt")
                    nc.sync.dma_start(
                        xt[:], xten[:, t * RL:(t + 1) * RL].rearrange("(a p) r -> p a r", p=128))
                    et = op.tile([64, RL], F32, tag="et")
                    nc.sync.dma_start(et[:], eten[:, t * RL:(t + 1) * RL])
                    mg = op.tile([64, RL], F32, tag="mg")
                    if s12 is not None:
                        sv = op.tile([2, RL], F32, tag="sv")
                    else:
                        sv = None
                    for q in range(RL // RM):
                        ps = pp.tile([m, RM], F32, tag="ps")
                        for kk in range(SEM // 128):
                            nc.tensor.matmul(ps[:], wtile[:, kk, :],
                                             xt[:, kk, q * RM:(q + 1) * RM],
                                             start=(kk == 0), stop=(kk == SEM // 128 - 1))
                        xb = op.tile([64, RM], F32, tag="xb")
                        nc.vector.tensor_scalar_add(xb[:], ps[0:64, :], btile[0:64, :])
                        _elu(nc, op, mg[:, q * RM:(q + 1) * RM], xb[:], [64, RM], "e1")
                        if s12 is not None:
                            nc.scalar.copy(sv[:, q * RM:(q + 1) * RM], ps[64:66, :])
                    nc.vector.tensor_add(mg[:], mg[:], et[:])
                    nc.scalar.mul(mg[:], mg[:], 0.5)
                    nc.sync.dma_start(oten[:, t * RL:(t + 1) * RL], mg[:])
                    if s12 is not None:
                        nc.sync.dma_start(s12[:, t * RL:(t + 1) * RL], sv[:])

            gemm(xu, eu, wu_sb, bu_sb, ou, 64, UPC, 1500, 500)
            gemm(xi, ei, wi_sb, bi_sb, oi, 66, IPC, 750, 375, s12=os12)
    return nc


# ================================================================ K2: attention
# Item table rows: 128 f16 (256B): [emb 0:64 | s1 @64 | pad]. Payload 66.
PAY2 = 66

def _build_k2():
    nc = bass.Bass("TRN2", target_bir_lowering=False)
    tbl = nc.dram_tensor("tbl", [NUM_ITEMS, TSTride], F16, kind="ExternalInput")
    adji = nc.dram_tensor("adji", [128, (NBLK * K * 128) // 16], I16, kind="ExternalInput")
    s2r = nc.dram_tensor("s2r", [128, NBLK * K], F32, kind="ExternalInput")
    itm = nc.dram_tensor("itm", [128, NBLK * 64], F32, kind="ExternalInput")
    oit = nc.dram_tensor("oit", [128, NBLK * 64], F16, kind="ExternalOutput")

    with TileContext(nc) as tc:
        with tc.tile_pool(name="g", bufs=2) as gp, \
             tc.tile_pool(name="t", bufs=2) as tp, \
             tc.tile_pool(name="s", bufs=1) as sp, \
             tc.tile_pool(name="m", bufs=2) as mp:
            nc.gpsimd.load_library(library_config.mlp)
            adj_sb = sp.tile([128, (NBLK * K * 128) // 16], I16, tag="adj")
            nc.sync.dma_start(adj_sb[:], adji[:])
            s2_sb = sp.tile([128, NBLK * K], F32, tag="s2")
            nc.sync.dma_start(s2_sb[:], s2r[:])
            itm_sb = sp.tile([128, NBLK * 64], F32, tag="itm")
            nc.sync.dma_start(itm_sb[:], itm[:])
            niregs = {}
            for u in range(NSUB):
                g = gp.tile([128, GC, PAY2], F16, tag="g")
                # GC*128 = 12288 idx in chunks of 896 (7 cols)
                base16 = u * (GC * 128) // 16
                col = 0
                left = GC * 128
                while left > 0:
                    n = min(NI_MAX, left)
                    _dma_gather(nc.gpsimd, g[:, col:col + n // 128, :], tbl[:],
                                adj_sb[:, base16:base16 + n // 16], n, PAY2, TSTride,
                                regs=niregs)
                    base16 += n // 16
                    col += n // 128
                    left -= n
                s1f = mp.tile([128, GC], F32, tag="s1f")
                nc.scalar.copy(s1f[:], g[:, :, 64])
                lg = mp.tile([128, GC], F32, tag="lg")
                nc.vector.tensor_add(lg[:], s1f[:], s2_sb[:, u * GC:(u + 1) * GC])
                lr = mp.tile([128, GC], F32, tag="lr")
                nc.scalar.mul(lr[:], lg[:], 0.2)
                nc.vector.tensor_max(lg[:], lg[:], lr[:])
                ex = mp.tile([128, SUBB, K], F32, tag="ex")
                nc.scalar.activation(ex[:].rearrange("p a b -> p (a b)"), lg[:],
                                     AF.Exp, scale=1.0)
                sm = mp.tile([128, SUBB], F32, tag="sm")
                nc.vector.reduce_sum(sm[:], ex[:], axis=mybir.AxisListType.X)
                nc.vector.reciprocal(sm[:], sm[:])
                att = mp.tile([128, SUBB, K], F16, tag="att")
                for bb in range(SUBB):
                    nc.vector.tensor_scalar_mul(att[:, bb, :], ex[:, bb, :], sm[:, bb:bb + 1])
                tmp = tp.tile([128, SUBB, K, 64], F16, tag="tmp")
                av = att[:]
                att_b = bass.AP(av.tensor, av.offset, list(av.ap) + [[0, 64]])
                nc.vector.tensor_mul(
                    tmp[:], g[:].rearrange("p (b k) d -> p b k d", b=SUBB)[:, :, :, 0:64],
                    att_b)
                hp = mp.tile([128, SUBB, 64], F32, tag="hp")
                nc.vector.reduce_sum(hp[:], tmp[:].rearrange("p b k d -> p b d k"),
                                     axis=mybir.AxisListType.X)
                he = mp.tile([128, SUBB * 64], F32, tag="he")
                _elu(nc, mp, he[:], hp[:].rearrange("p b d -> p (b d)"),
                     [128, SUBB * 64], "e2")
                fo = mp.tile([128, SUBB * 64], F32, tag="fo")
                nc.vector.tensor_add(fo[:], he[:],
                                     itm_sb[:, u * SUBB * 64:(u + 1) * SUBB * 64])
                fo16 = mp.tile([128, SUBB * 64], F16, tag="fo16")
                nc.scalar.mul(fo16[:], fo[:], 0.5)
                nc.sync.dma_start(oit[:, u * SUBB * 64:(u + 1) * SUBB * 64], fo16[:])
    return nc


# ================================================================ K3: spmm
def _build_k3(meta):
    """SpMM layer kernel from packing metadata.

    meta: nwin, cap[w][r] (regular slots), ex[w][r] (extra cols),
          off[w][r] = list of per-column psum offsets.
    Stream layout per (w, r): [extra cols | regular cols]; per window the
    first extra of r0 initializes psum (start=True); last regular matmul
    of the last nonempty range carries stop=True.
    """
    nwin = meta["nwin"]
    cap = meta["cap"]; ex = meta["ex"]; off = meta["off"]
    totslots = int(sum(cap[w][r] + 128 * ex[w][r]
                       for w in range(nwin) for r in range(NRANGE)))
    totregcol = int(sum(cap[w][r] // 128 for w in range(nwin) for r in range(NRANGE)))
    totexcol = int(sum(ex[w][r] for w in range(nwin) for r in range(NRANGE)))
    maxwcol = max(sum(cap[w][r] // 128 + ex[w][r] for r in range(NRANGE))
                  for w in range(nwin))
    maxwreg = max(sum(cap[w][r] // 128 for r in range(NRANGE)) for w in range(nwin))
    maxwex = max(sum(ex[w][r] for r in range(NRANGE)) for w in range(nwin))

    nc = bass.Bass("TRN2", target_bir_lowering=False)
    tbls = [nc.dram_tensor(f"tbl{r}", [RROWS[r], TSTride], F16, kind="ExternalInput")
            for r in range(NRANGE)]
    idx = nc.dram_tensor("idx", [128, totslots // 16], I16, kind="ExternalInput")
    wreg = nc.dram_tensor("wreg", [128, totregcol * SWC], F16, kind="ExternalInput")
    wext = nc.dram_tensor("wext", [128, max(totexcol, 1) * WIN], F16, kind="ExternalInput")
    out = nc.dram_tensor("out", [64, nwin * WIN], F16, kind="ExternalOutput")

    with TileContext(nc) as tc:
        with tc.tile_pool(name="s", bufs=1) as sp, \
             tc.tile_pool(name="g", bufs=3) as gp, \
             tc.tile_pool(name="w", bufs=3) as wp, \
             tc.tile_pool(name="o", bufs=3) as op, \
             tc.tile_pool(name="ps", bufs=4, space="PSUM") as pp:
            nc.gpsimd.load_library(library_config.mlp)
            idx_sb = sp.tile([128, totslots // 16], I16, tag="idx")
            nc.sync.dma_start(idx_sb[:], idx[:])
            niregs = {}
            i16 = 0          # cursor into idx (units of 16 slots)
            rcol = 0         # cursor into wreg (regular col index)
            ecol = 0         # cursor into wext (extra col index)
            for w in range(nwin):
                wcols = sum(cap[w][r] // 128 + ex[w][r] for r in range(NRANGE))
                wregc = sum(cap[w][r] // 128 for r in range(NRANGE))
                wexc = sum(ex[w][r] for r in range(NRANGE))
                gt = gp.tile([128, maxwcol, SWC], F16, tag="g")
                wr = wp.tile([128, maxwreg * SWC], F16, tag="wr")
                nc.sync.dma_start(wr[:, 0:wregc * SWC],
                                  wreg[:, rcol * SWC:(rcol + wregc) * SWC])
                if maxwex:
                    we = wp.tile([128, max(maxwex, 1) * WIN], F16, tag="we")
                    if wexc:
                        nc.sync.dma_start(we[:, 0:wexc * WIN],
                                          wext[:, ecol * WIN:(ecol + wexc) * WIN])
                # gathers for the whole window (extras first per range)
                col = 0
                colmap = []   # per range: (excolbase, regcolbase)
                for r in range(NRANGE):
                    nsl = cap[w][r] + 128 * ex[w][r]
                    colmap.append((col, col + ex[w][r]))
                    left = nsl
                    while left > 0:
                        n = min(NI_MAX, left)
                        _dma_gather(nc.gpsimd, gt[:, col:col + n // 128, :], tbls[r][:],
                                    idx_sb[:, i16:i16 + n // 16], n, SWC, TSTride,
                                    regs=niregs)
                        i16 += n // 16
                        col += n // 128
                        left -= n
                # matmuls
                ps = pp.tile([64, WIN], F32, tag="ps")
                first = True
                wrc = 0
                wec = 0
                last_r = max(r for r in range(NRANGE) if cap[w][r] > 0)
                for r in range(NRANGE):
                    exbase, regbase = colmap[r]
                    for e in range(ex[w][r]):
                        nc.tensor.matmul(ps[:], gt[:, exbase + e, :],
                                         we[:, wec * WIN:(wec + 1) * WIN],
                                         start=first, stop=False)
                        first = False
                        wec += 1
                    ncols = cap[w][r] // 128
                    for j in range(ncols):
                        o = off[w][r][j]
                        stop = (r == last_r and j == ncols - 1)
                        nc.tensor.matmul(ps[:, o:o + SWC], gt[:, regbase + j, :],
                                         wr[:, wrc * SWC:(wrc + 1) * SWC],
                                         start=first, stop=stop)
                        first = False
                        wrc += 1
                rcol += wregc
                ecol += wexc
                ot = op.tile([64, WIN], F16, tag="ot")
                nc.scalar.copy(ot[:], ps[:])
                nc.sync.dma_start(out[:, w * WIN:(w + 1) * WIN], ot[:])
    return nc


# ================================================================ host packing
def _pack_edges(core, pos, src, val, nwin):
    """Pack edges (dest position pos within core, source node src) into the
    per-(window, range) gather/weight layout. Returns per-core arrays + meta."""
    w = pos // WIN
    drel = (pos - w * WIN).astype(np.int64)
    rg = np.searchsorted(RS, src, side="right") - 1
    src_rel = (src - RS[rg]).astype(np.int64)
    order = np.lexsort((drel, rg, w, core))
    core, w, drel, rg, src_rel, val = (a[order] for a in (core, w, drel, rg, src_rel, val))

    key = (core * nwin + w) * NRANGE + rg
    cnt = np.bincount(key, minlength=NCORE * nwin * NRANGE).reshape(NCORE, nwin, NRANGE)
    cap = ((cnt.max(axis=0) + 127) // 128) * 128          # [nwin, NRANGE]
    cap = np.maximum(cap, 128)                            # every cell >= 1 col

    n = len(key)
    ar = np.arange(n)
    first = np.empty(n, bool); first[0] = True; first[1:] = key[1:] != key[:-1]
    slot = ar - np.maximum.accumulate(np.where(first, ar, 0))

    ncol = cap >> 7
    regcol_base = np.concatenate([[0], np.cumsum(ncol.reshape(-1))])[:-1]\
        .reshape(nwin, NRANGE)
    totregcol = int(ncol.sum())

    # data-driven column offsets: cover the across-core [min, max] dest range
    colj = slot >> 7
    gcol = regcol_base[w, rg] + colj
    lo = np.full(totregcol, WIN, np.int64)
    hi = np.full(totregcol, -1, np.int64)
    np.minimum.at(lo, gcol, drel)
    np.maximum.at(hi, gcol, drel)
    lo = np.minimum(lo, hi)                               # empty cols -> [hi,hi]
    offcol = np.clip((lo + hi + 1 - SWC) // 2, 0, WIN - SWC)
    off_e = offcol[gcol]
    spill = (drel < off_e) | (drel >= off_e + SWC)

    # extra column counts (same for all cores)
    skey = key[spill]
    scnt = np.bincount(skey, minlength=NCORE * nwin * NRANGE).reshape(NCORE, nwin, NRANGE)
    exc = (scnt.max(axis=0) + 127) // 128                 # [nwin, NRANGE] cols
    exc[:, 0] = np.maximum(exc[:, 0], 1)                  # psum initializer

    # per-(w,r) stream slot count and bases (same all cores)
    cell_slots = cap + 128 * exc                          # [nwin, NRANGE]
    cell_base = np.concatenate([[0], np.cumsum(cell_slots.reshape(-1))])[:-1]\
        .reshape(nwin, NRANGE)                            # base within core stream
    tot = int(cell_slots.sum())
    excol_base = np.concatenate([[0], np.cumsum(exc.reshape(-1))])[:-1]\
        .reshape(nwin, NRANGE)
    totexcol = int(exc.sum())

    idx_flat = np.zeros((NCORE, tot), np.int16)
    wreg = np.zeros((NCORE, 128, totregcol * SWC), np.float16)
    wext = np.zeros((NCORE, 128, max(totexcol, 1) * WIN), np.float16)

    # regular slots: stream position = cell_base + 128*exc (extras first) + slot
    spos = cell_base[w, rg] + 128 * exc[w, rg] + slot
    idx_flat[core, spos] = src_rel.astype(np.int16)
    reg = ~spill
    fw = (core[reg] * 128 + (slot[reg] & 127)) * (totregcol * SWC) \
        + (regcol_base[w[reg], rg[reg]] + colj[reg]) * SWC + (drel[reg] - off_e[reg])
    wreg.reshape(-1)[fw] = val[reg]

    # spilled edges -> extra slots (their regular slot stays as weight-0 pad)
    if spill.any():
        sc, sw_, srg, ssrc, sdrel, sval = (a[spill] for a in (core, w, rg, src_rel, drel, val))
        ns = len(sc)
        ars = np.arange(ns)
        sfirst = np.empty(ns, bool); sfirst[0] = True; sfirst[1:] = skey[1:] != skey[:-1]
        eslot = ars - np.maximum.accumulate(np.where(sfirst, ars, 0))
        espos = cell_base[sw_, srg] + eslot
        idx_flat[sc, espos] = ssrc.astype(np.int16)
        fx = (sc * 128 + (eslot & 127)) * (max(totexcol, 1) * WIN) \
            + (excol_base[sw_, srg] + (eslot >> 7)) * WIN + sdrel
        wext.reshape(-1)[fx] = sval

    # idx stream -> [128, tot/16] wrapped+replicated layout
    idx_arr = np.tile(idx_flat.reshape(NCORE, tot // 16, 16).transpose(0, 2, 1),
                      (1, 8, 1))

    off_tab = [[list(int(offcol[regcol_base[w_, r_] + j_])
                     for j_ in range(ncol[w_][r_]))
                for r_ in range(NRANGE)] for w_ in range(nwin)]
    meta = {"nwin": nwin,
            "cap": [[int(cap[w_][r_]) for r_ in range(NRANGE)] for w_ in range(nwin)],
            "ex": [[int(exc[w_][r_]) for r_ in range(NRANGE)] for w_ in range(nwin)],
            "off": off_tab}
    return idx_arr, wreg, wext, meta


def _edges_for_lists(rows, cols, vals, dlist):
    """Expand: for each core and each listed dest (position p in dlist[c]),
    all incoming edges. Returns (core, pos, src, val). dlist entries >= N are
    edgeless sentinels."""
    order0 = np.argsort(rows, kind="stable")
    rs, cs, vs = rows[order0], cols[order0], vals[order0]
    row_start = np.searchsorted(rs, np.arange(N + 1))
    rr = dlist.reshape(-1).astype(np.int64)
    rrc = np.minimum(rr, N)
    cnt = row_start[np.minimum(rrc + 1, N)] - row_start[rrc]
    cnt[rr >= N] = 0
    ent = np.repeat(np.arange(len(rr)), cnt)
    ofs = np.arange(len(ent)) - np.repeat(np.cumsum(cnt) - cnt, cnt)
    srcidx = row_start[rrc][ent] + ofs
    ndst = dlist.shape[1]
    e_core = ent // ndst
    e_pos = ent - e_core * ndst
    return e_core, e_pos, cs[srcidx], vs[srcidx]


def _prep(inputs):
    p = {}
    users = np.asarray(inputs["users"]);   items = np.asarray(inputs["items"])
    adj = np.asarray(inputs["adj_matrix"])
    rows = np.asarray(inputs["graph_rows"]).astype(np.int64)
    cols = np.asarray(inputs["graph_cols"]).astype(np.int64)
    vals = np.asarray(inputs["graph_vals"]).astype(np.float32)
    W_att = np.asarray(inputs["W_att"]); a_att = np.asarray(inputs["a_att"])
    v1 = W_att @ a_att[:HID, 0]; v2 = W_att @ a_att[HID:, 0]

    p["xu"] = np.ascontiguousarray(np.asarray(inputs["user_semantic_emb"]).astype(np.float16).T)
    p["xi"] = np.ascontiguousarray(np.asarray(inputs["semantic_emb"]).astype(np.float16).T)
    p["wu"] = np.asarray(inputs["W_usem"]).astype(np.float16)
    p["wi"] = np.concatenate([np.asarray(inputs["W_sem"]), v1[:, None], v2[:, None]],
                             axis=1).astype(np.float16)
    p["bu"] = np.asarray(inputs["b_usem"]).reshape(64, 1)
    p["bi"] = np.concatenate([np.asarray(inputs["b_sem"]), np.zeros(2, np.float32)]
                             ).reshape(66, 1).astype(np.float32)
    p["eu"] = np.ascontiguousarray(np.asarray(inputs["emb_user"]).T)
    p["ei"] = np.ascontiguousarray(np.asarray(inputs["emb_item"]).T)

    # K2 neighbor indices: gather slot i (of GC*128 per sub-batch) = col*128+p
    # -> adj[block b = (u*SUBB + col//K), item p, k = col%K]
    adj_pad = np.zeros((NCORE, IPAD, K), np.int64)
    for c in range(NCORE):
        adj_pad[c, :IPC] = adj[c * IPC:(c + 1) * IPC]
    slot_idx = np.transpose(adj_pad.reshape(NCORE, NBLK, 128, K), (0, 2, 1, 3))
    # flat stream per core: for sub-batch u, col cc, slot p: value adj[..]
    flat = np.transpose(adj_pad.reshape(NCORE, NBLK, 128, K), (0, 1, 3, 2))\
        .reshape(NCORE, NBLK * K * 128)            # (b, k) cols x 128 slots
    p["adji"] = np.tile(flat.reshape(NCORE, -1, 16).transpose(0, 2, 1),
                        (1, 8, 1)).astype(np.int16)

    # layer 1: all NPAD dests
    e_core = rows // DPC
    e_pos = rows - e_core * DPC
    p["l1"] = _pack_edges(e_core, e_pos, cols, vals, NWIN1)

    # batch dest list (layer 3 + K4)
    dlist = np.stack([np.concatenate([
        users[c * BPC:(c + 1) * BPC].astype(np.int64),
        items[c * BPC:(c + 1) * BPC].astype(np.int64) + NUM_USERS])
        for c in range(NCORE)])
    NWIN3 = (2 * BPC) // WIN                       # 2
    ec, ep, es, ev = _edges_for_lists(rows, cols, vals, dlist)
    p["l3"] = _pack_edges(ec, ep, es, ev, NWIN3)
    p["nwin3"] = NWIN3

    # layer 2 rows needed: sources of layer-3 edges + K4's rows
    need = np.zeros(N, bool)
    need[es] = True
    need[dlist.reshape(-1)] = True
    R2 = np.nonzero(need)[0]
    Lc = -(-len(R2) // NCORE)
    L2 = -(-Lc // WIN) * WIN
    NWIN2 = L2 // WIN
    lists2 = np.full((NCORE, L2), N, np.int64)
    for c in range(NCORE):
        seg = R2[c * Lc:(c + 1) * Lc]
        lists2[c, :len(seg)] = seg
    p["lists2"] = lists2
    ec, ep, es2, ev2 = _edges_for_lists(rows, cols, vals, lists2)
    p["l2"] = _pack_edges(ec, ep, es2, ev2, NWIN2)
    p["nwin2"] = NWIN2

    p["k4"] = []
    for c in range(NCORE):
        u = users[c * BPC:(c + 1) * BPC].astype(np.int64)
        it = items[c * BPC:(c + 1) * BPC].astype(np.int64) + NUM_USERS
        rws = np.concatenate([u, it])
        p["k4"].append(np.ascontiguousarray(rws.reshape(8, 128).T.astype(np.int32)))
    return p


# ================================================================ K4: final
def _build_k4():
    nc = bass.Bass("TRN2", target_bir_lowering=False)
    tb012 = nc.dram_tensor("tb012", [NPAD, 192], F16, kind="ExternalInput")
    tb3d = nc.dram_tensor("tb3d", [128, 8 * 64], F16, kind="ExternalInput")
    fidx = nc.dram_tensor("fidx", [128, 8], I32, kind="ExternalInput")
    out = nc.dram_tensor("out", [128, 4], F32, kind="ExternalOutput")

    with TileContext(nc) as tc:
        with tc.tile_pool(name="g", bufs=2) as gp, \
             tc.tile_pool(name="m", bufs=1) as mp:
            it = mp.tile([128, 8], I32, tag="it")
            nc.sync.dma_start(it[:], fidx[:])
            t3 = mp.tile([128, 8, 64], F16, tag="t3")
            nc.sync.dma_start(t3[:].rearrange("p a b -> p (a b)"), tb3d[:])
            acc = mp.tile([128, 8, 64], F32, tag="acc")
            nc.scalar.copy(acc[:], t3[:])
            g = gp.tile([128, 8, 192], F16, tag="g")
            for t in range(8):
                nc.gpsimd.indirect_dma_start(
                    out=g[:, t, :], out_offset=None, in_=tb012[:],
                    in_offset=bass.IndirectOffsetOnAxis(ap=it[:, t:t + 1], axis=0))
            for sl in range(3):
                gf = gp.tile([128, 8, 64], F32, tag="gf")
                nc.scalar.copy(gf[:], g[:].rearrange("p a (s b) -> p a s b", s=3)[:, :, sl, :])
                nc.vector.tensor_add(acc[:], acc[:], gf[:])
            nc.scalar.mul(acc[:], acc[:], 0.25)
            prod = mp.tile([128, 4, 64], F32, tag="prod")
            nc.vector.tensor_mul(prod[:], acc[:, 0:4, :], acc[:, 4:8, :])
            res = mp.tile([128, 4], F32, tag="res")
            nc.vector.reduce_sum(res[:], prod[:], axis=mybir.AxisListType.X)
            nc.sync.dma_start(out[:], res[:])
    return nc


_META = {}

def _run(name, builder, in_maps, meta_key=None):
    if name not in _BUILT or _META.get(name) != meta_key:
        nc = builder()
        mybir.codegen_inst_isa_subclasses(nc)
        _BUILT[name] = nc
        _META[name] = meta_key
    return bass_utils.run_bass_kernel_spmd(
        _BUILT[name], in_maps, core_ids=list(range(NCORE))).results


def _tables_from_nodes(node_tbl):
    """node_tbl [NPAD, 64] f16 -> 3 range tables [rows, 128] f16."""
    full = np.zeros((NPAD, TSTride), np.float16)
    full[:, 0:64] = node_tbl
    return [np.ascontiguousarray(full[RS[r]:RS[r + 1]]) for r in range(NRANGE)]


def kernel(**inputs):
    p = _prep(inputs)

    # ---------------- K1
    maps = [{
        "xu": p["xu"][:, c * UPC:(c + 1) * UPC],
        "xi": p["xi"][:, c * IPC:(c + 1) * IPC],
        "wu": p["wu"], "wi": p["wi"], "bu": p["bu"], "bi": p["bi"],
        "eu": p["eu"][:, c * UPC:(c + 1) * UPC],
        "ei": p["ei"][:, c * IPC:(c + 1) * IPC],
    } for c in range(NCORE)]
    r1 = _run("k1", _build_k1, maps)
    users_m = np.concatenate([r1[c]["ou"].T for c in range(NCORE)], 0)
    items_m = np.concatenate([r1[c]["oi"].T for c in range(NCORE)], 0)
    s1 = np.concatenate([r1[c]["os12"][0] for c in range(NCORE)])
    s2 = np.concatenate([r1[c]["os12"][1] for c in range(NCORE)])

    # ---------------- K2
    tblA = np.zeros((NUM_ITEMS, TSTride), np.float16)
    tblA[:, 0:64] = items_m
    tblA[:, 64] = s1
    maps = []
    for c in range(NCORE):
        s2c = np.zeros(IPAD, np.float32)
        s2c[:IPC] = s2[c * IPC:(c + 1) * IPC]
        s2r = np.transpose(np.broadcast_to(s2c.reshape(NBLK, 1, 128), (NBLK, K, 128)),
                           (2, 0, 1)).reshape(128, NBLK * K)
        imc = np.zeros((IPAD, 64), np.float32)
        imc[:IPC] = items_m[c * IPC:(c + 1) * IPC]
        itm = np.transpose(imc.reshape(NBLK, 128, 64), (1, 0, 2)).reshape(128, NBLK * 64)
        maps.append({"tbl": tblA, "adji": p["adji"][c],
                     "s2r": np.ascontiguousarray(s2r),
                     "itm": np.ascontiguousarray(itm)})
    r2 = _run("k2", _build_k2, maps)
    items_f = np.zeros((NUM_ITEMS, 64), np.float16)
    for c in range(NCORE):
        o = r2[c]["oit"].reshape(128, NBLK, 64).transpose(1, 0, 2).reshape(IPAD, 64)
        items_f[c * IPC:(c + 1) * IPC] = o[:IPC]

    # ---------------- K3 layers
    def run_spmm(name, pack, tbl_nodes):
        idx_arr, wreg, wext, meta = pack
        tbls = _tables_from_nodes(tbl_nodes)
        maps = [dict({f"tbl{r}": tbls[r] for r in range(NRANGE)},
                     idx=idx_arr[c], wreg=wreg[c], wext=wext[c])
                for c in range(NCORE)]
        mk = (meta["nwin"], tuple(map(tuple, meta["cap"])), tuple(map(tuple, meta["ex"])))
        r = _run(name, lambda: _build_k3(meta), maps, meta_key=mk)
        return r

    e0 = np.zeros((NPAD, 64), np.float16)
    e0[:NUM_USERS] = users_m.astype(np.float16)
    e0[NUM_USERS:N] = items_f

    r = run_spmm("k3", p["l1"], e0)
    e1 = np.zeros((NPAD, 64), np.float16)
    for c in range(NCORE):
        e1[c * DPC:(c + 1) * DPC] = r[c]["out"].T

    r = run_spmm("k3b", p["l2"], e1)
    e2 = np.zeros((NPAD, 64), np.float16)
    lists2 = p["lists2"]
    for c in range(NCORE):
        outc = r[c]["out"].T
        sel = lists2[c] < N
        e2[lists2[c][sel]] = outc[:len(lists2[c])][sel]

    r = run_spmm("k3c", p["l3"], e2)
    tb3d = []
    for c in range(NCORE):
        rowsc = r[c]["out"].T[:2 * BPC]                  # [1024, 64]
        tb3d.append(np.ascontiguousarray(
            rowsc.reshape(8, 128, 64).transpose(1, 0, 2).reshape(128, 8 * 64)))

    # ---------------- K4
    e012 = np.ascontiguousarray(np.concatenate([e0, e1, e2], axis=1))
    maps = [{"tb012": e012, "tb3d": tb3d[c], "fidx": p["k4"][c]}
            for c in range(NCORE)]
    r4 = _run("k4", _build_k4, maps)
    gamma = np.zeros(B, np.float32)
    for c in range(NCORE):
        gamma[c * BPC:(c + 1) * BPC] = r4[c]["out"].T.reshape(BPC)
    return gamma


# revision 14
# speedup vs baseline: 4.5023x; 1.0600x over previous
"""CoLaKG model kernel for 8 Trainium2 NeuronCores (self-contained).

Pipeline (6 bass SPMD launches; host does static prep + mechanical relayout):
  K1 gemm : semantic projections (users+items) + merge, + s1/s2 GAT scalars
  K2 attn : item-neighbor GAT attention (batched dma_gather + softmax + wsum)
  K3 spmm : LightGCN propagation layer 1 (all dests)
  K3b     : layer 2 at the ~87% of rows consumed downstream (list-addressed)
  K3c     : layer 3 at the batch's 2*512 rows per core
  K4 final: gather 4 embedding tables at (user,item) rows, mean, dot product

Perf design: all row gathers use the batched InstDMAGatherAnt (Q7 'mlp'
ucode library) at <=896 indices per instruction, amortizing the ~1us
SWDGE fixed cost ~7x vs per-column indirect DMA. Node tables are stored
at 256B stride with a 128B gathered payload (int16 indices -> 3 table
ranges). The SpMM packs edges per (512-dest window, range) sorted by
dest; each 128-edge column feeds one matmul into a fixed 64-wide psum
span (drift-tolerant placement; stragglers go to window-wide extra
columns; psum initialized by the first wide matmul's start flag).
"""
import copy
import numpy as np

import jax
jax.config.update("jax_compilation_cache_dir", "/tmp/.jax_bass_cache")
jax.config.update("jax_persistent_cache_min_entry_size_bytes", -1)
jax.config.update("jax_persistent_cache_min_compile_time_secs", 0.0)

import concourse.bass as bass
import concourse.mybir as mybir
from concourse.tile import TileContext
from concourse import bass_utils, library_config
import concourse.tile as tile_mod
from concourse.vector_clock import ScopedClock

F32 = mybir.dt.float32
F16 = mybir.dt.float16
I16 = mybir.dt.int16
I32 = mybir.dt.int32
AF = mybir.ActivationFunctionType

# ---------------------------------------------------------------- tile patch
MAX_WAITS = 1

def _split_sync_waits(nc, max_waits=MAX_WAITS):
    template = None
    counter = [0]
    for fn in nc.m.functions:
        for bb in fn.blocks:
            for inst in bb.instructions:
                if type(inst).__name__ == "InstNoOp":
                    template = copy.deepcopy(inst)
                    break
            if template is not None:
                break
        if template is not None:
            break
    for fn in nc.m.functions:
        for bb in fn.blocks:
            il = bb.instructions
            i = 0
            while i < len(il):
                inst = il[i]
                if template is None and type(inst).__name__ == "InstNoOp":
                    template = copy.deepcopy(inst)
                si = inst.sync_info
                if si is not None and si.on_wait is not None and len(si.on_wait) > max_waits:
                    assert template is not None, "no InstNoOp to clone"
                    waits = list(si.on_wait)
                    keep, rest = waits[:max_waits], waits[max_waits:]
                    si.on_wait.clear()
                    for w in keep:
                        si.on_wait.append(w)
                    carriers = []
                    while rest:
                        c = copy.deepcopy(template)
                        counter[0] += 1
                        c.name = f"I-waitsplit-{counter[0]}"
                        c.engine = inst.engine
                        c.sync_info = mybir.SyncInfo(on_wait=list(rest[:max_waits]), on_update=[])
                        carriers.append(c)
                        rest = rest[max_waits:]
                    for k, cinst in enumerate(carriers):
                        try:
                            nc.register_instruction(cinst, overwrite=True)
                        except Exception:
                            pass
                        il.insert(i + k, cinst)
                    i += len(carriers)
                i += 1

def _patched_drain_and_barrier(self, tick_clock, wait_clock):
    nc = self.nc
    nop0 = nc.sync.nop(nofuse=True, hint="predrain_waits")
    wait_clock.add_sem_waits(nop0.ins, ScopedClock({None: tick_clock.global_clock}))
    nc.sync.drain()
    nc.all_engine_barrier()
    assert self.sems is not None
    popped = nc._tile_sem_poison_stack.pop()
    assert popped is self._sem_poison
    nc.clear_and_free_semaphores(list(self.sems.allocated().values()))
    nc.all_engine_barrier()
    _split_sync_waits(nc)

tile_mod.TileContext._drain_and_barrier = _patched_drain_and_barrier

# ---------------------------------------------------------------- constants
NUM_USERS, NUM_ITEMS, D, SEM, HID, K = 60000, 30000, 64, 1024, 32, 32
N = NUM_USERS + NUM_ITEMS             # 90000
NPAD = 90112                          # 704*128
NCORE = 8
DPC = NPAD // NCORE                   # 11264 dest rows per core
WIN = 512                             # dests per window (one psum bank f32)
NWIN1 = DPC // WIN                    # 22 windows, layer-1
SWC = 64                              # psum span per regular matmul column
NRANGE = 3
RS = np.array([0, 32768, 65536, NPAD], dtype=np.int64)   # table range bounds
RROWS = [32768, 32768, 24576]
NI_MAX = 896                          # idx per gather instruction (ring cap)
TSTride = 128                         # table row stride (f16 elems) = 256B

UPC = NUM_USERS // NCORE              # 7500
IPC = NUM_ITEMS // NCORE              # 3750
IPAD = 3840
NBLK = IPAD // 128                    # 30
SUBB = 6                              # item blocks per attention sub-batch
NSUB = NBLK // SUBB                   # 10
GC = SUBB * K                         # 96 gather cols per K2 sub-batch
B = 4096
BPC = B // NCORE                      # 512

_BUILT = {}


def _dma_gather(g, out_ap, in_ap, idxs_ap, num_idxs, elem_size, elem_step,
                regs=None):
    """dma_gather with payload < stride (bass's public API asserts
    elem_size%256B which is only a stride requirement). regs: dict caching
    one GPSIMD register per distinct num_idxs value."""
    _in_ap = g.lower_ap_dma(in_ap, for_custom_bir_dma=True)
    _idxs_ap = g.lower_ap(idxs_ap)
    _out_ap = g.lower_ap(out_ap)
    if regs is None:
        reg = g.to_reg(num_idxs)
    else:
        if num_idxs not in regs:
            regs[num_idxs] = g.to_reg(num_idxs)
        reg = regs[num_idxs]
    stride_bytes = elem_step * mybir.dt.size(in_ap.dtype)
    assert stride_bytes % 256 == 0
    return g.add_instruction(
        mybir.InstDMAGatherAnt(
            name=g.bass.get_next_instruction_name(),
            ins=[*_in_ap, _idxs_ap, g.lower_val_access(reg)],
            outs=[_out_ap],
            transpose=False, num_idxs=num_idxs, elem_size=elem_size,
            stride_bytes_256=stride_bytes // 256, gen_mode=0,
            single_packet=True, queue_num=0,
            sbuf_tokens_per_rank=0, sbuf_free_dim_per_rank=0,
            sbuf_free_dim_pad_per_rank=0, sbuf_byte_offset=0,
        ))


def _elu(nc, pool, out_ap, in_ap, shape, tag):
    """out = elu(in) = max(x,0) + exp(min(x,0)) - 1   (no Elu in ACT table)."""
    mn = pool.tile(shape, F32, tag=tag + "_mn")
    nc.vector.tensor_scalar_min(mn[:], in_ap, 0.0)
    ex = pool.tile(shape, F32, tag=tag + "_ex")
    nc.scalar.activation(ex[:], mn[:], AF.Exp, scale=1.0)
    mx = pool.tile(shape, F32, tag=tag + "_mx")
    nc.vector.tensor_scalar_max(mx[:], in_ap, 0.0)
    nc.vector.tensor_add(out_ap, mx[:], ex[:])
    nc.vector.tensor_scalar_add(out_ap, out_ap, -1.0)


# ================================================================ K1: GEMM
def _build_k1():
    nc = bass.Bass("TRN2", target_bir_lowering=False)
    xu = nc.dram_tensor("xu", [SEM, UPC], F16, kind="ExternalInput")
    xi = nc.dram_tensor("xi", [SEM, IPC], F16, kind="ExternalInput")
    wu = nc.dram_tensor("wu", [SEM, 64], F16, kind="ExternalInput")
    wi = nc.dram_tensor("wi", [SEM, 66], F16, kind="ExternalInput")
    bu = nc.dram_tensor("bu", [64, 1], F32, kind="ExternalInput")
    bi = nc.dram_tensor("bi", [66, 1], F32, kind="ExternalInput")
    eu = nc.dram_tensor("eu", [64, UPC], F32, kind="ExternalInput")
    ei = nc.dram_tensor("ei", [64, IPC], F32, kind="ExternalInput")
    ou = nc.dram_tensor("ou", [64, UPC], F32, kind="ExternalOutput")
    oi = nc.dram_tensor("oi", [64, IPC], F32, kind="ExternalOutput")
    os12 = nc.dram_tensor("os12", [2, IPC], F32, kind="ExternalOutput")

    with TileContext(nc) as tc:
        with tc.tile_pool(name="w", bufs=1) as wp, \
             tc.tile_pool(name="x", bufs=3) as xp, \
             tc.tile_pool(name="o", bufs=2) as op, \
             tc.tile_pool(name="ps", bufs=2, space="PSUM") as pp:
            wu_sb = wp.tile([128, SEM // 128, 64], F16, tag="wu")
            nc.sync.dma_start(wu_sb[:], wu[:].rearrange("(a p) m -> p a m", p=128))
            wi_sb = wp.tile([128, SEM // 128, 66], F16, tag="wi")
            nc.sync.dma_start(wi_sb[:], wi[:].rearrange("(a p) m -> p a m", p=128))
            bu_sb = wp.tile([64, 1], F32, tag="bu")
            nc.sync.dma_start(bu_sb[:], bu[:])
            bi_sb = wp.tile([66, 1], F32, tag="bi")
            nc.sync.dma_start(bi_sb[:], bi[:])

            def gemm(xten, eten, wtile, btile, oten, m, rows, RL, RM, s12=None):
                # RL: DMA load tile; RM: matmul tile (psum bank limit 512 f32)
                for t in range(rows // RL):
                    xt = xp.tile([128, SEM // 128, RL], F16, tag="xt")
                    nc.sync.dma_start(
                        xt[:], xten[:, t * RL:(t + 1) * RL].rearrange("(a p) r -> p a r", p=128))
                    et = op.tile([64, RL], F32, tag="et")
                    nc.sync.dma_start(et[:], eten[:, t * RL:(t + 1) * RL])
                    mg = op.tile([64, RL], F32, tag="mg")
                    if s12 is not None:
                        sv = op.tile([2, RL], F32, tag="sv")
                    else:
                        sv = None
                    for q in range(RL // RM):
                        ps = pp.tile([m, RM], F32, tag="ps")
                        for kk in range(SEM // 128):
                            nc.tensor.matmul(ps[:], wtile[:, kk, :],
                                             xt[:, kk, q * RM:(q + 1) * RM],
                                             start=(kk == 0), stop=(kk == SEM // 128 - 1))
                        xb = op.tile([64, RM], F32, tag="xb")
                        nc.vector.tensor_scalar_add(xb[:], ps[0:64, :], btile[0:64, :])
                        _elu(nc, op, mg[:, q * RM:(q + 1) * RM], xb[:], [64, RM], "e1")
                        if s12 is not None:
                            nc.scalar.copy(sv[:, q * RM:(q + 1) * RM], ps[64:66, :])
                    nc.vector.tensor_add(mg[:], mg[:], et[:])
                    nc.scalar.mul(mg[:], mg[:], 0.5)
                    nc.sync.dma_start(oten[:, t * RL:(t + 1) * RL], mg[:])
                    if s12 is not None:
                        nc.sync.dma_start(s12[:, t * RL:(t + 1) * RL], sv[:])

            gemm(xu, eu, wu_sb, bu_sb, ou, 64, UPC, 1500, 500)
            gemm(xi, ei, wi_sb, bi_sb, oi, 66, IPC, 750, 375, s12=os12)
    return nc


# ================================================================ K2: attention
# Item table rows: 128 f16 (256B): [emb 0:64 | s1 @64 | pad]. Payload 66.
# Gather layout: column = quad of 4 items, partition p = (item%4)*32 + k.
# Weighted sum via PE: per quad, matmul(stationary=gathered [128,64],
# moving=masked unnormalized exp weights [128,4]) -> psum [64, 4 items];
# softmax normalization folded in by smearing 1/Z over partitions with a
# second matmul. Output is dim-major [64, items].
PAY2 = 66
QTOT = 1024                           # quads per core (4096 item slots)
NGRP = 8                              # psum groups of 128 quads (512 items)
QG = 128                              # quads per group

def _build_k2():
    nc = bass.Bass("TRN2", target_bir_lowering=False)
    tbl = nc.dram_tensor("tbl", [NUM_ITEMS, TSTride], F16, kind="ExternalInput")
    adjq = nc.dram_tensor("adjq", [128, (QTOT * 128) // 16], I16, kind="ExternalInput")
    s2q = nc.dram_tensor("s2q", [128, QTOT], F32, kind="ExternalInput")
    itmT = nc.dram_tensor("itmT", [64, NGRP * 512], F32, kind="ExternalInput")
    m16 = nc.dram_tensor("m16", [128, 4], F16, kind="ExternalInput")
    m32 = nc.dram_tensor("m32", [128, 4], F32, kind="ExternalInput")
    mt32 = nc.dram_tensor("mt32", [4, 128], F32, kind="ExternalInput")
    oit = nc.dram_tensor("oit", [64, NGRP * 512], F16, kind="ExternalOutput")

    with TileContext(nc) as tc:
        with tc.tile_pool(name="g", bufs=2) as gp, \
             tc.tile_pool(name="s", bufs=1) as sp, \
             tc.tile_pool(name="m", bufs=2) as mp, \
             tc.tile_pool(name="ps", bufs=2, space="PSUM") as pp:
            nc.gpsimd.load_library(library_config.mlp)
            adj_sb = sp.tile([128, (QTOT * 128) // 16], I16, tag="adj")
            nc.sync.dma_start(adj_sb[:], adjq[:])
            s2_sb = sp.tile([128, QTOT], F32, tag="s2")
            nc.sync.dma_start(s2_sb[:], s2q[:])
            m16_sb = sp.tile([128, 4], F16, tag="m16")
            nc.sync.dma_start(m16_sb[:], m16[:])
            m32_sb = sp.tile([128, 4], F32, tag="m32")
            nc.sync.dma_start(m32_sb[:], m32[:])
            mt32_sb = sp.tile([4, 128], F32, tag="mt32")
            nc.sync.dma_start(mt32_sb[:], mt32[:])
            niregs = {}
            for u in range(NGRP):
                g = gp.tile([128, QG, PAY2], F16, tag="g")
                base16 = u * (QG * 128) // 16
                col = 0
                left = QG * 128
                while left > 0:
                    n = min(NI_MAX, left)
                    _dma_gather(nc.gpsimd, g[:, col:col + n // 128, :], tbl[:],
                                adj_sb[:, base16:base16 + n // 16], n, PAY2, TSTride,
                                regs=niregs)
                    base16 += n // 16
                    col += n // 128
                    left -= n
                # attention logits + leaky relu + exp (unnormalized)
                lg = mp.tile([128, QG], F32, tag="lg")
                nc.vector.tensor_add(lg[:], g[:, :, 64], s2_sb[:, u * QG:(u + 1) * QG])
                lr = mp.tile([128, QG], F32, tag="lr")
                nc.scalar.mul(lr[:], lg[:], 0.2)
                nc.vector.tensor_max(lg[:], lg[:], lr[:])
                ex32 = mp.tile([128, QG], F32, tag="ex32")
                nc.scalar.activation(ex32[:], lg[:], AF.Exp, scale=1.0)
                # Z per (item-in-quad j, quad) then smear 1/Z over partitions
                psZ = pp.tile([4, 512], F32, tag="psZ")
                nc.tensor.matmul(psZ[:, 0:QG], m32_sb[:], ex32[:],
                                 start=True, stop=True)
                rz = mp.tile([4, QG], F32, tag="rz")
                nc.vector.reciprocal(rz[:], psZ[:, 0:QG])
                psR = pp.tile([128, 512], F32, tag="psR")
                nc.tensor.matmul(psR[:, 0:QG], mt32_sb[:], rz[:],
                                 start=True, stop=True)
                att = mp.tile([128, QG], F16, tag="att")
                nc.vector.tensor_mul(att[:], ex32[:], psR[:, 0:QG])
                # expand to masked moving blocks [128, QG, 4]
                attm = mp.tile([128, QG, 4], F16, tag="attm")
                av = att[:]
                att_b = bass.AP(av.tensor, av.offset, list(av.ap) + [[0, 4]])
                mv = m16_sb[:]
                m_b = bass.AP(mv.tensor, mv.offset,
                              [mv.ap[0], [0, QG], mv.ap[1]])
                nc.vector.tensor_mul(attm[:], att_b, m_b)
                # weighted sum: one matmul per quad into [64, 512] psum
                psH = pp.tile([64, 512], F32, tag="psH")
                for q in range(QG):
                    nc.tensor.matmul(psH[:, 4 * q:4 * q + 4], g[:, q, 0:64],
                                     attm[:, q, :],
                                     start=(q == 0), stop=(q == QG - 1))
                hT = mp.tile([64, 512], F32, tag="hT")
                nc.scalar.copy(hT[:], psH[:])
                he = mp.tile([64, 512], F32, tag="he")
                _elu(nc, mp, he[:], hT[:], [64, 512], "e2")
                it = mp.tile([64, 512], F32, tag="it")
                nc.sync.dma_start(it[:], itmT[:, u * 512:(u + 1) * 512])
                nc.vector.tensor_add(he[:], he[:], it[:])
                fo16 = mp.tile([64, 512], F16, tag="fo16")
                nc.scalar.mul(fo16[:], he[:], 0.5)
                nc.sync.dma_start(oit[:, u * 512:(u + 1) * 512], fo16[:])
    return nc


# ================================================================ K3: spmm
def _build_k3(meta):
    """SpMM layer kernel from packing metadata.

    meta: nwin, cap[w][r] (regular slots), ex[w][r] (extra cols),
          off[w][r] = list of per-column psum offsets.
    Stream layout per (w, r): [extra cols | regular cols]; per window the
    first extra of r0 initializes psum (start=True); last regular matmul
    of the last nonempty range carries stop=True.
    """
    nwin = meta["nwin"]
    cap = meta["cap"]; ex = meta["ex"]; off = meta["off"]
    totslots = int(sum(cap[w][r] + 128 * ex[w][r]
                       for w in range(nwin) for r in range(NRANGE)))
    totregcol = int(sum(cap[w][r] // 128 for w in range(nwin) for r in range(NRANGE)))
    totexcol = int(sum(ex[w][r] for w in range(nwin) for r in range(NRANGE)))
    maxwcol = max(sum(cap[w][r] // 128 + ex[w][r] for r in range(NRANGE))
                  for w in range(nwin))
    maxwreg = max(sum(cap[w][r] // 128 for r in range(NRANGE)) for w in range(nwin))
    maxwex = max(sum(ex[w][r] for r in range(NRANGE)) for w in range(nwin))

    nc = bass.Bass("TRN2", target_bir_lowering=False)
    tbls = [nc.dram_tensor(f"tbl{r}", [RROWS[r], TSTride], F16, kind="ExternalInput")
            for r in range(NRANGE)]
    idx = nc.dram_tensor("idx", [128, totslots // 16], I16, kind="ExternalInput")
    wreg = nc.dram_tensor("wreg", [128, totregcol * SWC], F16, kind="ExternalInput")
    wext = nc.dram_tensor("wext", [128, max(totexcol, 1) * WIN], F16, kind="ExternalInput")
    out = nc.dram_tensor("out", [64, nwin * WIN], F16, kind="ExternalOutput")

    with TileContext(nc) as tc:
        with tc.tile_pool(name="s", bufs=1) as sp, \
             tc.tile_pool(name="g", bufs=3) as gp, \
             tc.tile_pool(name="w", bufs=3) as wp, \
             tc.tile_pool(name="o", bufs=3) as op, \
             tc.tile_pool(name="ps", bufs=4, space="PSUM") as pp:
            nc.gpsimd.load_library(library_config.mlp)
            idx_sb = sp.tile([128, totslots // 16], I16, tag="idx")
            niregs = {}
            i16 = 0          # cursor into idx (units of 16 slots)
            rcol = 0         # cursor into wreg (regular col index)
            ecol = 0         # cursor into wext (extra col index)
            for w in range(nwin):
                wsl = sum(cap[w][r] + 128 * ex[w][r] for r in range(NRANGE)) // 16
                nc.sync.dma_start(idx_sb[:, i16:i16 + wsl], idx[:, i16:i16 + wsl])
                wcols = sum(cap[w][r] // 128 + ex[w][r] for r in range(NRANGE))
                wregc = sum(cap[w][r] // 128 for r in range(NRANGE))
                wexc = sum(ex[w][r] for r in range(NRANGE))
                gt = gp.tile([128, maxwcol, SWC], F16, tag="g")
                wr = wp.tile([128, maxwreg * SWC], F16, tag="wr")
                nc.sync.dma_start(wr[:, 0:wregc * SWC],
                                  wreg[:, rcol * SWC:(rcol + wregc) * SWC])
                if maxwex:
                    we = wp.tile([128, max(maxwex, 1) * WIN], F16, tag="we")
                    if wexc:
                        nc.sync.dma_start(we[:, 0:wexc * WIN],
                                          wext[:, ecol * WIN:(ecol + wexc) * WIN])
                # gathers for the whole window (extras first per range)
                col = 0
                colmap = []   # per range: (excolbase, regcolbase)
                for r in range(NRANGE):
                    nsl = cap[w][r] + 128 * ex[w][r]
                    colmap.append((col, col + ex[w][r]))
                    left = nsl
                    while left > 0:
                        n = min(NI_MAX, left)
                        _dma_gather(nc.gpsimd, gt[:, col:col + n // 128, :], tbls[r][:],
                                    idx_sb[:, i16:i16 + n // 16], n, SWC, TSTride,
                                    regs=niregs)
                        i16 += n // 16
                        col += n // 128
                        left -= n
                # matmuls
                ps = pp.tile([64, WIN], F32, tag="ps")
                first = True
                wrc = 0
                wec = 0
                last_r = max(r for r in range(NRANGE) if cap[w][r] > 0)
                for r in range(NRANGE):
                    exbase, regbase = colmap[r]
                    for e in range(ex[w][r]):
                        nc.tensor.matmul(ps[:], gt[:, exbase + e, :],
                                         we[:, wec * WIN:(wec + 1) * WIN],
                                         start=first, stop=False)
                        first = False
                        wec += 1
                    ncols = cap[w][r] // 128
                    for j in range(ncols):
                        o = off[w][r][j]
                        stop = (r == last_r and j == ncols - 1)
                        nc.tensor.matmul(ps[:, o:o + SWC], gt[:, regbase + j, :],
                                         wr[:, wrc * SWC:(wrc + 1) * SWC],
                                         start=first, stop=stop)
                        first = False
                        wrc += 1
                rcol += wregc
                ecol += wexc
                ot = op.tile([64, WIN], F16, tag="ot")
                nc.scalar.copy(ot[:], ps[:])
                nc.sync.dma_start(out[:, w * WIN:(w + 1) * WIN], ot[:])
    return nc


# ================================================================ host packing
def _pack_edges(core, pos, src, val, nwin):
    """Pack edges (dest position pos within core, source node src) into the
    per-(window, range) gather/weight layout. Returns per-core arrays + meta."""
    w = pos // WIN
    drel = (pos - w * WIN).astype(np.int64)
    rg = np.searchsorted(RS, src, side="right") - 1
    src_rel = (src - RS[rg]).astype(np.int64)
    order = np.lexsort((drel, rg, w, core))
    core, w, drel, rg, src_rel, val = (a[order] for a in (core, w, drel, rg, src_rel, val))

    key = (core * nwin + w) * NRANGE + rg
    cnt = np.bincount(key, minlength=NCORE * nwin * NRANGE).reshape(NCORE, nwin, NRANGE)
    cap = ((cnt.max(axis=0) + 127) // 128) * 128          # [nwin, NRANGE]
    cap = np.maximum(cap, 128)                            # every cell >= 1 col

    n = len(key)
    ar = np.arange(n)
    first = np.empty(n, bool); first[0] = True; first[1:] = key[1:] != key[:-1]
    slot = ar - np.maximum.accumulate(np.where(first, ar, 0))

    ncol = cap >> 7
    regcol_base = np.concatenate([[0], np.cumsum(ncol.reshape(-1))])[:-1]\
        .reshape(nwin, NRANGE)
    totregcol = int(ncol.sum())

    # data-driven column offsets: cover the across-core [min, max] dest range
    colj = slot >> 7
    gcol = regcol_base[w, rg] + colj
    lo = np.full(totregcol, WIN, np.int64)
    hi = np.full(totregcol, -1, np.int64)
    np.minimum.at(lo, gcol, drel)
    np.maximum.at(hi, gcol, drel)
    lo = np.minimum(lo, hi)                               # empty cols -> [hi,hi]
    offcol = np.clip((lo + hi + 1 - SWC) // 2, 0, WIN - SWC)
    off_e = offcol[gcol]
    spill = (drel < off_e) | (drel >= off_e + SWC)

    # extra column counts (same for all cores)
    skey = key[spill]
    scnt = np.bincount(skey, minlength=NCORE * nwin * NRANGE).reshape(NCORE, nwin, NRANGE)
    exc = (scnt.max(axis=0) + 127) // 128                 # [nwin, NRANGE] cols
    exc[:, 0] = np.maximum(exc[:, 0], 1)                  # psum initializer

    # per-(w,r) stream slot count and bases (same all cores)
    cell_slots = cap + 128 * exc                          # [nwin, NRANGE]
    cell_base = np.concatenate([[0], np.cumsum(cell_slots.reshape(-1))])[:-1]\
        .reshape(nwin, NRANGE)                            # base within core stream
    tot = int(cell_slots.sum())
    excol_base = np.concatenate([[0], np.cumsum(exc.reshape(-1))])[:-1]\
        .reshape(nwin, NRANGE)
    totexcol = int(exc.sum())

    idx_flat = np.zeros((NCORE, tot), np.int16)
    wreg = np.zeros((NCORE, 128, totregcol * SWC), np.float16)
    wext = np.zeros((NCORE, 128, max(totexcol, 1) * WIN), np.float16)

    # regular slots: stream position = cell_base + 128*exc (extras first) + slot
    spos = cell_base[w, rg] + 128 * exc[w, rg] + slot
    idx_flat[core, spos] = src_rel.astype(np.int16)
    reg = ~spill
    fw = (core[reg] * 128 + (slot[reg] & 127)) * (totregcol * SWC) \
        + (regcol_base[w[reg], rg[reg]] + colj[reg]) * SWC + (drel[reg] - off_e[reg])
    wreg.reshape(-1)[fw] = val[reg]

    # spilled edges -> extra slots (their regular slot stays as weight-0 pad)
    if spill.any():
        sc, sw_, srg, ssrc, sdrel, sval = (a[spill] for a in (core, w, rg, src_rel, drel, val))
        ns = len(sc)
        ars = np.arange(ns)
        sfirst = np.empty(ns, bool); sfirst[0] = True; sfirst[1:] = skey[1:] != skey[:-1]
        eslot = ars - np.maximum.accumulate(np.where(sfirst, ars, 0))
        espos = cell_base[sw_, srg] + eslot
        idx_flat[sc, espos] = ssrc.astype(np.int16)
        fx = (sc * 128 + (eslot & 127)) * (max(totexcol, 1) * WIN) \
            + (excol_base[sw_, srg] + (eslot >> 7)) * WIN + sdrel
        wext.reshape(-1)[fx] = sval

    # idx stream -> [128, tot/16] wrapped+replicated layout
    idx_arr = np.tile(idx_flat.reshape(NCORE, tot // 16, 16).transpose(0, 2, 1),
                      (1, 8, 1))

    off_tab = [[list(int(offcol[regcol_base[w_, r_] + j_])
                     for j_ in range(ncol[w_][r_]))
                for r_ in range(NRANGE)] for w_ in range(nwin)]
    meta = {"nwin": nwin,
            "cap": [[int(cap[w_][r_]) for r_ in range(NRANGE)] for w_ in range(nwin)],
            "ex": [[int(exc[w_][r_]) for r_ in range(NRANGE)] for w_ in range(nwin)],
            "off": off_tab}
    return idx_arr, wreg, wext, meta


def _edges_for_lists(rows, cols, vals, dlist):
    """Expand: for each core and each listed dest (position p in dlist[c]),
    all incoming edges. Returns (core, pos, src, val). dlist entries >= N are
    edgeless sentinels."""
    order0 = np.argsort(rows, kind="stable")
    rs, cs, vs = rows[order0], cols[order0], vals[order0]
    row_start = np.searchsorted(rs, np.arange(N + 1))
    rr = dlist.reshape(-1).astype(np.int64)
    rrc = np.minimum(rr, N)
    cnt = row_start[np.minimum(rrc + 1, N)] - row_start[rrc]
    cnt[rr >= N] = 0
    ent = np.repeat(np.arange(len(rr)), cnt)
    ofs = np.arange(len(ent)) - np.repeat(np.cumsum(cnt) - cnt, cnt)
    srcidx = row_start[rrc][ent] + ofs
    ndst = dlist.shape[1]
    e_core = ent // ndst
    e_pos = ent - e_core * ndst
    return e_core, e_pos, cs[srcidx], vs[srcidx]


def _prep(inputs):
    p = {}
    users = np.asarray(inputs["users"]);   items = np.asarray(inputs["items"])
    adj = np.asarray(inputs["adj_matrix"])
    rows = np.asarray(inputs["graph_rows"]).astype(np.int64)
    cols = np.asarray(inputs["graph_cols"]).astype(np.int64)
    vals = np.asarray(inputs["graph_vals"]).astype(np.float32)
    W_att = np.asarray(inputs["W_att"]); a_att = np.asarray(inputs["a_att"])
    v1 = W_att @ a_att[:HID, 0]; v2 = W_att @ a_att[HID:, 0]

    p["xu"] = np.ascontiguousarray(np.asarray(inputs["user_semantic_emb"]).astype(np.float16).T)
    p["xi"] = np.ascontiguousarray(np.asarray(inputs["semantic_emb"]).astype(np.float16).T)
    p["wu"] = np.asarray(inputs["W_usem"]).astype(np.float16)
    p["wi"] = np.concatenate([np.asarray(inputs["W_sem"]), v1[:, None], v2[:, None]],
                             axis=1).astype(np.float16)
    p["bu"] = np.asarray(inputs["b_usem"]).reshape(64, 1)
    p["bi"] = np.concatenate([np.asarray(inputs["b_sem"]), np.zeros(2, np.float32)]
                             ).reshape(66, 1).astype(np.float32)
    p["eu"] = np.ascontiguousarray(np.asarray(inputs["emb_user"]).T)
    p["ei"] = np.ascontiguousarray(np.asarray(inputs["emb_item"]).T)

    # K2 gather stream: col = quad, partition p = (item%4)*32 + k
    IPAD2 = QTOT * 4
    adj_pad = np.zeros((NCORE, IPAD2, K), np.int64)
    for c in range(NCORE):
        adj_pad[c, :IPC] = adj[c * IPC:(c + 1) * IPC]
    colv = np.arange(QTOT)
    pv = np.arange(128)
    item_cp = colv[:, None] * 4 + pv[None, :] // 32          # [QTOT, 128]
    k_cp = pv[None, :] % 32
    flat = adj_pad[:, item_cp, k_cp].reshape(NCORE, QTOT * 128)
    p["adji"] = np.tile(flat.reshape(NCORE, -1, 16).transpose(0, 2, 1),
                        (1, 8, 1)).astype(np.int16)
    p["item_cp"] = item_cp

    # layer 1: all NPAD dests
    e_core = rows // DPC
    e_pos = rows - e_core * DPC
    p["l1"] = _pack_edges(e_core, e_pos, cols, vals, NWIN1)

    # batch dest list (layer 3 + K4)
    dlist = np.stack([np.concatenate([
        users[c * BPC:(c + 1) * BPC].astype(np.int64),
        items[c * BPC:(c + 1) * BPC].astype(np.int64) + NUM_USERS])
        for c in range(NCORE)])
    NWIN3 = (2 * BPC) // WIN                       # 2
    ec, ep, es, ev = _edges_for_lists(rows, cols, vals, dlist)
    p["l3"] = _pack_edges(ec, ep, es, ev, NWIN3)
    p["nwin3"] = NWIN3

    # layer 2 rows needed: sources of layer-3 edges + K4's rows
    need = np.zeros(N, bool)
    need[es] = True
    need[dlist.reshape(-1)] = True
    R2 = np.nonzero(need)[0]
    Lc = -(-len(R2) // NCORE)
    L2 = -(-Lc // WIN) * WIN
    NWIN2 = L2 // WIN
    lists2 = np.full((NCORE, L2), N, np.int64)
    for c in range(NCORE):
        seg = R2[c * Lc:(c + 1) * Lc]
        lists2[c, :len(seg)] = seg
    p["lists2"] = lists2
    ec, ep, es2, ev2 = _edges_for_lists(rows, cols, vals, lists2)
    p["l2"] = _pack_edges(ec, ep, es2, ev2, NWIN2)
    p["nwin2"] = NWIN2

    p["k4"] = []
    for c in range(NCORE):
        u = users[c * BPC:(c + 1) * BPC].astype(np.int64)
        it = items[c * BPC:(c + 1) * BPC].astype(np.int64) + NUM_USERS
        rws = np.concatenate([u, it])
        p["k4"].append(np.ascontiguousarray(rws.reshape(8, 128).T.astype(np.int32)))
    return p


# ================================================================ K4: final
def _build_k4():
    nc = bass.Bass("TRN2", target_bir_lowering=False)
    tb012 = nc.dram_tensor("tb012", [NPAD, 192], F16, kind="ExternalInput")
    tb3d = nc.dram_tensor("tb3d", [128, 8 * 64], F16, kind="ExternalInput")
    fidx = nc.dram_tensor("fidx", [128, 8], I32, kind="ExternalInput")
    out = nc.dram_tensor("out", [128, 4], F32, kind="ExternalOutput")

    with TileContext(nc) as tc:
        with tc.tile_pool(name="g", bufs=2) as gp, \
             tc.tile_pool(name="m", bufs=1) as mp:
            it = mp.tile([128, 8], I32, tag="it")
            nc.sync.dma_start(it[:], fidx[:])
            t3 = mp.tile([128, 8, 64], F16, tag="t3")
            nc.sync.dma_start(t3[:].rearrange("p a b -> p (a b)"), tb3d[:])
            acc = mp.tile([128, 8, 64], F32, tag="acc")
            nc.scalar.copy(acc[:], t3[:])
            g = gp.tile([128, 8, 192], F16, tag="g")
            for t in range(8):
                nc.gpsimd.indirect_dma_start(
                    out=g[:, t, :], out_offset=None, in_=tb012[:],
                    in_offset=bass.IndirectOffsetOnAxis(ap=it[:, t:t + 1], axis=0))
            for sl in range(3):
                gf = gp.tile([128, 8, 64], F32, tag="gf")
                nc.scalar.copy(gf[:], g[:].rearrange("p a (s b) -> p a s b", s=3)[:, :, sl, :])
                nc.vector.tensor_add(acc[:], acc[:], gf[:])
            nc.scalar.mul(acc[:], acc[:], 0.25)
            prod = mp.tile([128, 4, 64], F32, tag="prod")
            nc.vector.tensor_mul(prod[:], acc[:, 0:4, :], acc[:, 4:8, :])
            res = mp.tile([128, 4], F32, tag="res")
            nc.vector.reduce_sum(res[:], prod[:], axis=mybir.AxisListType.X)
            nc.sync.dma_start(out[:], res[:])
    return nc


_META = {}

def _run(name, builder, in_maps, meta_key=None):
    if name not in _BUILT or _META.get(name) != meta_key:
        nc = builder()
        mybir.codegen_inst_isa_subclasses(nc)
        _BUILT[name] = nc
        _META[name] = meta_key
    return bass_utils.run_bass_kernel_spmd(
        _BUILT[name], in_maps, core_ids=list(range(NCORE))).results


def _tables_from_nodes(node_tbl):
    """node_tbl [NPAD, 64] f16 -> 3 range tables [rows, 128] f16."""
    full = np.zeros((NPAD, TSTride), np.float16)
    full[:, 0:64] = node_tbl
    return [np.ascontiguousarray(full[RS[r]:RS[r + 1]]) for r in range(NRANGE)]


def kernel(**inputs):
    p = _prep(inputs)

    # ---------------- K1
    maps = [{
        "xu": p["xu"][:, c * UPC:(c + 1) * UPC],
        "xi": p["xi"][:, c * IPC:(c + 1) * IPC],
        "wu": p["wu"], "wi": p["wi"], "bu": p["bu"], "bi": p["bi"],
        "eu": p["eu"][:, c * UPC:(c + 1) * UPC],
        "ei": p["ei"][:, c * IPC:(c + 1) * IPC],
    } for c in range(NCORE)]
    r1 = _run("k1", _build_k1, maps)
    users_m = np.concatenate([r1[c]["ou"].T for c in range(NCORE)], 0)
    items_m = np.concatenate([r1[c]["oi"].T for c in range(NCORE)], 0)
    s1 = np.concatenate([r1[c]["os12"][0] for c in range(NCORE)])
    s2 = np.concatenate([r1[c]["os12"][1] for c in range(NCORE)])

    # ---------------- K2
    tblA = np.zeros((NUM_ITEMS, TSTride), np.float16)
    tblA[:, 0:64] = items_m
    tblA[:, 64] = s1
    IPAD2 = QTOT * 4
    m16 = np.zeros((128, 4), np.float16)
    for j in range(4):
        m16[j * 32:(j + 1) * 32, j] = 1.0
    m32 = m16.astype(np.float32)
    mt32 = np.ascontiguousarray(m32.T)
    item_cp = p["item_cp"]
    maps = []
    for c in range(NCORE):
        s2c = np.zeros(IPAD2, np.float32)
        s2c[:IPC] = s2[c * IPC:(c + 1) * IPC]
        s2qc = np.ascontiguousarray(s2c[item_cp].T)          # [128, QTOT]
        imc = np.zeros((IPAD2, 64), np.float32)
        imc[:IPC] = items_m[c * IPC:(c + 1) * IPC]
        maps.append({"tbl": tblA, "adjq": p["adji"][c],
                     "s2q": s2qc,
                     "itmT": np.ascontiguousarray(imc.T),
                     "m16": m16, "m32": m32, "mt32": mt32})
    r2 = _run("k2", _build_k2, maps)
    items_f = np.zeros((NUM_ITEMS, 64), np.float16)
    for c in range(NCORE):
        items_f[c * IPC:(c + 1) * IPC] = r2[c]["oit"].T[:IPC]

    # ---------------- K3 layers
    def run_spmm(name, pack, tbl_nodes):
        idx_arr, wreg, wext, meta = pack
        tbls = _tables_from_nodes(tbl_nodes)
        maps = [dict({f"tbl{r}": tbls[r] for r in range(NRANGE)},
                     idx=idx_arr[c], wreg=wreg[c], wext=wext[c])
                for c in range(NCORE)]
        mk = (meta["nwin"], tuple(map(tuple, meta["cap"])), tuple(map(tuple, meta["ex"])))
        r = _run(name, lambda: _build_k3(meta), maps, meta_key=mk)
        return r

    e0 = np.zeros((NPAD, 64), np.float16)
    e0[:NUM_USERS] = users_m.astype(np.float16)
    e0[NUM_USERS:N] = items_f

    r = run_spmm("k3", p["l1"], e0)
    e1 = np.zeros((NPAD, 64), np.float16)
    for c in range(NCORE):
        e1[c * DPC:(c + 1) * DPC] = r[c]["out"].T

    r = run_spmm("k3b", p["l2"], e1)
    e2 = np.zeros((NPAD, 64), np.float16)
    lists2 = p["lists2"]
    for c in range(NCORE):
        outc = r[c]["out"].T
        sel = lists2[c] < N
        e2[lists2[c][sel]] = outc[:len(lists2[c])][sel]

    r = run_spmm("k3c", p["l3"], e2)
    tb3d = []
    for c in range(NCORE):
        rowsc = r[c]["out"].T[:2 * BPC]                  # [1024, 64]
        tb3d.append(np.ascontiguousarray(
            rowsc.reshape(8, 128, 64).transpose(1, 0, 2).reshape(128, 8 * 64)))

    # ---------------- K4
    e012 = np.ascontiguousarray(np.concatenate([e0, e1, e2], axis=1))
    maps = [{"tb012": e012, "tb3d": tb3d[c], "fidx": p["k4"][c]}
            for c in range(NCORE)]
    r4 = _run("k4", _build_k4, maps)
    gamma = np.zeros(B, np.float32)
    for c in range(NCORE):
        gamma[c * BPC:(c + 1) * BPC] = r4[c]["out"].T.reshape(BPC)
    return gamma


# revision 18
# speedup vs baseline: 4.5211x; 1.0042x over previous
"""CoLaKG model kernel for 8 Trainium2 NeuronCores (self-contained).

Pipeline (6 bass SPMD launches; host does static prep + mechanical relayout):
  K1 gemm : semantic projections (users+items) + merge, + s1/s2 GAT scalars
  K2 attn : item-neighbor GAT attention (batched dma_gather + softmax + wsum)
  K3 spmm : LightGCN propagation layer 1 (all dests)
  K3b     : layer 2 at the ~87% of rows consumed downstream (list-addressed)
  K3c     : layer 3 at the batch's 2*512 rows per core
  K4 final: gather 4 embedding tables at (user,item) rows, mean, dot product

Perf design: all row gathers use the batched InstDMAGatherAnt (Q7 'mlp'
ucode library) at <=896 indices per instruction, amortizing the ~1us
SWDGE fixed cost ~7x vs per-column indirect DMA. Node tables are stored
at 256B stride with a 128B gathered payload (int16 indices -> 3 table
ranges). The SpMM packs edges per (512-dest window, range) sorted by
dest; each 128-edge column feeds one matmul into a fixed 64-wide psum
span (drift-tolerant placement; stragglers go to window-wide extra
columns; psum initialized by the first wide matmul's start flag).
"""
import copy
import numpy as np

import jax
jax.config.update("jax_compilation_cache_dir", "/tmp/.jax_bass_cache")
jax.config.update("jax_persistent_cache_min_entry_size_bytes", -1)
jax.config.update("jax_persistent_cache_min_compile_time_secs", 0.0)

import concourse.bass as bass
import concourse.mybir as mybir
from concourse.tile import TileContext
from concourse import bass_utils, library_config
import concourse.tile as tile_mod
from concourse.vector_clock import ScopedClock

F32 = mybir.dt.float32
F16 = mybir.dt.float16
F8 = mybir.dt.float8e4
I16 = mybir.dt.int16
I32 = mybir.dt.int32
AF = mybir.ActivationFunctionType

# ---------------------------------------------------------------- tile patch
MAX_WAITS = 1

def _split_sync_waits(nc, max_waits=MAX_WAITS):
    template = None
    counter = [0]
    for fn in nc.m.functions:
        for bb in fn.blocks:
            for inst in bb.instructions:
                if type(inst).__name__ == "InstNoOp":
                    template = copy.deepcopy(inst)
                    break
            if template is not None:
                break
        if template is not None:
            break
    for fn in nc.m.functions:
        for bb in fn.blocks:
            il = bb.instructions
            i = 0
            while i < len(il):
                inst = il[i]
                if template is None and type(inst).__name__ == "InstNoOp":
                    template = copy.deepcopy(inst)
                si = inst.sync_info
                if si is not None and si.on_wait is not None and len(si.on_wait) > max_waits:
                    assert template is not None, "no InstNoOp to clone"
                    waits = list(si.on_wait)
                    keep, rest = waits[:max_waits], waits[max_waits:]
                    si.on_wait.clear()
                    for w in keep:
                        si.on_wait.append(w)
                    carriers = []
                    while rest:
                        c = copy.deepcopy(template)
                        counter[0] += 1
                        c.name = f"I-waitsplit-{counter[0]}"
                        c.engine = inst.engine
                        c.sync_info = mybir.SyncInfo(on_wait=list(rest[:max_waits]), on_update=[])
                        carriers.append(c)
                        rest = rest[max_waits:]
                    for k, cinst in enumerate(carriers):
                        try:
                            nc.register_instruction(cinst, overwrite=True)
                        except Exception:
                            pass
                        il.insert(i + k, cinst)
                    i += len(carriers)
                i += 1

def _patched_drain_and_barrier(self, tick_clock, wait_clock):
    nc = self.nc
    nop0 = nc.sync.nop(nofuse=True, hint="predrain_waits")
    wait_clock.add_sem_waits(nop0.ins, ScopedClock({None: tick_clock.global_clock}))
    nc.sync.drain()
    nc.all_engine_barrier()
    assert self.sems is not None
    popped = nc._tile_sem_poison_stack.pop()
    assert popped is self._sem_poison
    nc.clear_and_free_semaphores(list(self.sems.allocated().values()))
    nc.all_engine_barrier()
    _split_sync_waits(nc)

tile_mod.TileContext._drain_and_barrier = _patched_drain_and_barrier

# ---------------------------------------------------------------- constants
NUM_USERS, NUM_ITEMS, D, SEM, HID, K = 60000, 30000, 64, 1024, 32, 32
N = NUM_USERS + NUM_ITEMS             # 90000
NPAD = 90112                          # 704*128
NCORE = 8
DPC = NPAD // NCORE                   # 11264 dest rows per core
WIN = 512                             # dests per window (one psum bank f32)
NWIN1 = DPC // WIN                    # 22 windows, layer-1
SWC = 64                              # psum span per regular matmul column
NRANGE = 3
RS = np.array([0, 32768, 65536, NPAD], dtype=np.int64)   # table range bounds
RROWS = [32768, 32768, 24576]
NI_MAX = 896                          # idx per gather instruction (ring cap)
TSTride = 128                         # table row stride (f16 elems) = 256B

UPC = NUM_USERS // NCORE              # 7500
IPC = NUM_ITEMS // NCORE              # 3750
IPAD = 3840
NBLK = IPAD // 128                    # 30
SUBB = 6                              # item blocks per attention sub-batch
NSUB = NBLK // SUBB                   # 10
GC = SUBB * K                         # 96 gather cols per K2 sub-batch
B = 4096
BPC = B // NCORE                      # 512

_BUILT = {}


def _dma_gather(g, out_ap, in_ap, idxs_ap, num_idxs, elem_size, elem_step,
                regs=None):
    """dma_gather with payload < stride (bass's public API asserts
    elem_size%256B which is only a stride requirement). regs: dict caching
    one GPSIMD register per distinct num_idxs value."""
    _in_ap = g.lower_ap_dma(in_ap, for_custom_bir_dma=True)
    _idxs_ap = g.lower_ap(idxs_ap)
    _out_ap = g.lower_ap(out_ap)
    if regs is None:
        reg = g.to_reg(num_idxs)
    else:
        if num_idxs not in regs:
            regs[num_idxs] = g.to_reg(num_idxs)
        reg = regs[num_idxs]
    stride_bytes = elem_step * mybir.dt.size(in_ap.dtype)
    assert stride_bytes % 256 == 0
    return g.add_instruction(
        mybir.InstDMAGatherAnt(
            name=g.bass.get_next_instruction_name(),
            ins=[*_in_ap, _idxs_ap, g.lower_val_access(reg)],
            outs=[_out_ap],
            transpose=False, num_idxs=num_idxs, elem_size=elem_size,
            stride_bytes_256=stride_bytes // 256, gen_mode=0,
            single_packet=True, queue_num=0,
            sbuf_tokens_per_rank=0, sbuf_free_dim_per_rank=0,
            sbuf_free_dim_pad_per_rank=0, sbuf_byte_offset=0,
        ))


def _elu(nc, pool, out_ap, in_ap, shape, tag):
    """out = elu(in) = max(x,0) + exp(min(x,0)) - 1   (no Elu in ACT table)."""
    mn = pool.tile(shape, F32, tag=tag + "_mn")
    nc.vector.tensor_scalar_min(mn[:], in_ap, 0.0)
    ex = pool.tile(shape, F32, tag=tag + "_ex")
    nc.scalar.activation(ex[:], mn[:], AF.Exp, scale=1.0)
    mx = pool.tile(shape, F32, tag=tag + "_mx")
    nc.vector.tensor_scalar_max(mx[:], in_ap, 0.0)
    nc.vector.tensor_add(out_ap, mx[:], ex[:])
    nc.vector.tensor_scalar_add(out_ap, out_ap, -1.0)


# ================================================================ K1: GEMM
def _build_k1():
    nc = bass.Bass("TRN2", target_bir_lowering=False)
    xu = nc.dram_tensor("xu", [SEM, UPC], F16, kind="ExternalInput")
    xi = nc.dram_tensor("xi", [SEM, IPC], F16, kind="ExternalInput")
    wu = nc.dram_tensor("wu", [SEM, 64], F16, kind="ExternalInput")
    wi = nc.dram_tensor("wi", [SEM, 66], F16, kind="ExternalInput")
    bu = nc.dram_tensor("bu", [64, 1], F32, kind="ExternalInput")
    bi = nc.dram_tensor("bi", [66, 1], F32, kind="ExternalInput")
    eu = nc.dram_tensor("eu", [64, UPC], F32, kind="ExternalInput")
    ei = nc.dram_tensor("ei", [64, IPC], F32, kind="ExternalInput")
    ou = nc.dram_tensor("ou", [64, UPC], F32, kind="ExternalOutput")
    oi = nc.dram_tensor("oi", [64, IPC], F32, kind="ExternalOutput")
    os12 = nc.dram_tensor("os12", [2, IPC], F32, kind="ExternalOutput")

    with TileContext(nc) as tc:
        with tc.tile_pool(name="w", bufs=1) as wp, \
             tc.tile_pool(name="x", bufs=3) as xp, \
             tc.tile_pool(name="o", bufs=2) as op, \
             tc.tile_pool(name="ps", bufs=2, space="PSUM") as pp:
            wu_sb = wp.tile([128, SEM // 128, 64], F16, tag="wu")
            nc.sync.dma_start(wu_sb[:], wu[:].rearrange("(a p) m -> p a m", p=128))
            wi_sb = wp.tile([128, SEM // 128, 66], F16, tag="wi")
            nc.sync.dma_start(wi_sb[:], wi[:].rearrange("(a p) m -> p a m", p=128))
            bu_sb = wp.tile([64, 1], F32, tag="bu")
            nc.sync.dma_start(bu_sb[:], bu[:])
            bi_sb = wp.tile([66, 1], F32, tag="bi")
            nc.sync.dma_start(bi_sb[:], bi[:])

            def gemm(xten, eten, wtile, btile, oten, m, rows, RL, RM, s12=None,
                     xdt=F16, descale=1.0):
                # RL: DMA load tile; RM: matmul tile (psum bank limit 512 f32)
                for t in range(rows // RL):
                    xt = xp.tile([128, SEM // 128, RL], xdt, tag="xt")
                    nc.sync.dma_start(
                        xt[:], xten[:, t * RL:(t + 1) * RL].rearrange("(a p) r -> p a r", p=128))
                    et = op.tile([64, RL], F32, tag="et")
                    nc.sync.dma_start(et[:], eten[:, t * RL:(t + 1) * RL])
                    mg = op.tile([64, RL], F32, tag="mg")
                    if s12 is not None:
                        sv = op.tile([2, RL], F32, tag="sv")
                    else:
                        sv = None
                    for q in range(RL // RM):
                        ps = pp.tile([m, RM], F32, tag="ps")
                        for kk in range(SEM // 128):
                            nc.tensor.matmul(ps[:], wtile[:, kk, :],
                                             xt[:, kk, q * RM:(q + 1) * RM],
                                             start=(kk == 0), stop=(kk == SEM // 128 - 1))
                        xb = op.tile([64, RM], F32, tag="xb")
                        if descale != 1.0:
                            nc.vector.tensor_scalar(xb[:], ps[0:64, :], descale,
                                                    btile[0:64, :],
                                                    mybir.AluOpType.mult,
                                                    mybir.AluOpType.add)
                        else:
                            nc.vector.tensor_scalar_add(xb[:], ps[0:64, :], btile[0:64, :])
                        _elu(nc, op, mg[:, q * RM:(q + 1) * RM], xb[:], [64, RM], "e1")
                        if s12 is not None:
                            nc.scalar.copy(sv[:, q * RM:(q + 1) * RM], ps[64:66, :])
                    nc.vector.tensor_add(mg[:], mg[:], et[:])
                    nc.scalar.mul(mg[:], mg[:], 0.5)
                    nc.sync.dma_start(oten[:, t * RL:(t + 1) * RL], mg[:])
                    if s12 is not None:
                        nc.sync.dma_start(s12[:, t * RL:(t + 1) * RL], sv[:])

            gemm(xu, eu, wu_sb, bu_sb, ou, 64, UPC, 1500, 500)
            gemm(xi, ei, wi_sb, bi_sb, oi, 66, IPC, 750, 375, s12=os12)
    return nc


# ================================================================ K2: attention
# Item table rows: 128 f16 (256B): [emb 0:64 | s1 @64 | pad]. Payload 66.
# Gather layout: column = quad of 4 items, partition p = (item%4)*32 + k.
# Weighted sum via PE: per quad, matmul(stationary=gathered [128,64],
# moving=masked unnormalized exp weights [128,4]) -> psum [64, 4 items];
# softmax normalization folded in by smearing 1/Z over partitions with a
# second matmul. Output is dim-major [64, items].
PAY2 = 66
QTOT = 1024                           # quads per core (4096 item slots)
NGRP = 8                              # psum groups of 128 quads (512 items)
QG = 128                              # quads per group

def _build_k2():
    nc = bass.Bass("TRN2", target_bir_lowering=False)
    tbl = nc.dram_tensor("tbl", [NUM_ITEMS, TSTride], F16, kind="ExternalInput")
    adjq = nc.dram_tensor("adjq", [128, (QTOT * 128) // 16], I16, kind="ExternalInput")
    s2q = nc.dram_tensor("s2q", [128, QTOT], F32, kind="ExternalInput")
    itmT = nc.dram_tensor("itmT", [64, NGRP * 512], F32, kind="ExternalInput")
    m16 = nc.dram_tensor("m16", [128, 4], F16, kind="ExternalInput")
    m32 = nc.dram_tensor("m32", [128, 4], F32, kind="ExternalInput")
    mt32 = nc.dram_tensor("mt32", [4, 128], F32, kind="ExternalInput")
    oit = nc.dram_tensor("oit", [64, NGRP * 512], F16, kind="ExternalOutput")

    with TileContext(nc) as tc:
        with tc.tile_pool(name="g", bufs=3) as gp, \
             tc.tile_pool(name="s", bufs=1) as sp, \
             tc.tile_pool(name="m", bufs=2) as mp, \
             tc.tile_pool(name="ps", bufs=2, space="PSUM") as pp:
            nc.gpsimd.load_library(library_config.mlp)
            adj_sb = sp.tile([128, (QTOT * 128) // 16], I16, tag="adj")
            for uu in range(NGRP):
                a0 = uu * (QG * 128) // 16
                a1 = (uu + 1) * (QG * 128) // 16
                nc.sync.dma_start(adj_sb[:, a0:a1], adjq[:, a0:a1])
            s2_sb = sp.tile([128, QTOT], F32, tag="s2")
            nc.sync.dma_start(s2_sb[:], s2q[:])
            m16_sb = sp.tile([128, 4], F16, tag="m16")
            nc.sync.dma_start(m16_sb[:], m16[:])
            m32_sb = sp.tile([128, 4], F32, tag="m32")
            nc.sync.dma_start(m32_sb[:], m32[:])
            mt32_sb = sp.tile([4, 128], F32, tag="mt32")
            nc.sync.dma_start(mt32_sb[:], mt32[:])
            niregs = {}
            for u in range(NGRP):
                g = gp.tile([128, QG, PAY2], F16, tag="g")
                base16 = u * (QG * 128) // 16
                col = 0
                left = QG * 128
                while left > 0:
                    n = min(NI_MAX, left)
                    _dma_gather(nc.gpsimd, g[:, col:col + n // 128, :], tbl[:],
                                adj_sb[:, base16:base16 + n // 16], n, PAY2, TSTride,
                                regs=niregs)
                    base16 += n // 16
                    col += n // 128
                    left -= n
                # attention logits + leaky relu + exp (unnormalized)
                lg = mp.tile([128, QG], F32, tag="lg")
                nc.vector.tensor_add(lg[:], g[:, :, 64], s2_sb[:, u * QG:(u + 1) * QG])
                lr = mp.tile([128, QG], F32, tag="lr")
                nc.scalar.mul(lr[:], lg[:], 0.2)
                nc.vector.tensor_max(lg[:], lg[:], lr[:])
                ex32 = mp.tile([128, QG], F32, tag="ex32")
                nc.scalar.activation(ex32[:], lg[:], AF.Exp, scale=1.0)
                # Z per (item-in-quad j, quad) then smear 1/Z over partitions
                psZ = pp.tile([4, 512], F32, tag="psZ")
                nc.tensor.matmul(psZ[:, 0:QG], m32_sb[:], ex32[:],
                                 start=True, stop=True)
                rz = mp.tile([4, QG], F32, tag="rz")
                nc.vector.reciprocal(rz[:], psZ[:, 0:QG])
                psR = pp.tile([128, 512], F32, tag="psR")
                nc.tensor.matmul(psR[:, 0:QG], mt32_sb[:], rz[:],
                                 start=True, stop=True)
                att = mp.tile([128, QG], F16, tag="att")
                nc.vector.tensor_mul(att[:], ex32[:], psR[:, 0:QG])
                # expand to masked moving blocks [128, QG, 4]
                attm = mp.tile([128, QG, 4], F16, tag="attm")
                av = att[:]
                att_b = bass.AP(av.tensor, av.offset, list(av.ap) + [[0, 4]])
                mv = m16_sb[:]
                m_b = bass.AP(mv.tensor, mv.offset,
                              [mv.ap[0], [0, QG], mv.ap[1]])
                nc.vector.tensor_mul(attm[:], att_b, m_b)
                # weighted sum: one matmul per quad into [64, 512] psum
                psH = pp.tile([64, 512], F32, tag="psH")
                for q in range(QG):
                    nc.tensor.matmul(psH[:, 4 * q:4 * q + 4], g[:, q, 0:64],
                                     attm[:, q, :],
                                     start=(q == 0), stop=(q == QG - 1))
                hT = mp.tile([64, 512], F32, tag="hT")
                nc.scalar.copy(hT[:], psH[:])
                he = mp.tile([64, 512], F32, tag="he")
                _elu(nc, mp, he[:], hT[:], [64, 512], "e2")
                it = mp.tile([64, 512], F32, tag="it")
                nc.sync.dma_start(it[:], itmT[:, u * 512:(u + 1) * 512])
                nc.vector.tensor_add(he[:], he[:], it[:])
                fo16 = mp.tile([64, 512], F16, tag="fo16")
                nc.scalar.mul(fo16[:], he[:], 0.5)
                nc.sync.dma_start(oit[:, u * 512:(u + 1) * 512], fo16[:])
    return nc


# ================================================================ K3: spmm
def _build_k3(meta):
    """SpMM layer kernel from packing metadata.

    meta: nwin, cap[w][r] (regular slots), ex[w][r] (extra cols),
          off[w][r] = list of per-column psum offsets.
    Stream layout per (w, r): [extra cols | regular cols]; per window the
    first extra of r0 initializes psum (start=True); last regular matmul
    of the last nonempty range carries stop=True.
    """
    nwin = meta["nwin"]
    cap = meta["cap"]; ex = meta["ex"]; off = meta["off"]
    totslots = int(sum(cap[w][r] + 128 * ex[w][r]
                       for w in range(nwin) for r in range(NRANGE)))
    totregcol = int(sum(cap[w][r] // 128 for w in range(nwin) for r in range(NRANGE)))
    totexcol = int(sum(ex[w][r] for w in range(nwin) for r in range(NRANGE)))
    maxwcol = max(sum(cap[w][r] // 128 + ex[w][r] for r in range(NRANGE))
                  for w in range(nwin))
    maxwreg = max(sum(cap[w][r] // 128 for r in range(NRANGE)) for w in range(nwin))
    maxwex = max(sum(ex[w][r] for r in range(NRANGE)) for w in range(nwin))

    nc = bass.Bass("TRN2", target_bir_lowering=False)
    tbls = [nc.dram_tensor(f"tbl{r}", [RROWS[r], TSTride], F16, kind="ExternalInput")
            for r in range(NRANGE)]
    idx = nc.dram_tensor("idx", [128, totslots // 16], I16, kind="ExternalInput")
    wreg = nc.dram_tensor("wreg", [128, totregcol * SWC], F16, kind="ExternalInput")
    wext = nc.dram_tensor("wext", [128, max(totexcol, 1) * WIN], F16, kind="ExternalInput")
    out = nc.dram_tensor("out", [64, nwin * WIN], F16, kind="ExternalOutput")

    with TileContext(nc) as tc:
        with tc.tile_pool(name="s", bufs=1) as sp, \
             tc.tile_pool(name="g", bufs=3) as gp, \
             tc.tile_pool(name="w", bufs=3) as wp, \
             tc.tile_pool(name="o", bufs=3) as op, \
             tc.tile_pool(name="ps", bufs=4, space="PSUM") as pp:
            nc.gpsimd.load_library(library_config.mlp)
            idx_sb = sp.tile([128, totslots // 16], I16, tag="idx")
            niregs = {}
            i16 = 0          # cursor into idx (units of 16 slots)
            rcol = 0         # cursor into wreg (regular col index)
            ecol = 0         # cursor into wext (extra col index)
            for w in range(nwin):
                wsl = sum(cap[w][r] + 128 * ex[w][r] for r in range(NRANGE)) // 16
                nc.sync.dma_start(idx_sb[:, i16:i16 + wsl], idx[:, i16:i16 + wsl])
                wcols = sum(cap[w][r] // 128 + ex[w][r] for r in range(NRANGE))
                wregc = sum(cap[w][r] // 128 for r in range(NRANGE))
                wexc = sum(ex[w][r] for r in range(NRANGE))
                gt = gp.tile([128, maxwcol, SWC], F16, tag="g")
                wr = wp.tile([128, maxwreg * SWC], F16, tag="wr")
                nc.sync.dma_start(wr[:, 0:wregc * SWC],
                                  wreg[:, rcol * SWC:(rcol + wregc) * SWC])
                if maxwex:
                    we = wp.tile([128, max(maxwex, 1) * WIN], F16, tag="we")
                    if wexc:
                        nc.sync.dma_start(we[:, 0:wexc * WIN],
                                          wext[:, ecol * WIN:(ecol + wexc) * WIN])
                # gathers for the whole window (extras first per range)
                col = 0
                colmap = []   # per range: (excolbase, regcolbase)
                for r in range(NRANGE):
                    nsl = cap[w][r] + 128 * ex[w][r]
                    colmap.append((col, col + ex[w][r]))
                    left = nsl
                    while left > 0:
                        n = min(NI_MAX, left)
                        _dma_gather(nc.gpsimd, gt[:, col:col + n // 128, :], tbls[r][:],
                                    idx_sb[:, i16:i16 + n // 16], n, SWC, TSTride,
                                    regs=niregs)
                        i16 += n // 16
                        col += n // 128
                        left -= n
                # matmuls
                ps = pp.tile([64, WIN], F32, tag="ps")
                first = True
                wrc = 0
                wec = 0
                last_r = max(r for r in range(NRANGE) if cap[w][r] > 0)
                for r in range(NRANGE):
                    exbase, regbase = colmap[r]
                    for e in range(ex[w][r]):
                        nc.tensor.matmul(ps[:], gt[:, exbase + e, :],
                                         we[:, wec * WIN:(wec + 1) * WIN],
                                         start=first, stop=False)
                        first = False
                        wec += 1
                    ncols = cap[w][r] // 128
                    for j in range(ncols):
                        o = off[w][r][j]
                        stop = (r == last_r and j == ncols - 1)
                        nc.tensor.matmul(ps[:, o:o + SWC], gt[:, regbase + j, :],
                                         wr[:, wrc * SWC:(wrc + 1) * SWC],
                                         start=first, stop=stop)
                        first = False
                        wrc += 1
                rcol += wregc
                ecol += wexc
                ot = op.tile([64, WIN], F16, tag="ot")
                nc.scalar.copy(ot[:], ps[:])
                nc.sync.dma_start(out[:, w * WIN:(w + 1) * WIN], ot[:])
    return nc


# ================================================================ host packing
def _pack_edges(core, pos, src, val, nwin):
    """Pack edges (dest position pos within core, source node src) into the
    per-(window, range) gather/weight layout. Returns per-core arrays + meta."""
    w = pos // WIN
    drel = (pos - w * WIN).astype(np.int64)
    rg = np.searchsorted(RS, src, side="right") - 1
    src_rel = (src - RS[rg]).astype(np.int64)
    order = np.lexsort((drel, rg, w, core))
    core, w, drel, rg, src_rel, val = (a[order] for a in (core, w, drel, rg, src_rel, val))

    key = (core * nwin + w) * NRANGE + rg
    cnt = np.bincount(key, minlength=NCORE * nwin * NRANGE).reshape(NCORE, nwin, NRANGE)
    cap = ((cnt.max(axis=0) + 127) // 128) * 128          # [nwin, NRANGE]
    cap = np.maximum(cap, 128)                            # every cell >= 1 col

    n = len(key)
    ar = np.arange(n)
    first = np.empty(n, bool); first[0] = True; first[1:] = key[1:] != key[:-1]
    slot = ar - np.maximum.accumulate(np.where(first, ar, 0))

    ncol = cap >> 7
    regcol_base = np.concatenate([[0], np.cumsum(ncol.reshape(-1))])[:-1]\
        .reshape(nwin, NRANGE)
    totregcol = int(ncol.sum())

    # data-driven column offsets: cover the across-core [min, max] dest range
    colj = slot >> 7
    gcol = regcol_base[w, rg] + colj
    lo = np.full(totregcol, WIN, np.int64)
    hi = np.full(totregcol, -1, np.int64)
    np.minimum.at(lo, gcol, drel)
    np.maximum.at(hi, gcol, drel)
    lo = np.minimum(lo, hi)                               # empty cols -> [hi,hi]
    offcol = np.clip((lo + hi + 1 - SWC) // 2, 0, WIN - SWC)
    off_e = offcol[gcol]
    spill = (drel < off_e) | (drel >= off_e + SWC)

    # extra column counts (same for all cores)
    skey = key[spill]
    scnt = np.bincount(skey, minlength=NCORE * nwin * NRANGE).reshape(NCORE, nwin, NRANGE)
    exc = (scnt.max(axis=0) + 127) // 128                 # [nwin, NRANGE] cols
    exc[:, 0] = np.maximum(exc[:, 0], 1)                  # psum initializer

    # per-(w,r) stream slot count and bases (same all cores)
    cell_slots = cap + 128 * exc                          # [nwin, NRANGE]
    cell_base = np.concatenate([[0], np.cumsum(cell_slots.reshape(-1))])[:-1]\
        .reshape(nwin, NRANGE)                            # base within core stream
    tot = int(cell_slots.sum())
    excol_base = np.concatenate([[0], np.cumsum(exc.reshape(-1))])[:-1]\
        .reshape(nwin, NRANGE)
    totexcol = int(exc.sum())

    idx_flat = np.zeros((NCORE, tot), np.int16)
    wreg = np.zeros((NCORE, 128, totregcol * SWC), np.float16)
    wext = np.zeros((NCORE, 128, max(totexcol, 1) * WIN), np.float16)

    # regular slots: stream position = cell_base + 128*exc (extras first) + slot
    spos = cell_base[w, rg] + 128 * exc[w, rg] + slot
    idx_flat[core, spos] = src_rel.astype(np.int16)
    reg = ~spill
    fw = (core[reg] * 128 + (slot[reg] & 127)) * (totregcol * SWC) \
        + (regcol_base[w[reg], rg[reg]] + colj[reg]) * SWC + (drel[reg] - off_e[reg])
    wreg.reshape(-1)[fw] = val[reg]

    # spilled edges -> extra slots (their regular slot stays as weight-0 pad)
    if spill.any():
        sc, sw_, srg, ssrc, sdrel, sval = (a[spill] for a in (core, w, rg, src_rel, drel, val))
        ns = len(sc)
        ars = np.arange(ns)
        sfirst = np.empty(ns, bool); sfirst[0] = True; sfirst[1:] = skey[1:] != skey[:-1]
        eslot = ars - np.maximum.accumulate(np.where(sfirst, ars, 0))
        espos = cell_base[sw_, srg] + eslot
        idx_flat[sc, espos] = ssrc.astype(np.int16)
        fx = (sc * 128 + (eslot & 127)) * (max(totexcol, 1) * WIN) \
            + (excol_base[sw_, srg] + (eslot >> 7)) * WIN + sdrel
        wext.reshape(-1)[fx] = sval

    # idx stream -> [128, tot/16] wrapped+replicated layout
    idx_arr = np.tile(idx_flat.reshape(NCORE, tot // 16, 16).transpose(0, 2, 1),
                      (1, 8, 1))

    off_tab = [[list(int(offcol[regcol_base[w_, r_] + j_])
                     for j_ in range(ncol[w_][r_]))
                for r_ in range(NRANGE)] for w_ in range(nwin)]
    meta = {"nwin": nwin,
            "cap": [[int(cap[w_][r_]) for r_ in range(NRANGE)] for w_ in range(nwin)],
            "ex": [[int(exc[w_][r_]) for r_ in range(NRANGE)] for w_ in range(nwin)],
            "off": off_tab}
    return idx_arr, wreg, wext, meta


def _edges_for_lists(rows, cols, vals, dlist):
    """Expand: for each core and each listed dest (position p in dlist[c]),
    all incoming edges. Returns (core, pos, src, val). dlist entries >= N are
    edgeless sentinels."""
    order0 = np.argsort(rows, kind="stable")
    rs, cs, vs = rows[order0], cols[order0], vals[order0]
    row_start = np.searchsorted(rs, np.arange(N + 1))
    rr = dlist.reshape(-1).astype(np.int64)
    rrc = np.minimum(rr, N)
    cnt = row_start[np.minimum(rrc + 1, N)] - row_start[rrc]
    cnt[rr >= N] = 0
    ent = np.repeat(np.arange(len(rr)), cnt)
    ofs = np.arange(len(ent)) - np.repeat(np.cumsum(cnt) - cnt, cnt)
    srcidx = row_start[rrc][ent] + ofs
    ndst = dlist.shape[1]
    e_core = ent // ndst
    e_pos = ent - e_core * ndst
    return e_core, e_pos, cs[srcidx], vs[srcidx]


def _prep(inputs):
    p = {}
    users = np.asarray(inputs["users"]);   items = np.asarray(inputs["items"])
    adj = np.asarray(inputs["adj_matrix"])
    rows = np.asarray(inputs["graph_rows"]).astype(np.int64)
    cols = np.asarray(inputs["graph_cols"]).astype(np.int64)
    vals = np.asarray(inputs["graph_vals"]).astype(np.float32)
    W_att = np.asarray(inputs["W_att"]); a_att = np.asarray(inputs["a_att"])
    v1 = W_att @ a_att[:HID, 0]; v2 = W_att @ a_att[HID:, 0]

    p["xu"] = np.ascontiguousarray(np.asarray(inputs["user_semantic_emb"]).astype(np.float16).T)
    p["xi"] = np.ascontiguousarray(np.asarray(inputs["semantic_emb"]).astype(np.float16).T)
    p["wu"] = np.asarray(inputs["W_usem"]).astype(np.float16)
    p["wi"] = np.concatenate([np.asarray(inputs["W_sem"]), v1[:, None], v2[:, None]],
                             axis=1).astype(np.float16)
    p["bu"] = np.asarray(inputs["b_usem"]).reshape(64, 1)
    p["bi"] = np.concatenate([np.asarray(inputs["b_sem"]), np.zeros(2, np.float32)]
                             ).reshape(66, 1).astype(np.float32)
    p["eu"] = np.ascontiguousarray(np.asarray(inputs["emb_user"]).T)
    p["ei"] = np.ascontiguousarray(np.asarray(inputs["emb_item"]).T)

    # K2 gather stream: col = quad, partition p = (item%4)*32 + k
    IPAD2 = QTOT * 4
    adj_pad = np.zeros((NCORE, IPAD2, K), np.int64)
    for c in range(NCORE):
        adj_pad[c, :IPC] = adj[c * IPC:(c + 1) * IPC]
    colv = np.arange(QTOT)
    pv = np.arange(128)
    item_cp = colv[:, None] * 4 + pv[None, :] // 32          # [QTOT, 128]
    k_cp = pv[None, :] % 32
    flat = adj_pad[:, item_cp, k_cp].reshape(NCORE, QTOT * 128)
    p["adji"] = np.tile(flat.reshape(NCORE, -1, 16).transpose(0, 2, 1),
                        (1, 8, 1)).astype(np.int16)
    p["item_cp"] = item_cp

    # layer 1: all NPAD dests
    e_core = rows // DPC
    e_pos = rows - e_core * DPC
    p["l1"] = _pack_edges(e_core, e_pos, cols, vals, NWIN1)

    # batch dest list (layer 3 + K4)
    dlist = np.stack([np.concatenate([
        users[c * BPC:(c + 1) * BPC].astype(np.int64),
        items[c * BPC:(c + 1) * BPC].astype(np.int64) + NUM_USERS])
        for c in range(NCORE)])
    NWIN3 = (2 * BPC) // WIN                       # 2
    ec, ep, es, ev = _edges_for_lists(rows, cols, vals, dlist)
    p["l3"] = _pack_edges(ec, ep, es, ev, NWIN3)
    p["nwin3"] = NWIN3

    # layer 2 rows needed: sources of layer-3 edges + K4's rows
    need = np.zeros(N, bool)
    need[es] = True
    need[dlist.reshape(-1)] = True
    R2 = np.nonzero(need)[0]
    Lc = -(-len(R2) // NCORE)
    L2 = -(-Lc // WIN) * WIN
    NWIN2 = L2 // WIN
    lists2 = np.full((NCORE, L2), N, np.int64)
    for c in range(NCORE):
        seg = R2[c * Lc:(c + 1) * Lc]
        lists2[c, :len(seg)] = seg
    p["lists2"] = lists2
    ec, ep, es2, ev2 = _edges_for_lists(rows, cols, vals, lists2)
    p["l2"] = _pack_edges(ec, ep, es2, ev2, NWIN2)
    p["nwin2"] = NWIN2

    p["k4"] = []
    for c in range(NCORE):
        u = users[c * BPC:(c + 1) * BPC].astype(np.int64)
        it = items[c * BPC:(c + 1) * BPC].astype(np.int64) + NUM_USERS
        rws = np.concatenate([u, it])
        p["k4"].append(np.ascontiguousarray(rws.reshape(8, 128).T.astype(np.int32)))
    return p


# ================================================================ K4: final
def _build_k4():
    nc = bass.Bass("TRN2", target_bir_lowering=False)
    tb012 = nc.dram_tensor("tb012", [NPAD, 192], F16, kind="ExternalInput")
    tb3d = nc.dram_tensor("tb3d", [128, 8 * 64], F16, kind="ExternalInput")
    fidx = nc.dram_tensor("fidx", [128, 8], I32, kind="ExternalInput")
    out = nc.dram_tensor("out", [128, 4], F32, kind="ExternalOutput")

    with TileContext(nc) as tc:
        with tc.tile_pool(name="g", bufs=2) as gp, \
             tc.tile_pool(name="m", bufs=1) as mp:
            it = mp.tile([128, 8], I32, tag="it")
            nc.sync.dma_start(it[:], fidx[:])
            t3 = mp.tile([128, 8, 64], F16, tag="t3")
            nc.sync.dma_start(t3[:].rearrange("p a b -> p (a b)"), tb3d[:])
            acc = mp.tile([128, 8, 64], F32, tag="acc")
            nc.scalar.copy(acc[:], t3[:])
            g = gp.tile([128, 8, 192], F16, tag="g")
            for t in range(8):
                nc.gpsimd.indirect_dma_start(
                    out=g[:, t, :], out_offset=None, in_=tb012[:],
                    in_offset=bass.IndirectOffsetOnAxis(ap=it[:, t:t + 1], axis=0))
            for sl in range(3):
                gf = gp.tile([128, 8, 64], F32, tag="gf")
                nc.scalar.copy(gf[:], g[:].rearrange("p a (s b) -> p a s b", s=3)[:, :, sl, :])
                nc.vector.tensor_add(acc[:], acc[:], gf[:])
            nc.scalar.mul(acc[:], acc[:], 0.25)
            prod = mp.tile([128, 4, 64], F32, tag="prod")
            nc.vector.tensor_mul(prod[:], acc[:, 0:4, :], acc[:, 4:8, :])
            res = mp.tile([128, 4], F32, tag="res")
            nc.vector.reduce_sum(res[:], prod[:], axis=mybir.AxisListType.X)
            nc.sync.dma_start(out[:], res[:])
    return nc


_META = {}

def _run(name, builder, in_maps, meta_key=None):
    if name not in _BUILT or _META.get(name) != meta_key:
        nc = builder()
        mybir.codegen_inst_isa_subclasses(nc)
        _BUILT[name] = nc
        _META[name] = meta_key
    return bass_utils.run_bass_kernel_spmd(
        _BUILT[name], in_maps, core_ids=list(range(NCORE))).results


def _tables_from_nodes(node_tbl):
    """node_tbl [NPAD, 64] f16 -> 3 range tables [rows, 128] f16."""
    full = np.zeros((NPAD, TSTride), np.float16)
    full[:, 0:64] = node_tbl
    return [np.ascontiguousarray(full[RS[r]:RS[r + 1]]) for r in range(NRANGE)]


def kernel(**inputs):
    p = _prep(inputs)

    # ---------------- K1
    maps = [{
        "xu": p["xu"][:, c * UPC:(c + 1) * UPC],
        "xi": p["xi"][:, c * IPC:(c + 1) * IPC],
        "wu": p["wu"], "wi": p["wi"], "bu": p["bu"], "bi": p["bi"],
        "eu": p["eu"][:, c * UPC:(c + 1) * UPC],
        "ei": p["ei"][:, c * IPC:(c + 1) * IPC],
    } for c in range(NCORE)]
    r1 = _run("k1", _build_k1, maps)
    users_m = np.concatenate([r1[c]["ou"].T for c in range(NCORE)], 0)
    items_m = np.concatenate([r1[c]["oi"].T for c in range(NCORE)], 0)
    s1 = np.concatenate([r1[c]["os12"][0] for c in range(NCORE)])
    s2 = np.concatenate([r1[c]["os12"][1] for c in range(NCORE)])

    # ---------------- K2
    tblA = np.zeros((NUM_ITEMS, TSTride), np.float16)
    tblA[:, 0:64] = items_m
    tblA[:, 64] = s1
    IPAD2 = QTOT * 4
    m16 = np.zeros((128, 4), np.float16)
    for j in range(4):
        m16[j * 32:(j + 1) * 32, j] = 1.0
    m32 = m16.astype(np.float32)
    mt32 = np.ascontiguousarray(m32.T)
    item_cp = p["item_cp"]
    maps = []
    for c in range(NCORE):
        s2c = np.zeros(IPAD2, np.float32)
        s2c[:IPC] = s2[c * IPC:(c + 1) * IPC]
        s2qc = np.ascontiguousarray(s2c[item_cp].T)          # [128, QTOT]
        imc = np.zeros((IPAD2, 64), np.float32)
        imc[:IPC] = items_m[c * IPC:(c + 1) * IPC]
        maps.append({"tbl": tblA, "adjq": p["adji"][c],
                     "s2q": s2qc,
                     "itmT": np.ascontiguousarray(imc.T),
                     "m16": m16, "m32": m32, "mt32": mt32})
    r2 = _run("k2", _build_k2, maps)
    items_f = np.zeros((NUM_ITEMS, 64), np.float16)
    for c in range(NCORE):
        items_f[c * IPC:(c + 1) * IPC] = r2[c]["oit"].T[:IPC]

    # ---------------- K3 layers
    def run_spmm(name, pack, tbl_nodes):
        idx_arr, wreg, wext, meta = pack
        tbls = _tables_from_nodes(tbl_nodes)
        maps = [dict({f"tbl{r}": tbls[r] for r in range(NRANGE)},
                     idx=idx_arr[c], wreg=wreg[c], wext=wext[c])
                for c in range(NCORE)]
        mk = (meta["nwin"], tuple(map(tuple, meta["cap"])), tuple(map(tuple, meta["ex"])))
        r = _run(name, lambda: _build_k3(meta), maps, meta_key=mk)
        return r

    e0 = np.zeros((NPAD, 64), np.float16)
    e0[:NUM_USERS] = users_m.astype(np.float16)
    e0[NUM_USERS:N] = items_f

    r = run_spmm("k3", p["l1"], e0)
    e1 = np.zeros((NPAD, 64), np.float16)
    for c in range(NCORE):
        e1[c * DPC:(c + 1) * DPC] = r[c]["out"].T

    r = run_spmm("k3b", p["l2"], e1)
    e2 = np.zeros((NPAD, 64), np.float16)
    lists2 = p["lists2"]
    for c in range(NCORE):
        outc = r[c]["out"].T
        sel = lists2[c] < N
        e2[lists2[c][sel]] = outc[:len(lists2[c])][sel]

    r = run_spmm("k3c", p["l3"], e2)
    tb3d = []
    for c in range(NCORE):
        rowsc = r[c]["out"].T[:2 * BPC]                  # [1024, 64]
        tb3d.append(np.ascontiguousarray(
            rowsc.reshape(8, 128, 64).transpose(1, 0, 2).reshape(128, 8 * 64)))

    # ---------------- K4
    e012 = np.ascontiguousarray(np.concatenate([e0, e1, e2], axis=1))
    maps = [{"tb012": e012, "tb3d": tb3d[c], "fidx": p["k4"][c]}
            for c in range(NCORE)]
    r4 = _run("k4", _build_k4, maps)
    gamma = np.zeros(B, np.float32)
    for c in range(NCORE):
        gamma[c * BPC:(c + 1) * BPC] = r4[c]["out"].T.reshape(BPC)
    return gamma
